# revision 13
# baseline (speedup 1.0000x reference)
"""Trainium2 Bass kernel for nn_Attention_13700945674736 (sparse local-window attention).

Strategy (8 NeuronCores, data-parallel over batch, 4 samples/core):
  - Permute the sequence axis s = 64*i + j  ->  s' = 16*j + i (image transpose).
    The 7x11 local window becomes a 1-D band |ds'| <= 83, so each 128-query
    tile only attends to 3 aligned 128-key chunks instead of 1024.
  - Exact per-chunk band is [128c-80, 128c+208): pairs at distance 81..83
    need |dj|=5 AND |di|>=1 simultaneously, which the 2-D mask forbids for
    the first/last 3 keys of a chunk, so halfwidth 80 suffices on the low
    side (and symmetrically 79+128 on the high side; 208 = 128+80).
  - Heads are padded to 64-partition slots (host-padded weights) so every
    engine access pattern starts at a 32-aligned partition.
  - All matmul operands are bf16; PSUM accumulation and softmax reductions
    stay fp32.
  - attnT[k, q] band tiles; exp on ScalarE (3 chunk-groups per head -> few,
    large activations); binary window mask applied multiplicatively, split
    between DVE and GPSIMD(Pool) for engine balance; @V uses
    lhsT=[V|0|ones|0] so softmax denominators land at partitions 64:112 of
    the same PSUM tile; the projection bias is folded in via a constant-1
    row of aoT.
  - po is a single persistent 2-bank PSUM tile; q-tile t of every head uses
    column (t%2)*512+(t//2)*128 so concurrently-open accumulation groups
    alternate banks.  Closed tiles keep their data across same-bank
    start=True (only has_written bits are cleared), so the per-head divides
    run in 2 halves while the next head's @V already accumulates.
  - PSUM budget: pat 2x[128,1024] (4 banks) + proj 2x[128,512] (2 banks)
    + po [128,1024] (2 banks) = 8 banks exactly.
  - Evacuation copies (Q/K/V/out PSUM->SBUF) are split between ScalarE and
    DVE by static schedule tables; output stores go through the SP queue so
    DMA setup does not stall the Activation sequencer.
"""

import sys

sys.path.insert(0, "/opt/trn_rl_repo")

import numpy as np

import concourse.bass as bass
from concourse import bacc
import concourse.mybir as mybir
import concourse.tile as tile
from concourse.bass_utils import run_bass_kernel_spmd

# ---------------------------------------------------------------- constants
B, S, C = 32, 1024, 384
H, D = 8, 48
HI, WI = 16, 64
N_CORES = 8
BL = B // N_CORES  # samples per core
SCALE = float(D) ** -0.5
F32 = mybir.dt.float32
BF16 = mybir.dt.bfloat16
PD = BF16  # precision of expT / m01 / vv

# s' = 16*j + i  <->  s = 64*i + j ;  PERM[s'] = s
_sp = np.arange(S)
PERM = (_sp % HI) * WI + (_sp // HI)

NQT = S // 128  # 8 query tiles (and key chunks)
WPADQ = 64 * H  # padded Q (and K) section width: 512
WQW = 2 * WPADQ + C  # 1408

# exact per-chunk bands: key-chunk c attends to queries [QLO[c], QHI[c])
QLO = [max(0, 128 * c - 80) for c in range(NQT)]
QHI = [min(S, 128 * c + 208) for c in range(NQT)]
WC = [QHI[c] - QLO[c] for c in range(NQT)]
OFFC = list(np.cumsum([0] + WC[:-1]))
BAND_W = sum(WC)  # 2144

# chunk groups per PSUM pat tile (each group width <= 1024 f32 = 2 banks)
CH_GROUPS = [(0, 1, 2), (3, 4, 5), (6, 7)]
GRP_BASE = [OFFC[g[0]] for g in CH_GROUPS]
GRP_W = [OFFC[g[-1]] + WC[g[-1]] - OFFC[g[0]] for g in CH_GROUPS]
assert max(GRP_W) <= 1024

# po column slot for q-tile t (identical for every head; open groups t,t+1
# always land in different PSUM banks)
PO_COL = [(t % 2) * 512 + (t // 2) * 128 for t in range(NQT)]

# ------------------------------------------------- engine schedule tables
# 'a' = ScalarE(Act) copy, 'd' = DVE tensor_copy
QK_EVAC = "aadaaadaaadaaada"  # 16 per sample (Q then K, pair-major)
V_EVAC = "aadaaada"  # 8 per sample
OUT_EVAC = "aadaaada"  # 8 per sample
# 'd' = DVE, 'p' = Pool(GPSIMD) for the 3 mask multiplies of each head.
# g1 (chunks 3-5) sits mid-head where its latency hides; g0/g2 gate the
# @V start/tail chains, so they stay on the fast DVE.
MASK_ENG = ["dpd"] * H
# divide granularity: q-tile ranges; later ranges close later, and finer
# tail pieces release the po columns the next head's early @V chunks need.
DIV_RANGES = [(0, 4), (4, 6), (6, 8)]

# ---------------------------------------------------------------- bass program
_CACHE = {}


def _build():
    if "nc" in _CACHE:
        return _CACHE["nc"]

    nc = bacc.Bacc(None, target_bir_lowering=False)
    xT_d = nc.declare_dram_parameter("xT", [BL, C, S], BF16, isOutput=False)
    wq_d = nc.declare_dram_parameter("wq_pad", [C, WQW], BF16, isOutput=False)
    wp_d = nc.declare_dram_parameter("wp_pad", [4, 128, C], BF16, isOutput=False)
    ones_d = nc.declare_dram_parameter("ones_row", [1, S], BF16, isOutput=False)
    m_d = nc.declare_dram_parameter("m01", [128, BAND_W], PD, isOutput=False)
    out_d = nc.declare_dram_parameter("out", [BL, S, C], F32, isOutput=True)

    with tile.TileContext(nc) as tc:
        with (
            tc.tile_pool(name="singles", bufs=1) as singles,
            tc.tile_pool(name="xt_pool", bufs=3) as xt_pool,
            tc.tile_pool(name="out_pool", bufs=4) as out_pool,
            tc.tile_pool(name="ps_proj", bufs=2, space="PSUM") as ps_proj,
            tc.tile_pool(name="ps_pat", bufs=2, space="PSUM") as ps_pat,
            tc.tile_pool(name="ps_po", bufs=1, space="PSUM") as ps_po,
        ):
            # ---- constants.  Q/K/V weight sections load as separate DMAs so
            # the first projection matmuls only wait for their own section.
            w_sb = singles.tile([128, 3, WQW], BF16)
            wq_v = wq_d.rearrange("(c p) w -> p c w", p=128)
            nc.scalar.dma_start(w_sb[:, :, 0:WPADQ], wq_v[:, :, 0:WPADQ])
            nc.scalar.dma_start(
                w_sb[:, :, WPADQ : 2 * WPADQ], wq_v[:, :, WPADQ : 2 * WPADQ]
            )
            nc.scalar.dma_start(w_sb[:, :, 2 * WPADQ :], wq_v[:, :, 2 * WPADQ :])
            wp_sb = singles.tile([128, 4, C], BF16)
            nc.gpsimd.dma_start(wp_sb[:, :, :], wp_d.rearrange("f p c -> p f c"))
            m_sb = singles.tile([128, BAND_W], PD)
            nc.gpsimd.dma_start(m_sb, m_d[:, :])

            # persistent attention-output PSUM tile (2 banks)
            po = ps_po.tile([128, S], F32, tag="po")

            # ---- per-sample tiles, double-buffered for cross-sample overlap
            qTs, kTs, vvs, aoTs, expTs, dens = [], [], [], [], [], []
            for i in range(2):
                qTs.append(singles.tile([128, 4, S], BF16, name=f"qT{i}"))
                kTs.append(singles.tile([128, 4, S], BF16, name=f"kT{i}"))
                vvs.append(singles.tile([128, NQT, H, 128], PD, name=f"vv{i}"))
                aoTs.append(singles.tile([128, 4, S], BF16, name=f"aoT{i}"))
                expTs.append(singles.tile([128, BAND_W], PD, name=f"expT{i}"))
                dens.append(singles.tile([48, 2, 512], F32, name=f"den{i}"))
            def setup_set(i):
                """One-time constant sections of buffer set i (Pool engine).
                Set 1 is deferred until after proj(0) is issued so sample-0
                mask multiplies are not queued behind 20us of memsets."""
                vv, aoT = vvs[i], aoTs[i]
                nc.gpsimd.memset(vv[:, :, :, D : D + 16], 0.0)
                nc.gpsimd.memset(vv[:, :, :, D + 16 : 112], 1.0)
                nc.gpsimd.memset(vv[:, :, :, 112:128], 0.0)
                # zero dead rows (48:64, 112:128); starts must be 32-aligned so
                # cover 32:64 / 96:128 — live rows are rewritten by the divides.
                nc.gpsimd.memset(aoT[32:64, :, :], 0.0)
                nc.gpsimd.memset(aoT[96:128, :, :], 0.0)
                # constant-1 row: proj picks up b_proj from wp_pad[0][48]
                nc.gpsimd.dma_start(aoT[48:49, 0, :], ones_d[:, :])

            def evac(engine, dst, src):
                if engine == "a":
                    nc.scalar.copy(dst, src)
                else:
                    nc.vector.tensor_copy(dst, src)

            def load_xt(b):
                xt = xt_pool.tile([128, 3, S], BF16)
                nc.sync.dma_start(
                    xt[:, :, :], xT_d[b].rearrange("(c p) s -> p c s", p=128)
                )
                return xt

            def proj(b, xt):
                """QKV projection for sample b into qT/kT/vv set b%2."""
                qT, kT, vv = qTs[b % 2], kTs[b % 2], vvs[b % 2]
                ei = iter(QK_EVAC)
                for qk in range(2):
                    dst = qT if qk == 0 else kT
                    for pair in range(4):
                        ncol = qk * WPADQ + pair * 128
                        for half in range(2):
                            ps = ps_proj.tile([128, 512], F32, tag="mm")
                            for ci in range(3):
                                nc.tensor.matmul(
                                    ps[:, :],
                                    w_sb[:, ci, ncol : ncol + 128],
                                    xt[:, ci, half * 512 : (half + 1) * 512],
                                    start=(ci == 0),
                                    stop=(ci == 2),
                                )
                            seg = dst[:, pair, half * 512 : (half + 1) * 512]
                            evac(next(ei), seg, ps[:, :])
                vi = iter(V_EVAC)
                for st in range(NQT):
                    psv = ps_proj.tile([128, 512], F32, tag="mm")
                    for ci in range(3):
                        nc.tensor.matmul(
                            psv[:, 0:C],
                            xt[:, ci, st * 128 : (st + 1) * 128],
                            w_sb[:, ci, 2 * WPADQ : 2 * WPADQ + C],
                            start=(ci == 0),
                            stop=(ci == 2),
                        )
                    evac(
                        next(vi),
                        vv[:, st, :, 0:D],
                        psv[:, 0:C].rearrange("p (h d) -> p h d", h=H),
                    )

            def attention(b):
                qT, kT, vv, aoT = qTs[b % 2], kTs[b % 2], vvs[b % 2], aoTs[b % 2]
                for h in range(H):
                    pair, sub = divmod(h, 2)
                    p0 = sub * 64
                    expT = expTs[h % 2]
                    den_sb = dens[h % 2]
                    # ---- QK^T band + exp (3 chunk-groups); masks for g0/g1
                    # issue here, g2's mask issues later so DVE's in-order
                    # queue does not park the divides behind it.
                    def mask_mul(gi):
                        gbase, gw = GRP_BASE[gi], GRP_W[gi]
                        tt = (
                            nc.vector.tensor_tensor
                            if MASK_ENG[h][gi] == "d"
                            else nc.gpsimd.tensor_tensor
                        )
                        tt(
                            expT[:, gbase : gbase + gw],
                            expT[:, gbase : gbase + gw],
                            m_sb[:, gbase : gbase + gw],
                            mybir.AluOpType.mult,
                        )

                    for gi, grp in enumerate(CH_GROUPS):
                        gbase, gw = GRP_BASE[gi], GRP_W[gi]
                        pat = ps_pat.tile([128, 1024], F32, tag="attn")
                        for c in grp:
                            lo = OFFC[c] - gbase
                            hi = lo + WC[c]
                            a = lo
                            while a < hi:
                                b2 = min(hi, (a // 512 + 1) * 512)
                                nc.tensor.matmul(
                                    pat[:, a:b2],
                                    kT[p0 : p0 + D, pair, c * 128 : (c + 1) * 128],
                                    qT[
                                        p0 : p0 + D,
                                        pair,
                                        QLO[c] + (a - lo) : QLO[c] + (b2 - lo),
                                    ],
                                    start=True,
                                    stop=True,
                                )
                                a = b2
                        nc.scalar.activation(
                            expT[:, gbase : gbase + gw],
                            pat[:, 0:gw],
                            mybir.ActivationFunctionType.Exp,
                            scale=SCALE,
                        )
                        if gi < 2:
                            mask_mul(gi)

                    # ---- @V with ones rows at 64:112 -> denominators,
                    # interleaved with the per-range normalizes so divides
                    # start the moment their q-tiles close.
                    po_q = po[:, :].rearrange(
                        "p (tlo thi u) -> p thi tlo u", tlo=2, thi=4, u=128
                    )
                    den_flat = den_sb[:, :, :].rearrange("p a b -> p (a b)")

                    def av_chunk(c):
                        lhsT = vv[:, c, h, :]
                        for t in range(max(c - 1, 0), min(c + 2, NQT)):
                            pc = PO_COL[t]
                            qs = max(128 * t, QLO[c])
                            qe = min(128 * t + 128, QHI[c])
                            nc.tensor.matmul(
                                po[:, pc + (qs - 128 * t) : pc + (qe - 128 * t)],
                                lhsT,
                                expT[:, OFFC[c] + (qs - QLO[c]) : OFFC[c] + (qe - QLO[c])],
                                start=(c == max(t - 1, 0)),
                                stop=(c == min(t + 1, NQT - 1)),
                            )

                    def divide(ta, tb):
                        nt = (tb - ta) // 2
                        den_v = den_flat[:, ta * 128 : tb * 128].rearrange(
                            "p (thi tlo u) -> p thi tlo u", thi=nt, tlo=2, u=128
                        )
                        ao_v = aoT[
                            p0 : p0 + D, pair, ta * 128 : tb * 128
                        ].rearrange("p (thi tlo u) -> p thi tlo u", thi=nt, tlo=2, u=128)
                        nc.vector.reciprocal(
                            den_v, po_q[64 : 64 + D, ta // 2 : tb // 2]
                        )
                        nc.vector.tensor_tensor(
                            ao_v,
                            po_q[0:D, ta // 2 : tb // 2],
                            den_v,
                            mybir.AluOpType.mult,
                        )

                    for c in range(5):
                        av_chunk(c)
                    divide(*DIV_RANGES[0])  # tiles 0-3 closed at chunk 4
                    mask_mul(2)
                    av_chunk(5)
                    av_chunk(6)
                    divide(*DIV_RANGES[1])  # tiles 4-5 closed at chunk 6
                    av_chunk(7)
                    divide(*DIV_RANGES[2])

            def out_proj(b):
                aoT = aoTs[b % 2]
                oi = iter(OUT_EVAC)
                for sp in range(NQT // 2):  # store pairs of q-tiles
                    ot = out_pool.tile([128, 2, C], F32)
                    for k in range(2):
                        st = 2 * sp + k
                        psp = ps_proj.tile([128, 512], F32, tag="mm")
                        for p in range(4):
                            nc.tensor.matmul(
                                psp[:, 0:C],
                                aoT[:, p, st * 128 : (st + 1) * 128],
                                wp_sb[:, p, :],
                                start=(p == 0),
                                stop=(p == 3),
                            )
                        evac(next(oi), ot[:, k, :], psp[:, 0:C])
                    nc.sync.dma_start(
                        out_d[b, 2 * sp * 128 : (2 * sp + 2) * 128, :].rearrange(
                            "(k p) c -> p k c", k=2
                        ),
                        ot[:, :, :],
                    )

            # ---------------- main pipeline
            setup_set(0)
            xts = {0: load_xt(0)}
            proj(0, xts[0])
            setup_set(1)
            for b in range(BL):
                if b + 1 < BL:
                    xts[b + 1] = load_xt(b + 1)  # prefetch during attention(b)
                attention(b)
                if b + 1 < BL:
                    proj(b + 1, xts[b + 1])
                out_proj(b)

    nc.finalize()
    _CACHE["nc"] = nc
    return nc


# ---------------------------------------------------------------- host wrapper
def _np_bf16(a):
    import ml_dtypes

    return np.asarray(a, dtype=ml_dtypes.bfloat16)


def _build_m01(mask):
    """[128, BAND_W] banded 0/1 mask in exact-band layout (rows = key within
    chunk c, cols = q in [QLO[c], QHI[c]))."""
    mp = np.asarray(mask)[np.ix_(PERM, PERM)]
    good = np.isfinite(mp) & (mp == 0.0)
    m01 = np.zeros((128, BAND_W), np.float32)
    covered = 0
    for c in range(NQT):
        blk = good[QLO[c] : QHI[c], c * 128 : (c + 1) * 128]  # [q, k]
        m01[:, OFFC[c] : OFFC[c] + WC[c]] = blk.T.astype(np.float32)
        covered += int(blk.sum())
    assert covered == int(good.sum()), "mask not covered by band layout"
    return m01


def _pad_wqkv(w_qkv):
    """[384, 1152] -> [384, 1408]: Q/K head h at cols h*64..h*64+48 (zero pad),
    V kept natural at cols 1024:1408."""
    out = np.zeros((C, WQW), np.float32)
    for sec in range(2):  # Q, K
        for h in range(H):
            out[:, sec * WPADQ + h * 64 : sec * WPADQ + h * 64 + D] = w_qkv[
                :, sec * C + h * D : sec * C + (h + 1) * D
            ]
    out[:, 2 * WPADQ :] = w_qkv[:, 2 * C :]
    return out


def _pad_wproj(w_proj, b_proj):
    """[384, 384] -> [4, 128, 384]: pair p rows 0:48 = head 2p, 64:112 = head 2p+1.
    Row 48 of pair 0 carries b_proj (matched by the constant-1 row in aoT)."""
    out = np.zeros((4, 128, C), np.float32)
    for p in range(4):
        out[p, 0:D] = w_proj[(2 * p) * D : (2 * p + 1) * D]
        out[p, 64 : 64 + D] = w_proj[(2 * p + 1) * D : (2 * p + 2) * D]
    out[0, D] = b_proj
    return out


def kernel(x, w_qkv, w_proj, b_proj, mask):
    x = np.asarray(x, np.float32)
    w_qkv = np.asarray(w_qkv, np.float32)
    w_proj = np.asarray(w_proj, np.float32)
    b_proj = np.asarray(b_proj, np.float32)

    nc = _build()

    xT = _np_bf16(np.ascontiguousarray(x[:, PERM, :].transpose(0, 2, 1)))  # [B, C, S']
    wq_pad = _np_bf16(_pad_wqkv(w_qkv))
    wp_pad = _np_bf16(_pad_wproj(w_proj, b_proj))
    ones_row = _np_bf16(np.ones((1, S), np.float32))
    m01 = _build_m01(mask)
    if PD == BF16:
        m01 = _np_bf16(m01)

    in_maps = [
        {
            "xT": xT[c * BL : (c + 1) * BL],
            "wq_pad": wq_pad,
            "wp_pad": wp_pad,
            "ones_row": ones_row,
            "m01": m01,
        }
        for c in range(N_CORES)
    ]
    res = run_bass_kernel_spmd(nc, in_maps, list(range(N_CORES)))
    out_p = np.concatenate([res.results[c]["out"] for c in range(N_CORES)], axis=0)
    out = np.empty_like(out_p)
    out[:, PERM, :] = out_p
    return out


# revision 20
# speedup vs baseline: 1.3404x; 1.3404x over previous
"""Trainium2 Bass kernel for nn_Attention_13700945674736 (sparse local-window attention).

Strategy (8 NeuronCores, data-parallel over batch, 4 samples/core):
  - Permute the sequence axis s = 64*i + j  ->  s' = 16*j + i (image transpose).
    The 7x11 local window becomes a 1-D band |ds'| <= 83, so each 128-query
    tile only attends to 3 aligned 128-key chunks instead of 1024.
  - Exact per-chunk band is [128c-80, 128c+208): pairs at distance 81..83
    need |dj|=5 AND |di|>=1 simultaneously, which the 2-D mask forbids for
    the first/last 3 keys of a chunk, so halfwidth 80 suffices on the low
    side (and symmetrically 79+128 on the high side; 208 = 128+80).
  - Heads are padded to 64-partition slots (host-padded weights) so every
    engine access pattern starts at a 32-aligned partition.
  - All matmul operands are bf16; PSUM accumulation and softmax reductions
    stay fp32.
  - attnT[k, q] band tiles; exp on ScalarE (3 chunk-groups per head -> few,
    large activations); binary window mask applied multiplicatively, split
    between DVE and GPSIMD(Pool) for engine balance; @V uses
    lhsT=[V|0|ones|0] so softmax denominators land at partitions 64:112 of
    the same PSUM tile; the projection bias is folded in via a constant-1
    row of aoT.
  - po is a single persistent 2-bank PSUM tile; q-tile t of every head uses
    column (t%2)*512+(t//2)*128 so concurrently-open accumulation groups
    alternate banks.  Closed tiles keep their data across same-bank
    start=True (only has_written bits are cleared), so the per-head divides
    run in 2 halves while the next head's @V already accumulates.
  - PSUM budget: pat 2x[128,1024] (4 banks) + proj 2x[128,512] (2 banks)
    + po [128,1024] (2 banks) = 8 banks exactly.
  - Evacuation copies (Q/K/V/out PSUM->SBUF) are split between ScalarE and
    DVE by static schedule tables; output stores go through the SP queue so
    DMA setup does not stall the Activation sequencer.
"""

import sys

sys.path.insert(0, "/opt/trn_rl_repo")

import numpy as np

import concourse.bass as bass
from concourse import bacc
import concourse.mybir as mybir
import concourse.tile as tile
from concourse.bass_utils import run_bass_kernel_spmd

# ---------------------------------------------------------------- constants
B, S, C = 32, 1024, 384
H, D = 8, 48
HI, WI = 16, 64
N_CORES = 8
BL = B // N_CORES  # samples per core
SCALE = float(D) ** -0.5
F32 = mybir.dt.float32
BF16 = mybir.dt.bfloat16
PD = BF16  # precision of expT / m01 / vv

# s' = 16*j + i  <->  s = 64*i + j ;  PERM[s'] = s
_sp = np.arange(S)
PERM = (_sp % HI) * WI + (_sp // HI)

NQT = S // 128  # 8 query tiles (and key chunks)
WPADQ = 64 * H  # padded Q (and K) section width: 512
WQW = 2 * WPADQ + C  # 1408

# exact per-chunk bands: key-chunk c attends to queries [QLO[c], QHI[c])
QLO = [max(0, 128 * c - 80) for c in range(NQT)]
QHI = [min(S, 128 * c + 208) for c in range(NQT)]
WC = [QHI[c] - QLO[c] for c in range(NQT)]
OFFC = list(np.cumsum([0] + WC[:-1]))
BAND_W = sum(WC)  # 2144

# chunk groups per PSUM pat tile (each group width <= 1024 f32 = 2 banks)
CH_GROUPS = [(0, 1, 2), (3, 4, 5), (6, 7)]
GRP_BASE = [OFFC[g[0]] for g in CH_GROUPS]
GRP_W = [OFFC[g[-1]] + WC[g[-1]] - OFFC[g[0]] for g in CH_GROUPS]
assert max(GRP_W) <= 1024

# po column slot for q-tile t (identical for every head; open groups t,t+1
# always land in different PSUM banks)
PO_COL = [(t % 2) * 512 + (t // 2) * 128 for t in range(NQT)]

# ------------------------------------------------- engine schedule tables
# 'a' = ScalarE(Act) copy, 'd' = DVE tensor_copy
QK_EVAC = "aaaaaaaaaaaaaaaa"  # 16 per sample (Q then K, pair-major)
V_EVAC = "aaaaaaaa"  # 8 per sample
OUT_EVAC = "aaaaaaaa"  # 8 per sample
# 'd' = DVE, 'p' = Pool(GPSIMD) for the 3 mask multiplies of each head.
# g1 (chunks 3-5) sits mid-head where its latency hides; g0/g2 gate the
# @V start/tail chains, so they stay on the fast DVE.
MASK_ENG = ["ddd"] * H
# divide granularity: q-tile ranges; later ranges close later, and finer
# tail pieces release the po columns the next head's early @V chunks need.
DIV_RANGES = [(0, 8)]

# ---------------------------------------------------------------- bass program
_CACHE = {}


def _build():
    if "nc" in _CACHE:
        return _CACHE["nc"]

    nc = bacc.Bacc(None, target_bir_lowering=False)
    xT_d = nc.declare_dram_parameter("xT", [BL, C, S], BF16, isOutput=False)
    wq_d = nc.declare_dram_parameter("wq_pad", [C, WQW], BF16, isOutput=False)
    wp_d = nc.declare_dram_parameter("wp_pad", [4, 128, C], BF16, isOutput=False)
    ones_d = nc.declare_dram_parameter("ones_row", [1, S], BF16, isOutput=False)
    m_d = nc.declare_dram_parameter("m01", [128, BAND_W], PD, isOutput=False)
    out_d = nc.declare_dram_parameter("out", [BL, S, C], F32, isOutput=True)

    with tile.TileContext(nc) as tc:
        with (
            tc.tile_pool(name="singles", bufs=1) as singles,
            tc.tile_pool(name="xt_pool", bufs=3) as xt_pool,
            tc.tile_pool(name="out_pool", bufs=4) as out_pool,
            tc.tile_pool(name="ps_proj", bufs=2, space="PSUM") as ps_proj,
            tc.tile_pool(name="ps_pat", bufs=2, space="PSUM") as ps_pat,
            tc.tile_pool(name="ps_po", bufs=1, space="PSUM") as ps_po,
        ):
            # ---- constants.  Q/K/V weight sections load as separate DMAs so
            # the first projection matmuls only wait for their own section.
            w_sb = singles.tile([128, 3, WQW], BF16)
            wq_v = wq_d.rearrange("(c p) w -> p c w", p=128)
            nc.scalar.dma_start(w_sb[:, :, 0:WPADQ], wq_v[:, :, 0:WPADQ])
            nc.scalar.dma_start(
                w_sb[:, :, WPADQ : 2 * WPADQ], wq_v[:, :, WPADQ : 2 * WPADQ]
            )
            nc.scalar.dma_start(w_sb[:, :, 2 * WPADQ :], wq_v[:, :, 2 * WPADQ :])
            wp_sb = singles.tile([128, 4, C], BF16)
            nc.gpsimd.dma_start(wp_sb[:, :, :], wp_d.rearrange("f p c -> p f c"))
            m_sb = singles.tile([128, BAND_W], PD)
            nc.gpsimd.dma_start(m_sb, m_d[:, :])

            # persistent attention-output PSUM tile (2 banks)
            po = ps_po.tile([128, S], F32, tag="po")

            # ---- per-sample tiles, double-buffered for cross-sample overlap
            qTs, kTs, vvs, aoTs, expTs, dens = [], [], [], [], [], []
            for i in range(2):
                qTs.append(singles.tile([128, 4, S], BF16, name=f"qT{i}"))
                kTs.append(singles.tile([128, 4, S], BF16, name=f"kT{i}"))
                vvs.append(singles.tile([128, NQT, H, 128], PD, name=f"vv{i}"))
                aoTs.append(singles.tile([128, 4, S], BF16, name=f"aoT{i}"))
                expTs.append(singles.tile([128, BAND_W], PD, name=f"expT{i}"))
                dens.append(singles.tile([48, 2, 512], F32, name=f"den{i}"))
            def setup_set(i):
                """One-time constant sections of buffer set i (Pool engine).
                Set 1 is deferred until after proj(0) is issued so sample-0
                mask multiplies are not queued behind 20us of memsets."""
                vv, aoT = vvs[i], aoTs[i]
                nc.gpsimd.memset(vv[:, :, :, D : D + 16], 0.0)
                nc.gpsimd.memset(vv[:, :, :, D + 16 : 112], 1.0)
                nc.gpsimd.memset(vv[:, :, :, 112:128], 0.0)
                # zero dead rows (48:64, 112:128); starts must be 32-aligned so
                # cover 32:64 / 96:128 — live rows are rewritten by the divides.
                nc.gpsimd.memset(aoT[32:64, :, :], 0.0)
                nc.gpsimd.memset(aoT[96:128, :, :], 0.0)
                # constant-1 row: proj picks up b_proj from wp_pad[0][48]
                nc.gpsimd.dma_start(aoT[48:49, 0, :], ones_d[:, :])

            def evac(engine, dst, src):
                if engine == "a":
                    nc.scalar.copy(dst, src)
                else:
                    nc.vector.tensor_copy(dst, src)

            def load_xt(b):
                xt = xt_pool.tile([128, 3, S], BF16)
                nc.sync.dma_start(
                    xt[:, :, :], xT_d[b].rearrange("(c p) s -> p c s", p=128)
                )
                return xt

            def proj_units(b, xt):
                """QKV projection for sample b as a list of issue-units."""
                qT, kT, vv = qTs[b % 2], vvs[b % 2], None  # placeholder
                qT, kT, vv = qTs[b % 2], kTs[b % 2], vvs[b % 2]
                units = []

                def qk_unit(qk, pair, half, eng):
                    def u():
                        dst = qT if qk == 0 else kT
                        ncol = qk * WPADQ + pair * 128
                        ps = ps_proj.tile([128, 512], F32, tag="mm")
                        for ci in range(3):
                            nc.tensor.matmul(
                                ps[:, :],
                                w_sb[:, ci, ncol : ncol + 128],
                                xt[:, ci, half * 512 : (half + 1) * 512],
                                start=(ci == 0),
                                stop=(ci == 2),
                            )
                        seg = dst[:, pair, half * 512 : (half + 1) * 512]
                        evac(eng, seg, ps[:, :])

                    return u

                def v_unit(st, eng):
                    def u():
                        psv = ps_proj.tile([128, 512], F32, tag="mm")
                        for ci in range(3):
                            nc.tensor.matmul(
                                psv[:, 0:C],
                                xt[:, ci, st * 128 : (st + 1) * 128],
                                w_sb[:, ci, 2 * WPADQ : 2 * WPADQ + C],
                                start=(ci == 0),
                                stop=(ci == 2),
                            )
                        evac(
                            eng,
                            vv[:, st, :, 0:D],
                            psv[:, 0:C].rearrange("p (h d) -> p h d", h=H),
                        )

                    return u

                ei = iter(QK_EVAC)
                for qk in range(2):
                    for pair in range(4):
                        for half in range(2):
                            units.append(qk_unit(qk, pair, half, next(ei)))
                vi = iter(V_EVAC)
                for st in range(NQT):
                    units.append(v_unit(st, next(vi)))
                return units

            def proj(b, xt):
                for u in proj_units(b, xt):
                    u()

            def attention(b, fillers=()):
                """Per-head attention; after each head, drain a few filler
                units (next sample's projection / previous sample's output
                projection) so PE has work during the divide chains."""
                fillers = list(fillers)
                fi = 0
                qT, kT, vv, aoT = qTs[b % 2], kTs[b % 2], vvs[b % 2], aoTs[b % 2]
                for h in range(H):
                    pair, sub = divmod(h, 2)
                    p0 = sub * 64
                    expT = expTs[h % 2]
                    den_sb = dens[h % 2]
                    # ---- QK^T band + exp (3 chunk-groups); masks for g0/g1
                    # issue here, g2's mask issues later so DVE's in-order
                    # queue does not park the divides behind it.
                    def mask_mul(gi):
                        gbase, gw = GRP_BASE[gi], GRP_W[gi]
                        tt = (
                            nc.vector.tensor_tensor
                            if MASK_ENG[h][gi] == "d"
                            else nc.gpsimd.tensor_tensor
                        )
                        tt(
                            expT[:, gbase : gbase + gw],
                            expT[:, gbase : gbase + gw],
                            m_sb[:, gbase : gbase + gw],
                            mybir.AluOpType.mult,
                        )

                    for gi, grp in enumerate(CH_GROUPS):
                        gbase, gw = GRP_BASE[gi], GRP_W[gi]
                        pat = ps_pat.tile([128, 1024], F32, tag="attn")
                        for c in grp:
                            lo = OFFC[c] - gbase
                            hi = lo + WC[c]
                            a = lo
                            while a < hi:
                                b2 = min(hi, (a // 512 + 1) * 512)
                                nc.tensor.matmul(
                                    pat[:, a:b2],
                                    kT[p0 : p0 + D, pair, c * 128 : (c + 1) * 128],
                                    qT[
                                        p0 : p0 + D,
                                        pair,
                                        QLO[c] + (a - lo) : QLO[c] + (b2 - lo),
                                    ],
                                    start=True,
                                    stop=True,
                                )
                                a = b2
                        nc.scalar.activation(
                            expT[:, gbase : gbase + gw],
                            pat[:, 0:gw],
                            mybir.ActivationFunctionType.Exp,
                            scale=SCALE,
                        )
                        if gi < 2:
                            mask_mul(gi)

                    # ---- @V with ones rows at 64:112 -> denominators,
                    # interleaved with the per-range normalizes so divides
                    # start the moment their q-tiles close.
                    po_q = po[:, :].rearrange(
                        "p (tlo thi u) -> p thi tlo u", tlo=2, thi=4, u=128
                    )
                    den_flat = den_sb[:, :, :].rearrange("p a b -> p (a b)")

                    def av_chunk(c):
                        lhsT = vv[:, c, h, :]
                        for t in range(max(c - 1, 0), min(c + 2, NQT)):
                            pc = PO_COL[t]
                            qs = max(128 * t, QLO[c])
                            qe = min(128 * t + 128, QHI[c])
                            nc.tensor.matmul(
                                po[:, pc + (qs - 128 * t) : pc + (qe - 128 * t)],
                                lhsT,
                                expT[:, OFFC[c] + (qs - QLO[c]) : OFFC[c] + (qe - QLO[c])],
                                start=(c == max(t - 1, 0)),
                                stop=(c == min(t + 1, NQT - 1)),
                            )

                    def divide(ta, tb):
                        nt = (tb - ta) // 2
                        den_v = den_flat[:, ta * 128 : tb * 128].rearrange(
                            "p (thi tlo u) -> p thi tlo u", thi=nt, tlo=2, u=128
                        )
                        ao_v = aoT[
                            p0 : p0 + D, pair, ta * 128 : tb * 128
                        ].rearrange("p (thi tlo u) -> p thi tlo u", thi=nt, tlo=2, u=128)
                        nc.vector.reciprocal(
                            den_v, po_q[64 : 64 + D, ta // 2 : tb // 2]
                        )
                        nc.vector.tensor_tensor(
                            ao_v,
                            po_q[0:D, ta // 2 : tb // 2],
                            den_v,
                            mybir.AluOpType.mult,
                        )

                    g2_start = CH_GROUPS[2][0]
                    for c in range(NQT):
                        if c == g2_start:
                            mask_mul(2)  # just before @V enters group 2
                        av_chunk(c)
                        for ta, tb in DIV_RANGES:
                            # tile tb-1 stops at chunk min(tb, NQT-1)
                            if min(tb, NQT - 1) == c:
                                divide(ta, tb)
                    # drain filler units evenly across heads
                    target = (h + 1) * len(fillers) // H
                    while fi < target:
                        fillers[fi]()
                        fi += 1

            def out_proj_units(b):
                aoT = aoTs[b % 2]
                units = []

                def sp_unit(sp, e0, e1):
                    def u():
                        ot = out_pool.tile([128, 2, C], F32)
                        for k, eng in ((0, e0), (1, e1)):
                            st = 2 * sp + k
                            psp = ps_proj.tile([128, 512], F32, tag="mm")
                            for p in range(4):
                                nc.tensor.matmul(
                                    psp[:, 0:C],
                                    aoT[:, p, st * 128 : (st + 1) * 128],
                                    wp_sb[:, p, :],
                                    start=(p == 0),
                                    stop=(p == 3),
                                )
                            evac(eng, ot[:, k, :], psp[:, 0:C])
                        nc.sync.dma_start(
                            out_d[b, 2 * sp * 128 : (2 * sp + 2) * 128, :].rearrange(
                                "(k p) c -> p k c", k=2
                            ),
                            ot[:, :, :],
                        )

                    return u

                oi = iter(OUT_EVAC)
                for sp in range(NQT // 2):  # store pairs of q-tiles
                    units.append(sp_unit(sp, next(oi), next(oi)))
                return units

            def out_proj(b):
                for u in out_proj_units(b):
                    u()

            # ---------------- main pipeline: attention(b) runs interleaved
            # with proj(b+1) and out_proj(b-1) as filler units.
            setup_set(0)
            xts = {0: load_xt(0)}
            proj(0, xts[0])
            setup_set(1)
            for b in range(BL):
                fillers = []
                if b + 1 < BL:
                    xts[b + 1] = load_xt(b + 1)  # prefetch during attention(b)
                    fillers += proj_units(b + 1, xts[b + 1])
                if b > 0:
                    fillers += out_proj_units(b - 1)
                attention(b, fillers)
            out_proj(BL - 1)

    nc.finalize()
    _CACHE["nc"] = nc
    return nc


# ---------------------------------------------------------------- host wrapper
def _np_bf16(a):
    import ml_dtypes

    return np.asarray(a, dtype=ml_dtypes.bfloat16)


def _build_m01(mask):
    """[128, BAND_W] banded 0/1 mask in exact-band layout (rows = key within
    chunk c, cols = q in [QLO[c], QHI[c]))."""
    mp = np.asarray(mask)[np.ix_(PERM, PERM)]
    good = np.isfinite(mp) & (mp == 0.0)
    m01 = np.zeros((128, BAND_W), np.float32)
    covered = 0
    for c in range(NQT):
        blk = good[QLO[c] : QHI[c], c * 128 : (c + 1) * 128]  # [q, k]
        m01[:, OFFC[c] : OFFC[c] + WC[c]] = blk.T.astype(np.float32)
        covered += int(blk.sum())
    assert covered == int(good.sum()), "mask not covered by band layout"
    return m01


def _pad_wqkv(w_qkv):
    """[384, 1152] -> [384, 1408]: Q/K head h at cols h*64..h*64+48 (zero pad),
    V kept natural at cols 1024:1408."""
    out = np.zeros((C, WQW), np.float32)
    for sec in range(2):  # Q, K
        for h in range(H):
            out[:, sec * WPADQ + h * 64 : sec * WPADQ + h * 64 + D] = w_qkv[
                :, sec * C + h * D : sec * C + (h + 1) * D
            ]
    out[:, 2 * WPADQ :] = w_qkv[:, 2 * C :]
    return out


def _pad_wproj(w_proj, b_proj):
    """[384, 384] -> [4, 128, 384]: pair p rows 0:48 = head 2p, 64:112 = head 2p+1.
    Row 48 of pair 0 carries b_proj (matched by the constant-1 row in aoT)."""
    out = np.zeros((4, 128, C), np.float32)
    for p in range(4):
        out[p, 0:D] = w_proj[(2 * p) * D : (2 * p + 1) * D]
        out[p, 64 : 64 + D] = w_proj[(2 * p + 1) * D : (2 * p + 2) * D]
    out[0, D] = b_proj
    return out


def kernel(x, w_qkv, w_proj, b_proj, mask):
    x = np.asarray(x, np.float32)
    w_qkv = np.asarray(w_qkv, np.float32)
    w_proj = np.asarray(w_proj, np.float32)
    b_proj = np.asarray(b_proj, np.float32)

    nc = _build()

    xT = _np_bf16(np.ascontiguousarray(x[:, PERM, :].transpose(0, 2, 1)))  # [B, C, S']
    wq_pad = _np_bf16(_pad_wqkv(w_qkv))
    wp_pad = _np_bf16(_pad_wproj(w_proj, b_proj))
    ones_row = _np_bf16(np.ones((1, S), np.float32))
    m01 = _build_m01(mask)
    if PD == BF16:
        m01 = _np_bf16(m01)

    in_maps = [
        {
            "xT": xT[c * BL : (c + 1) * BL],
            "wq_pad": wq_pad,
            "wp_pad": wp_pad,
            "ones_row": ones_row,
            "m01": m01,
        }
        for c in range(N_CORES)
    ]
    res = run_bass_kernel_spmd(nc, in_maps, list(range(N_CORES)))
    out_p = np.concatenate([res.results[c]["out"] for c in range(N_CORES)], axis=0)
    out = np.empty_like(out_p)
    out[:, PERM, :] = out_p
    return out


# revision 50
# speedup vs baseline: 1.4338x; 1.0697x over previous
"""Trainium2 Bass kernel for nn_Attention_13700945674736 (sparse local-window attention).

Strategy (8 NeuronCores, data-parallel over batch, 4 samples/core):
  - Permute the sequence axis s = 64*i + j  ->  s' = 16*j + i (image transpose).
    The 7x11 local window becomes a 1-D band |ds'| <= 83, so each 128-query
    tile only attends to 3 aligned 128-key chunks instead of 1024.
  - Exact per-chunk band is [128c-80, 128c+208): pairs at distance 81..83
    need |dj|=5 AND |di|>=1 simultaneously, which the 2-D mask forbids for
    the first/last 3 keys of a chunk, so halfwidth 80 suffices on the low
    side (and symmetrically 79+128 on the high side; 208 = 128+80).
  - Heads are padded to 64-partition slots (host-padded weights) so every
    engine access pattern starts at a 32-aligned partition.
  - All matmul operands are bf16; PSUM accumulation and softmax reductions
    stay fp32.
  - attnT[k, q] band tiles; exp on ScalarE (3 chunk-groups per head -> few,
    large activations); binary window mask applied multiplicatively, split
    between DVE and GPSIMD(Pool) for engine balance; @V uses
    lhsT=[V|0|ones|0] so softmax denominators land at partitions 64:112 of
    the same PSUM tile; the projection bias is folded in via a constant-1
    row of aoT.
  - po is a single persistent 2-bank PSUM tile; q-tile t of every head uses
    column (t%2)*512+(t//2)*128 so concurrently-open accumulation groups
    alternate banks.  Closed tiles keep their data across same-bank
    start=True (only has_written bits are cleared), so the per-head divides
    run in 2 halves while the next head's @V already accumulates.
  - PSUM budget: pat 2x[128,1024] (4 banks) + proj 2x[128,512] (2 banks)
    + po [128,1024] (2 banks) = 8 banks exactly.
  - Evacuation copies (Q/K/V/out PSUM->SBUF) are split between ScalarE and
    DVE by static schedule tables; output stores go through the SP queue so
    DMA setup does not stall the Activation sequencer.
"""

import sys

sys.path.insert(0, "/opt/trn_rl_repo")

import numpy as np

import concourse.bass as bass
from concourse import bacc
import concourse.mybir as mybir
import concourse.tile as tile
from concourse.bass_utils import run_bass_kernel_spmd

# ---------------------------------------------------------------- constants
B, S, C = 32, 1024, 384
H, D = 8, 48
HI, WI = 16, 64
N_CORES = 8
BL = B // N_CORES  # samples per core
SCALE = float(D) ** -0.5
F32 = mybir.dt.float32
BF16 = mybir.dt.bfloat16
PD = BF16  # precision of expT / m01 / vv

# s' = 16*j + i  <->  s = 64*i + j ;  PERM[s'] = s
_sp = np.arange(S)
PERM = (_sp % HI) * WI + (_sp // HI)

NQT = S // 128  # 8 query tiles (and key chunks)
WPADQ = 64 * H  # padded Q (and K) section width: 512
WQW = 2 * WPADQ + C  # 1408

# exact per-chunk bands: key-chunk c attends to queries [QLO[c], QHI[c])
QLO = [max(0, 128 * c - 80) for c in range(NQT)]
QHI = [min(S, 128 * c + 208) for c in range(NQT)]
WC = [QHI[c] - QLO[c] for c in range(NQT)]
OFFC = list(np.cumsum([0] + WC[:-1]))
BAND_W = sum(WC)  # 2144

# chunk groups per PSUM pat tile (each group width <= 1024 f32 = 2 banks)
CH_GROUPS = [(0, 1, 2), (3, 4, 5), (6, 7)]
GRP_BASE = [OFFC[g[0]] for g in CH_GROUPS]
GRP_W = [OFFC[g[-1]] + WC[g[-1]] - OFFC[g[0]] for g in CH_GROUPS]
assert max(GRP_W) <= 1024

# po column slot for q-tile t (identical for every head; open groups t,t+1
# always land in different PSUM banks)
PO_COL = [(t % 2) * 512 + (t // 2) * 128 for t in range(NQT)]

# ------------------------------------------------- engine schedule tables
# 'a' = ScalarE(Act) copy, 'd' = DVE tensor_copy
QK_EVAC = "aadaadaadaadaada"  # 16 per sample (Q then K, pair-major)
V_EVAC = "aaadaaaa"  # 8 per sample
OUT_EVAC = "aaaaaaaa"  # 8 per sample
QK_EVAC_N = QK_EVAC  # startup narrow units (16)
V_EVAC_N = V_EVAC  # startup narrow units (8)
# 'd' = DVE, 'p' = Pool(GPSIMD) for the 3 mask multiplies of each head.
# g1 (chunks 3-5) sits mid-head where its latency hides; g0/g2 gate the
# @V start/tail chains, so they stay on the fast DVE.
MASK_ENG = ["ddd"] * H
# divide granularity: q-tile ranges; later ranges close later, and finer
# tail pieces release the po columns the next head's early @V chunks need.
DIV_RANGES = [(0, 8)]
# defer Q/K pairs 2-3 of proj(b) into attention(b)'s own filler list
DEFER_LATE = True

# ---------------------------------------------------------------- bass program
_CACHE = {}


def _build():
    if "nc" in _CACHE:
        return _CACHE["nc"]

    nc = bacc.Bacc(None, target_bir_lowering=False)
    xT_d = nc.declare_dram_parameter("xT", [BL, C, S], BF16, isOutput=False)
    wq_d = nc.declare_dram_parameter("wq_pad", [C, WQW], BF16, isOutput=False)
    wp_d = nc.declare_dram_parameter("wp_pad", [4, 128, C], BF16, isOutput=False)
    ones_d = nc.declare_dram_parameter("ones_row", [1, S], BF16, isOutput=False)
    m_d = nc.declare_dram_parameter("m01", [128, BAND_W], PD, isOutput=False)
    out_d = nc.declare_dram_parameter("out", [BL, S, C], F32, isOutput=True)

    with tile.TileContext(nc) as tc:
        with (
            tc.tile_pool(name="singles", bufs=1) as singles,
            tc.tile_pool(name="xt_pool", bufs=3) as xt_pool,
            tc.tile_pool(name="out_pool", bufs=4) as out_pool,
            tc.tile_pool(name="ps_proj", bufs=2, space="PSUM") as ps_proj,
            tc.tile_pool(name="ps_pat", bufs=2, space="PSUM") as ps_pat,
            tc.tile_pool(name="ps_po", bufs=1, space="PSUM") as ps_po,
        ):
            # ---- constants.  Q/K/V weight sections load as separate DMAs so
            # the first projection matmuls only wait for their own section.
            w_sb = singles.tile([128, 3, WQW], BF16)
            wq_v = wq_d.rearrange("(c p) w -> p c w", p=128)
            nc.scalar.dma_start(w_sb[:, :, 0:WPADQ], wq_v[:, :, 0:WPADQ])
            nc.scalar.dma_start(
                w_sb[:, :, WPADQ : 2 * WPADQ], wq_v[:, :, WPADQ : 2 * WPADQ]
            )
            nc.scalar.dma_start(w_sb[:, :, 2 * WPADQ :], wq_v[:, :, 2 * WPADQ :])
            wp_sb = singles.tile([128, 4, C], BF16)
            m_sb = singles.tile([128, BAND_W], PD)

            def load_consts():
                # issued after proj(0) so these transfers cannot jump ahead
                # of the startup-critical xt/wQ DMAs on the DMA engines
                nc.sync.dma_start(m_sb, m_d[:, :])
                nc.sync.dma_start(wp_sb[:, :, :], wp_d.rearrange("f p c -> p f c"))

            # persistent attention-output PSUM tile (2 banks)
            po = ps_po.tile([128, S], F32, tag="po")
            _prj_n = [0]

            def prj_slot():
                _prj_n[0] += 1
                ps = ps_proj.tile([128, 512], F32, tag="mm", name=f"prj{_prj_n[0]}")
                return ps

            # ---- per-sample tiles, double-buffered for cross-sample overlap
            qTs, kTs, vvs, aoTs, expTs, dens = [], [], [], [], [], []
            for i in range(2):
                qTs.append(singles.tile([128, 4, S], BF16, name=f"qT{i}"))
                kTs.append(singles.tile([128, 4, S], BF16, name=f"kT{i}"))
                vvs.append(singles.tile([128, NQT, H, 128], PD, name=f"vv{i}"))
                aoTs.append(singles.tile([128, 4, S], BF16, name=f"aoT{i}"))
                expTs.append(singles.tile([128, BAND_W], PD, name=f"expT{i}"))
                dens.append(singles.tile([48, 2, 512], F32, name=f"den{i}"))
            def setup_set(i):
                """One-time constant sections of buffer set i (Pool engine).
                Set 1 is deferred until after proj(0) is issued so sample-0
                mask multiplies are not queued behind 20us of memsets."""
                vv, aoT = vvs[i], aoTs[i]
                nc.gpsimd.memset(vv[:, :, :, D : D + 16], 0.0)
                nc.gpsimd.memset(vv[:, :, :, D + 16 : 112], 1.0)
                nc.gpsimd.memset(vv[:, :, :, 112:128], 0.0)
                # zero dead rows (48:64, 112:128); starts must be 32-aligned so
                # cover 32:64 / 96:128 — live rows are rewritten by the divides.
                nc.gpsimd.memset(aoT[32:64, :, :], 0.0)
                nc.gpsimd.memset(aoT[96:128, :, :], 0.0)
                # constant-1 row: proj picks up b_proj from wp_pad[0][48]
                nc.gpsimd.dma_start(aoT[48:49, 0, :], ones_d[:, :])

            def evac(engine, dst, src):
                if engine == "a":
                    nc.scalar.copy(dst, src)
                else:
                    nc.vector.tensor_copy(dst, src)

            def load_xt(b, split=False):
                xt = xt_pool.tile([128, 3, S], BF16)
                src = xT_d[b].rearrange("(c p) s -> p c s", p=128)
                if split:
                    # startup: two sync-queue halves so the first projection
                    # matmuls only wait for the first half
                    nc.sync.dma_start(xt[:, :, 0:512], src[:, :, 0:512])
                    nc.sync.dma_start(xt[:, :, 512:S], src[:, :, 512:S])
                else:
                    nc.sync.dma_start(xt[:, :, :], src)
                return xt

            def _qk_mm(xt, qk, pair, half, dstps):
                ncol = qk * WPADQ + pair * 128
                for ci in range(3):
                    nc.tensor.matmul(
                        dstps,
                        w_sb[:, ci, ncol : ncol + 128],
                        xt[:, ci, half * 512 : (half + 1) * 512],
                        start=(ci == 0),
                        stop=(ci == 2),
                    )

            def _v_mm(xt, st, dstps):
                for ci in range(3):
                    nc.tensor.matmul(
                        dstps,
                        xt[:, ci, st * 128 : (st + 1) * 128],
                        w_sb[:, ci, 2 * WPADQ : 2 * WPADQ + C],
                        start=(ci == 0),
                        stop=(ci == 2),
                    )

            def proj_units(b, xt):
                """QKV projection for sample b as narrow issue-units over the
                two rotating psP halves."""
                qT, kT, vv = qTs[b % 2], kTs[b % 2], vvs[b % 2]

                def qk_unit(qk, pair, half, eng):
                    def u():
                        dst = qT if qk == 0 else kT
                        ps = prj_slot()
                        _qk_mm(xt, qk, pair, half, ps)
                        evac(
                            eng,
                            dst[:, pair, half * 512 : (half + 1) * 512],
                            ps,
                        )

                    return u

                def v_unit(st, eng):
                    def u():
                        ps = prj_slot()
                        _v_mm(xt, st, ps[:, 0:C])
                        evac(
                            eng,
                            vv[:, st, :, 0:D],
                            ps[:, 0:C].rearrange("p (h d) -> p h d", h=H),
                        )

                    return u

                ei = iter(QK_EVAC)
                qkp = {}
                for qk in range(2):
                    for pair in range(4):
                        for half in range(2):
                            qkp[(qk, pair, half)] = qk_unit(qk, pair, half, next(ei))
                vi = iter(V_EVAC)
                v_units = [v_unit(st, next(vi)) for st in range(NQT)]
                # early: Q/K pairs 0-1 + all V (needed by heads 0-3 and @V);
                # late: Q/K pairs 2-3, deferrable into the next attention's
                # own filler list (pair 2 first used by head 4).
                early, late = [], []
                for (qk, pair, half), u in qkp.items():
                    (early if pair < 2 else late).append(u)
                early.extend(v_units)
                return early, late

            def attention(b, fillers=(), tail_div=False):
                """Per-head attention; after each head, drain a few filler
                units (next sample's projection / previous sample's output
                projection) so PE has work during the divide chains.
                tail_div: the last head's divide runs in q-tile pairs so the
                final output projection unblocks progressively."""
                fillers = list(fillers)
                fi = 0
                qT, kT, vv, aoT = qTs[b % 2], kTs[b % 2], vvs[b % 2], aoTs[b % 2]
                for h in range(H):
                    div_ranges = (
                        [(0, 2), (2, 4), (4, 6), (6, 8)]
                        if (tail_div and h == H - 1)
                        else DIV_RANGES
                    )
                    pair, sub = divmod(h, 2)
                    p0 = sub * 64
                    expT = expTs[h % 2]
                    den_sb = dens[h % 2]
                    # ---- QK^T band + exp (3 chunk-groups); masks for g0/g1
                    # issue here, g2's mask issues later so DVE's in-order
                    # queue does not park the divides behind it.
                    def mask_mul(gi):
                        gbase, gw = GRP_BASE[gi], GRP_W[gi]
                        tt = (
                            nc.vector.tensor_tensor
                            if MASK_ENG[h][gi] == "d"
                            else nc.gpsimd.tensor_tensor
                        )
                        tt(
                            expT[:, gbase : gbase + gw],
                            expT[:, gbase : gbase + gw],
                            m_sb[:, gbase : gbase + gw],
                            mybir.AluOpType.mult,
                        )

                    for gi, grp in enumerate(CH_GROUPS):
                        gbase, gw = GRP_BASE[gi], GRP_W[gi]
                        pat = ps_pat.tile([128, 1024], F32, tag="attn")
                        for c in grp:
                            lo = OFFC[c] - gbase
                            hi = lo + WC[c]
                            a = lo
                            while a < hi:
                                b2 = min(hi, (a // 512 + 1) * 512)
                                nc.tensor.matmul(
                                    pat[:, a:b2],
                                    kT[p0 : p0 + D, pair, c * 128 : (c + 1) * 128],
                                    qT[
                                        p0 : p0 + D,
                                        pair,
                                        QLO[c] + (a - lo) : QLO[c] + (b2 - lo),
                                    ],
                                    start=True,
                                    stop=True,
                                )
                                a = b2
                        nc.scalar.activation(
                            expT[:, gbase : gbase + gw],
                            pat[:, 0:gw],
                            mybir.ActivationFunctionType.Exp,
                            scale=SCALE,
                        )
                        if gi < 2:
                            mask_mul(gi)

                    # ---- @V with ones rows at 64:112 -> denominators,
                    # interleaved with the per-range normalizes so divides
                    # start the moment their q-tiles close.
                    po_q = po[:, :].rearrange(
                        "p (tlo thi u) -> p thi tlo u", tlo=2, thi=4, u=128
                    )
                    den_flat = den_sb[:, :, :].rearrange("p a b -> p (a b)")

                    def av_chunk(c):
                        lhsT = vv[:, c, h, :]
                        for t in range(max(c - 1, 0), min(c + 2, NQT)):
                            pc = PO_COL[t]
                            qs = max(128 * t, QLO[c])
                            qe = min(128 * t + 128, QHI[c])
                            nc.tensor.matmul(
                                po[:, pc + (qs - 128 * t) : pc + (qe - 128 * t)],
                                lhsT,
                                expT[:, OFFC[c] + (qs - QLO[c]) : OFFC[c] + (qe - QLO[c])],
                                start=(c == max(t - 1, 0)),
                                stop=(c == min(t + 1, NQT - 1)),
                            )

                    def divide(ta, tb):
                        nt = (tb - ta) // 2
                        den_v = den_flat[:, ta * 128 : tb * 128].rearrange(
                            "p (thi tlo u) -> p thi tlo u", thi=nt, tlo=2, u=128
                        )
                        ao_v = aoT[
                            p0 : p0 + D, pair, ta * 128 : tb * 128
                        ].rearrange("p (thi tlo u) -> p thi tlo u", thi=nt, tlo=2, u=128)
                        nc.vector.reciprocal(
                            den_v, po_q[64 : 64 + D, ta // 2 : tb // 2]
                        )
                        nc.vector.tensor_tensor(
                            ao_v,
                            po_q[0:D, ta // 2 : tb // 2],
                            den_v,
                            mybir.AluOpType.mult,
                        )

                    g2_start = CH_GROUPS[2][0]
                    for c in range(NQT):
                        if c == g2_start:
                            mask_mul(2)  # just before @V enters group 2
                        av_chunk(c)
                        for ta, tb in div_ranges:
                            # tile tb-1 stops at chunk min(tb, NQT-1)
                            if min(tb, NQT - 1) == c:
                                divide(ta, tb)
                    # drain filler units evenly across heads
                    target = (h + 1) * len(fillers) // H
                    while fi < target:
                        fillers[fi]()
                        fi += 1

            def _op_mm(aoT, st, dstps):
                for p in range(4):
                    nc.tensor.matmul(
                        dstps,
                        aoT[:, p, st * 128 : (st + 1) * 128],
                        wp_sb[:, p, :],
                        start=(p == 0),
                        stop=(p == 3),
                    )

            def out_proj_units(b):
                aoT = aoTs[b % 2]
                units = []

                def st_unit(st, eng):
                    def u():
                        ot = out_pool.tile([128, C], F32, tag="ot", name=f"ot{b}_{st}")
                        ps = prj_slot()
                        _op_mm(aoT, st, ps[:, 0:C])
                        evac(eng, ot[:, :], ps[:, 0:C])
                        nc.sync.dma_start(
                            out_d[b, st * 128 : (st + 1) * 128, :], ot[:, :]
                        )

                    return u

                oi = iter(OUT_EVAC)
                for st in range(NQT):
                    units.append(st_unit(st, next(oi)))
                return units

            def out_proj_tail(b):
                # final sample: narrow evacs on alternating engines so the
                # drain is not serialized on one engine or on psP
                aoT = aoTs[b % 2]
                for sp in range(NQT // 2):
                    ot = out_pool.tile([128, 2, C], F32)
                    for k, eng in ((0, "a"), (1, "d")):
                        st = 2 * sp + k
                        ps = prj_slot()
                        _op_mm(aoT, st, ps[:, 0:C])
                        evac(eng, ot[:, k, :], ps[:, 0:C])
                    nc.sync.dma_start(
                        out_d[b, 2 * sp * 128 : (2 * sp + 2) * 128, :].rearrange(
                            "(k p) c -> p k c", k=2
                        ),
                        ot[:, :, :],
                    )

            # ---------------- main pipeline: attention(b) runs interleaved
            # with filler units: the tail of sample b's own projection
            # (Q/K pairs 2-3, first used by head 4), proj(b+1)'s early
            # units, and out_proj(b-1).
            setup_set(0)
            xts = {0: load_xt(0, split=True)}
            early0, late0 = proj_units(0, xts[0])
            for u in early0:
                u()
            load_consts()
            setup_set(1)
            carry = late0 if DEFER_LATE else []
            if not DEFER_LATE:
                for u in late0:
                    u()
            for b in range(BL):
                fillers = list(carry)
                carry = []
                if b + 1 < BL:
                    xts[b + 1] = load_xt(b + 1)  # prefetch during attention(b)
                    early, late = proj_units(b + 1, xts[b + 1])
                    fillers += early
                    if DEFER_LATE:
                        carry = late
                    else:
                        fillers += late
                if b > 0:
                    fillers += out_proj_units(b - 1)
                attention(b, fillers, tail_div=(b == BL - 1))
            out_proj_tail(BL - 1)

    nc.finalize()
    _CACHE["nc"] = nc
    return nc


# ---------------------------------------------------------------- host wrapper
def _np_bf16(a):
    import ml_dtypes

    return np.asarray(a, dtype=ml_dtypes.bfloat16)


def _build_m01(mask):
    """[128, BAND_W] banded 0/1 mask in exact-band layout (rows = key within
    chunk c, cols = q in [QLO[c], QHI[c]))."""
    mp = np.asarray(mask)[np.ix_(PERM, PERM)]
    good = np.isfinite(mp) & (mp == 0.0)
    m01 = np.zeros((128, BAND_W), np.float32)
    covered = 0
    for c in range(NQT):
        blk = good[QLO[c] : QHI[c], c * 128 : (c + 1) * 128]  # [q, k]
        m01[:, OFFC[c] : OFFC[c] + WC[c]] = blk.T.astype(np.float32)
        covered += int(blk.sum())
    assert covered == int(good.sum()), "mask not covered by band layout"
    return m01


def _pad_wqkv(w_qkv):
    """[384, 1152] -> [384, 1408]: Q/K head h at cols h*64..h*64+48 (zero pad),
    V kept natural at cols 1024:1408."""
    out = np.zeros((C, WQW), np.float32)
    for sec in range(2):  # Q, K
        for h in range(H):
            out[:, sec * WPADQ + h * 64 : sec * WPADQ + h * 64 + D] = w_qkv[
                :, sec * C + h * D : sec * C + (h + 1) * D
            ]
    out[:, 2 * WPADQ :] = w_qkv[:, 2 * C :]
    return out


def _pad_wproj(w_proj, b_proj):
    """[384, 384] -> [4, 128, 384]: pair p rows 0:48 = head 2p, 64:112 = head 2p+1.
    Row 48 of pair 0 carries b_proj (matched by the constant-1 row in aoT)."""
    out = np.zeros((4, 128, C), np.float32)
    for p in range(4):
        out[p, 0:D] = w_proj[(2 * p) * D : (2 * p + 1) * D]
        out[p, 64 : 64 + D] = w_proj[(2 * p + 1) * D : (2 * p + 2) * D]
    out[0, D] = b_proj
    return out


def kernel(x, w_qkv, w_proj, b_proj, mask):
    x = np.asarray(x, np.float32)
    w_qkv = np.asarray(w_qkv, np.float32)
    w_proj = np.asarray(w_proj, np.float32)
    b_proj = np.asarray(b_proj, np.float32)

    nc = _build()

    xT = _np_bf16(np.ascontiguousarray(x[:, PERM, :].transpose(0, 2, 1)))  # [B, C, S']
    wq_pad = _np_bf16(_pad_wqkv(w_qkv))
    wp_pad = _np_bf16(_pad_wproj(w_proj, b_proj))
    ones_row = _np_bf16(np.ones((1, S), np.float32))
    m01 = _build_m01(mask)
    if PD == BF16:
        m01 = _np_bf16(m01)

    in_maps = [
        {
            "xT": xT[c * BL : (c + 1) * BL],
            "wq_pad": wq_pad,
            "wp_pad": wp_pad,
            "ones_row": ones_row,
            "m01": m01,
        }
        for c in range(N_CORES)
    ]
    res = run_bass_kernel_spmd(nc, in_maps, list(range(N_CORES)))
    out_p = np.concatenate([res.results[c]["out"] for c in range(N_CORES)], axis=0)
    out = np.empty_like(out_p)
    out[:, PERM, :] = out_p
    return out


# revision 56
# speedup vs baseline: 1.4342x; 1.0003x over previous
"""Trainium2 Bass kernel for nn_Attention_13700945674736 (sparse local-window attention).

Strategy (8 NeuronCores, data-parallel over batch, 4 samples/core):
  - Permute the sequence axis s = 64*i + j  ->  s' = 16*j + i (image transpose).
    The 7x11 local window becomes a 1-D band |ds'| <= 83, so each 128-query
    tile only attends to 3 aligned 128-key chunks instead of 1024.
  - Exact per-chunk band is [128c-80, 128c+208): pairs at distance 81..83
    need |dj|=5 AND |di|>=1 simultaneously, which the 2-D mask forbids for
    the first/last 3 keys of a chunk, so halfwidth 80 suffices on the low
    side (and symmetrically 79+128 on the high side; 208 = 128+80).
  - Heads are padded to 64-partition slots (host-padded weights) so every
    engine access pattern starts at a 32-aligned partition.
  - All matmul operands are bf16; PSUM accumulation and softmax reductions
    stay fp32.
  - attnT[k, q] band tiles; exp on ScalarE (3 chunk-groups per head -> few,
    large activations); binary window mask applied multiplicatively on DVE;
    @V uses lhsT=[V|0|ones|0] so softmax denominators land at partitions
    64:112 of the same PSUM tile; the projection bias is folded in via a
    constant-1 row of aoT.
  - po is a single persistent 2-bank PSUM tile; q-tile t of every head uses
    column (t%2)*512+(t//2)*128 so concurrently-open accumulation groups
    alternate banks.  Closed tiles keep their data across same-bank
    start=True (only has_written bits are cleared), so the per-head divides
    run in 2 halves while the next head's @V already accumulates.
  - PSUM budget: pat 2x[128,1024] (4 banks) + proj 2x[128,512] (2 banks)
    + po [128,1024] (2 banks) = 8 banks exactly.
  - Evacuation copies (Q/K/V/out PSUM->SBUF) are split between ScalarE and
    DVE by static schedule tables; output stores go through the SP queue so
    DMA setup does not stall the Activation sequencer.
  - Software pipelining: attention(b) drains "filler" issue-units after each
    head -- sample b's own deferred Q/K pair-2/3 projection, proj(b+1)'s
    units, and out_proj(b-1)'s units (round-robin mixed) -- so the PE always
    has independent matmul work while the per-head softmax-divide chains
    (@V -> reciprocal -> multiply, serialized by the shared po tile) drain
    on DVE.  The last sample's final head divides in q-tile pairs so the
    closing output projection unblocks progressively.
"""

import sys

sys.path.insert(0, "/opt/trn_rl_repo")

import numpy as np

import concourse.bass as bass
from concourse import bacc
import concourse.mybir as mybir
import concourse.tile as tile
from concourse.bass_utils import run_bass_kernel_spmd

# ---------------------------------------------------------------- constants
B, S, C = 32, 1024, 384
H, D = 8, 48
HI, WI = 16, 64
N_CORES = 8
BL = B // N_CORES  # samples per core
SCALE = float(D) ** -0.5
F32 = mybir.dt.float32
BF16 = mybir.dt.bfloat16
PD = BF16  # precision of expT / m01 / vv

# s' = 16*j + i  <->  s = 64*i + j ;  PERM[s'] = s
_sp = np.arange(S)
PERM = (_sp % HI) * WI + (_sp // HI)

NQT = S // 128  # 8 query tiles (and key chunks)
WPADQ = 64 * H  # padded Q (and K) section width: 512
WQW = 2 * WPADQ + C  # 1408

# exact per-chunk bands: key-chunk c attends to queries [QLO[c], QHI[c])
QLO = [max(0, 128 * c - 80) for c in range(NQT)]
QHI = [min(S, 128 * c + 208) for c in range(NQT)]
WC = [QHI[c] - QLO[c] for c in range(NQT)]
OFFC = list(np.cumsum([0] + WC[:-1]))
BAND_W = sum(WC)  # 2144

# chunk groups per PSUM pat tile (each group width <= 1024 f32 = 2 banks)
CH_GROUPS = [(0, 1, 2), (3, 4, 5), (6, 7)]
GRP_BASE = [OFFC[g[0]] for g in CH_GROUPS]
GRP_W = [OFFC[g[-1]] + WC[g[-1]] - OFFC[g[0]] for g in CH_GROUPS]
assert max(GRP_W) <= 1024

# po column slot for q-tile t (identical for every head; open groups t,t+1
# always land in different PSUM banks)
PO_COL = [(t % 2) * 512 + (t // 2) * 128 for t in range(NQT)]

# ------------------------------------------------- engine schedule tables
# 'a' = ScalarE(Act) copy, 'd' = DVE tensor_copy
QK_EVAC = "aadaadaadaadaada"  # 16 per sample (Q then K, pair-major)
V_EVAC = "aaadaaaa"  # 8 per sample
OUT_EVAC = "aaaaaaaa"  # 8 per sample
# 'd' = DVE, 'p' = Pool(GPSIMD) for the 3 mask multiplies of each head.
# g1 (chunks 3-5) sits mid-head where its latency hides; g0/g2 gate the
# @V start/tail chains, so they stay on the fast DVE.
MASK_ENG = ["ddd"] * H
# divide granularity: q-tile ranges; later ranges close later, and finer
# tail pieces release the po columns the next head's early @V chunks need.
DIV_RANGES = [(0, 8)]
# defer Q/K pairs 2-3 of proj(b) into attention(b)'s own filler list
DEFER_LATE = True

# ---------------------------------------------------------------- bass program
_CACHE = {}


def _build():
    if "nc" in _CACHE:
        return _CACHE["nc"]

    nc = bacc.Bacc(None, target_bir_lowering=False)
    xT_d = nc.declare_dram_parameter("xT", [BL, C, S], BF16, isOutput=False)
    wq_d = nc.declare_dram_parameter("wq_pad", [C, WQW], BF16, isOutput=False)
    wp_d = nc.declare_dram_parameter("wp_pad", [4, 128, C], BF16, isOutput=False)
    ones_d = nc.declare_dram_parameter("ones_row", [1, S], BF16, isOutput=False)
    m_d = nc.declare_dram_parameter("m01", [128, BAND_W], PD, isOutput=False)
    out_d = nc.declare_dram_parameter("out", [BL, S, C], F32, isOutput=True)

    with tile.TileContext(nc) as tc:
        with (
            tc.tile_pool(name="singles", bufs=1) as singles,
            tc.tile_pool(name="xt_pool", bufs=3) as xt_pool,
            tc.tile_pool(name="out_pool", bufs=4) as out_pool,
            tc.tile_pool(name="ps_proj", bufs=2, space="PSUM") as ps_proj,
            tc.tile_pool(name="ps_pat", bufs=2, space="PSUM") as ps_pat,
            tc.tile_pool(name="ps_po", bufs=1, space="PSUM") as ps_po,
        ):
            # ---- constants.  Q/K/V weight sections load as separate DMAs so
            # the first projection matmuls only wait for their own section.
            w_sb = singles.tile([128, 3, WQW], BF16)
            wq_v = wq_d.rearrange("(c p) w -> p c w", p=128)
            nc.scalar.dma_start(w_sb[:, :, 0:WPADQ], wq_v[:, :, 0:WPADQ])
            nc.scalar.dma_start(
                w_sb[:, :, WPADQ : 2 * WPADQ], wq_v[:, :, WPADQ : 2 * WPADQ]
            )
            nc.scalar.dma_start(w_sb[:, :, 2 * WPADQ :], wq_v[:, :, 2 * WPADQ :])
            wp_sb = singles.tile([128, 4, C], BF16)
            m_sb = singles.tile([128, BAND_W], PD)

            def load_consts():
                # issued after proj(0) so these transfers cannot jump ahead
                # of the startup-critical xt/wQ DMAs on the DMA engines
                nc.sync.dma_start(m_sb, m_d[:, :])
                nc.sync.dma_start(wp_sb[:, :, :], wp_d.rearrange("f p c -> p f c"))

            # persistent attention-output PSUM tile (2 banks)
            po = ps_po.tile([128, S], F32, tag="po")
            _prj_n = [0]

            def prj_slot():
                _prj_n[0] += 1
                ps = ps_proj.tile([128, 512], F32, tag="mm", name=f"prj{_prj_n[0]}")
                return ps

            # ---- per-sample tiles, double-buffered for cross-sample overlap
            qTs, kTs, vvs, aoTs, expTs, dens = [], [], [], [], [], []
            for i in range(2):
                qTs.append(singles.tile([128, 4, S], BF16, name=f"qT{i}"))
                kTs.append(singles.tile([128, 4, S], BF16, name=f"kT{i}"))
                vvs.append(singles.tile([128, NQT, H, 128], PD, name=f"vv{i}"))
                aoTs.append(singles.tile([128, 4, S], BF16, name=f"aoT{i}"))
                expTs.append(singles.tile([128, BAND_W], PD, name=f"expT{i}"))
                dens.append(singles.tile([48, 2, 512], F32, name=f"den{i}"))
            def setup_set(i):
                """One-time constant sections of buffer set i (Pool engine).
                Set 1 is deferred until after proj(0) is issued so sample-0
                mask multiplies are not queued behind 20us of memsets."""
                vv, aoT = vvs[i], aoTs[i]
                nc.gpsimd.memset(vv[:, :, :, D : D + 16], 0.0)
                nc.gpsimd.memset(vv[:, :, :, D + 16 : 112], 1.0)
                nc.gpsimd.memset(vv[:, :, :, 112:128], 0.0)
                # zero dead rows (48:64, 112:128); starts must be 32-aligned so
                # cover 32:64 / 96:128 — live rows are rewritten by the divides.
                nc.gpsimd.memset(aoT[32:64, :, :], 0.0)
                nc.gpsimd.memset(aoT[96:128, :, :], 0.0)
                # constant-1 row: proj picks up b_proj from wp_pad[0][48]
                nc.gpsimd.dma_start(aoT[48:49, 0, :], ones_d[:, :])

            def evac(engine, dst, src):
                if engine == "a":
                    nc.scalar.copy(dst, src)
                else:
                    nc.vector.tensor_copy(dst, src)

            def load_xt(b, split=False):
                xt = xt_pool.tile([128, 3, S], BF16)
                src = xT_d[b].rearrange("(c p) s -> p c s", p=128)
                if split:
                    # startup: two sync-queue halves so the first projection
                    # matmuls only wait for the first half
                    nc.sync.dma_start(xt[:, :, 0:512], src[:, :, 0:512])
                    nc.sync.dma_start(xt[:, :, 512:S], src[:, :, 512:S])
                else:
                    nc.sync.dma_start(xt[:, :, :], src)
                return xt

            def _qk_mm(xt, qk, pair, half, dstps):
                ncol = qk * WPADQ + pair * 128
                for ci in range(3):
                    nc.tensor.matmul(
                        dstps,
                        w_sb[:, ci, ncol : ncol + 128],
                        xt[:, ci, half * 512 : (half + 1) * 512],
                        start=(ci == 0),
                        stop=(ci == 2),
                    )

            def _v_mm(xt, st, dstps):
                for ci in range(3):
                    nc.tensor.matmul(
                        dstps,
                        xt[:, ci, st * 128 : (st + 1) * 128],
                        w_sb[:, ci, 2 * WPADQ : 2 * WPADQ + C],
                        start=(ci == 0),
                        stop=(ci == 2),
                    )

            def proj_units(b, xt):
                """QKV projection for sample b as narrow issue-units over the
                two rotating psP halves."""
                qT, kT, vv = qTs[b % 2], kTs[b % 2], vvs[b % 2]

                def qk_unit(qk, pair, half, eng):
                    def u():
                        dst = qT if qk == 0 else kT
                        ps = prj_slot()
                        _qk_mm(xt, qk, pair, half, ps)
                        evac(
                            eng,
                            dst[:, pair, half * 512 : (half + 1) * 512],
                            ps,
                        )

                    return u

                def v_unit(st, eng):
                    def u():
                        ps = prj_slot()
                        _v_mm(xt, st, ps[:, 0:C])
                        evac(
                            eng,
                            vv[:, st, :, 0:D],
                            ps[:, 0:C].rearrange("p (h d) -> p h d", h=H),
                        )

                    return u

                ei = iter(QK_EVAC)
                qkp = {}
                for qk in range(2):
                    for pair in range(4):
                        for half in range(2):
                            qkp[(qk, pair, half)] = qk_unit(qk, pair, half, next(ei))
                vi = iter(V_EVAC)
                v_units = [v_unit(st, next(vi)) for st in range(NQT)]
                # early: Q/K pairs 0-1 + all V (needed by heads 0-3 and @V);
                # late: Q/K pairs 2-3, deferrable into the next attention's
                # own filler list (pair 2 first used by head 4).
                early, late = [], []
                for (qk, pair, half), u in qkp.items():
                    (early if pair < 2 else late).append(u)
                early.extend(v_units)
                return early, late

            def attention(b, fillers=(), tail_div=False, must=()):
                """Per-head attention; after each head, drain a few filler
                units (next sample's projection / previous sample's output
                projection) so PE has work during the divide chains.
                `must` units carry this sample's own deferred Q/K pair-2/3
                projection: they MUST all be issued by the end of head 2,
                before head 4 issues reads of those qT/kT sections (the tile
                framework resolves dependencies in issue order).
                tail_div: the last head's divide runs in q-tile pairs so the
                final output projection unblocks progressively."""
                fillers = list(must) + list(fillers)
                n_must = len(must)
                fi = 0
                qT, kT, vv, aoT = qTs[b % 2], kTs[b % 2], vvs[b % 2], aoTs[b % 2]
                for h in range(H):
                    div_ranges = (
                        [(0, 2), (2, 4), (4, 6), (6, 8)]
                        if (tail_div and h == H - 1)
                        else DIV_RANGES
                    )
                    pair, sub = divmod(h, 2)
                    p0 = sub * 64
                    expT = expTs[h % 2]
                    den_sb = dens[h % 2]
                    # ---- QK^T band + exp (3 chunk-groups); masks for g0/g1
                    # issue here, g2's mask issues later so DVE's in-order
                    # queue does not park the divides behind it.
                    def mask_mul(gi):
                        gbase, gw = GRP_BASE[gi], GRP_W[gi]
                        tt = (
                            nc.vector.tensor_tensor
                            if MASK_ENG[h][gi] == "d"
                            else nc.gpsimd.tensor_tensor
                        )
                        tt(
                            expT[:, gbase : gbase + gw],
                            expT[:, gbase : gbase + gw],
                            m_sb[:, gbase : gbase + gw],
                            mybir.AluOpType.mult,
                        )

                    for gi, grp in enumerate(CH_GROUPS):
                        gbase, gw = GRP_BASE[gi], GRP_W[gi]
                        pat = ps_pat.tile([128, 1024], F32, tag="attn")
                        for c in grp:
                            lo = OFFC[c] - gbase
                            hi = lo + WC[c]
                            a = lo
                            while a < hi:
                                b2 = min(hi, (a // 512 + 1) * 512)
                                nc.tensor.matmul(
                                    pat[:, a:b2],
                                    kT[p0 : p0 + D, pair, c * 128 : (c + 1) * 128],
                                    qT[
                                        p0 : p0 + D,
                                        pair,
                                        QLO[c] + (a - lo) : QLO[c] + (b2 - lo),
                                    ],
                                    start=True,
                                    stop=True,
                                )
                                a = b2
                        nc.scalar.activation(
                            expT[:, gbase : gbase + gw],
                            pat[:, 0:gw],
                            mybir.ActivationFunctionType.Exp,
                            scale=SCALE,
                        )
                        if gi < 2:
                            mask_mul(gi)

                    # ---- @V with ones rows at 64:112 -> denominators,
                    # interleaved with the per-range normalizes so divides
                    # start the moment their q-tiles close.
                    po_q = po[:, :].rearrange(
                        "p (tlo thi u) -> p thi tlo u", tlo=2, thi=4, u=128
                    )
                    den_flat = den_sb[:, :, :].rearrange("p a b -> p (a b)")

                    def av_chunk(c):
                        lhsT = vv[:, c, h, :]
                        for t in range(max(c - 1, 0), min(c + 2, NQT)):
                            pc = PO_COL[t]
                            qs = max(128 * t, QLO[c])
                            qe = min(128 * t + 128, QHI[c])
                            nc.tensor.matmul(
                                po[:, pc + (qs - 128 * t) : pc + (qe - 128 * t)],
                                lhsT,
                                expT[:, OFFC[c] + (qs - QLO[c]) : OFFC[c] + (qe - QLO[c])],
                                start=(c == max(t - 1, 0)),
                                stop=(c == min(t + 1, NQT - 1)),
                            )

                    def divide(ta, tb):
                        nt = (tb - ta) // 2
                        den_v = den_flat[:, ta * 128 : tb * 128].rearrange(
                            "p (thi tlo u) -> p thi tlo u", thi=nt, tlo=2, u=128
                        )
                        ao_v = aoT[
                            p0 : p0 + D, pair, ta * 128 : tb * 128
                        ].rearrange("p (thi tlo u) -> p thi tlo u", thi=nt, tlo=2, u=128)
                        nc.vector.reciprocal(
                            den_v, po_q[64 : 64 + D, ta // 2 : tb // 2]
                        )
                        nc.vector.tensor_tensor(
                            ao_v,
                            po_q[0:D, ta // 2 : tb // 2],
                            den_v,
                            mybir.AluOpType.mult,
                        )

                    g2_start = CH_GROUPS[2][0]
                    for c in range(NQT):
                        if c == g2_start:
                            mask_mul(2)  # just before @V enters group 2
                        av_chunk(c)
                        for ta, tb in div_ranges:
                            # tile tb-1 stops at chunk min(tb, NQT-1)
                            if min(tb, NQT - 1) == c:
                                divide(ta, tb)
                    # drain filler units evenly across heads.  The deferred
                    # own-projection units sit at the head of the list: the
                    # drain targets guarantee they are issued by the end of
                    # head 2, before head 4 issues reads of qT/kT pairs 2-3
                    # (the tile framework resolves deps in issue order).
                    target = (h + 1) * len(fillers) // H
                    if h == 2 and n_must > target:
                        target = n_must
                    while fi < target:
                        fillers[fi]()
                        fi += 1

            def _op_mm(aoT, st, dstps):
                for p in range(4):
                    nc.tensor.matmul(
                        dstps,
                        aoT[:, p, st * 128 : (st + 1) * 128],
                        wp_sb[:, p, :],
                        start=(p == 0),
                        stop=(p == 3),
                    )

            def out_proj_units(b):
                aoT = aoTs[b % 2]
                units = []

                def st_unit(st, eng):
                    def u():
                        ot = out_pool.tile([128, C], F32, tag="ot", name=f"ot{b}_{st}")
                        ps = prj_slot()
                        _op_mm(aoT, st, ps[:, 0:C])
                        evac(eng, ot[:, :], ps[:, 0:C])
                        nc.sync.dma_start(
                            out_d[b, st * 128 : (st + 1) * 128, :], ot[:, :]
                        )

                    return u

                oi = iter(OUT_EVAC)
                for st in range(NQT):
                    units.append(st_unit(st, next(oi)))
                return units

            def out_proj_tail(b):
                # final sample: narrow evacs on alternating engines so the
                # drain is not serialized on one engine or on psP
                aoT = aoTs[b % 2]
                for sp in range(NQT // 2):
                    ot = out_pool.tile([128, 2, C], F32)
                    for k, eng in ((0, "a"), (1, "d")):
                        st = 2 * sp + k
                        ps = prj_slot()
                        _op_mm(aoT, st, ps[:, 0:C])
                        evac(eng, ot[:, k, :], ps[:, 0:C])
                    nc.sync.dma_start(
                        out_d[b, 2 * sp * 128 : (2 * sp + 2) * 128, :].rearrange(
                            "(k p) c -> p k c", k=2
                        ),
                        ot[:, :, :],
                    )

            # ---------------- main pipeline: attention(b) runs interleaved
            # with filler units: the tail of sample b's own projection
            # (Q/K pairs 2-3, first used by head 4), proj(b+1)'s early
            # units, and out_proj(b-1).
            setup_set(0)
            xts = {0: load_xt(0, split=True)}
            early0, late0 = proj_units(0, xts[0])
            for u in early0:
                u()
            load_consts()
            setup_set(1)
            carry = late0 if DEFER_LATE else []
            if not DEFER_LATE:
                for u in late0:
                    u()
            for b in range(BL):
                must = list(carry)
                carry = []
                fillers = []
                if b + 1 < BL:
                    xts[b + 1] = load_xt(b + 1)  # prefetch during attention(b)
                    early, late = proj_units(b + 1, xts[b + 1])
                    fillers += early
                    if DEFER_LATE:
                        carry = late
                    else:
                        fillers += late
                if b > 0:
                    # interleave the previous sample's output-projection units
                    # among the projection units (round-robin) so each drain
                    # point mixes both kinds of work
                    ou = out_proj_units(b - 1)
                    mixed = []
                    k = max(1, len(fillers) // max(1, len(ou)))
                    oi2 = iter(ou)
                    for idx, u in enumerate(fillers):
                        mixed.append(u)
                        if idx % k == k - 1:
                            nu = next(oi2, None)
                            if nu is not None:
                                mixed.append(nu)
                    mixed.extend(oi2)
                    fillers = mixed
                attention(b, fillers, tail_div=(b == BL - 1), must=must)
            out_proj_tail(BL - 1)

    nc.finalize()
    _CACHE["nc"] = nc
    return nc


# ---------------------------------------------------------------- host wrapper
def _np_bf16(a):
    import ml_dtypes

    return np.asarray(a, dtype=ml_dtypes.bfloat16)


def _build_m01(mask):
    """[128, BAND_W] banded 0/1 mask in exact-band layout (rows = key within
    chunk c, cols = q in [QLO[c], QHI[c]))."""
    mp = np.asarray(mask)[np.ix_(PERM, PERM)]
    good = np.isfinite(mp) & (mp == 0.0)
    m01 = np.zeros((128, BAND_W), np.float32)
    covered = 0
    for c in range(NQT):
        blk = good[QLO[c] : QHI[c], c * 128 : (c + 1) * 128]  # [q, k]
        m01[:, OFFC[c] : OFFC[c] + WC[c]] = blk.T.astype(np.float32)
        covered += int(blk.sum())
    assert covered == int(good.sum()), "mask not covered by band layout"
    return m01


def _pad_wqkv(w_qkv):
    """[384, 1152] -> [384, 1408]: Q/K head h at cols h*64..h*64+48 (zero pad),
    V kept natural at cols 1024:1408."""
    out = np.zeros((C, WQW), np.float32)
    for sec in range(2):  # Q, K
        for h in range(H):
            out[:, sec * WPADQ + h * 64 : sec * WPADQ + h * 64 + D] = w_qkv[
                :, sec * C + h * D : sec * C + (h + 1) * D
            ]
    out[:, 2 * WPADQ :] = w_qkv[:, 2 * C :]
    return out


def _pad_wproj(w_proj, b_proj):
    """[384, 384] -> [4, 128, 384]: pair p rows 0:48 = head 2p, 64:112 = head 2p+1.
    Row 48 of pair 0 carries b_proj (matched by the constant-1 row in aoT)."""
    out = np.zeros((4, 128, C), np.float32)
    for p in range(4):
        out[p, 0:D] = w_proj[(2 * p) * D : (2 * p + 1) * D]
        out[p, 64 : 64 + D] = w_proj[(2 * p + 1) * D : (2 * p + 2) * D]
    out[0, D] = b_proj
    return out


def kernel(x, w_qkv, w_proj, b_proj, mask):
    x = np.asarray(x, np.float32)
    w_qkv = np.asarray(w_qkv, np.float32)
    w_proj = np.asarray(w_proj, np.float32)
    b_proj = np.asarray(b_proj, np.float32)

    nc = _build()

    xT = _np_bf16(np.ascontiguousarray(x[:, PERM, :].transpose(0, 2, 1)))  # [B, C, S']
    wq_pad = _np_bf16(_pad_wqkv(w_qkv))
    wp_pad = _np_bf16(_pad_wproj(w_proj, b_proj))
    ones_row = _np_bf16(np.ones((1, S), np.float32))
    m01 = _build_m01(mask)
    if PD == BF16:
        m01 = _np_bf16(m01)

    in_maps = [
        {
            "xT": xT[c * BL : (c + 1) * BL],
            "wq_pad": wq_pad,
            "wp_pad": wp_pad,
            "ones_row": ones_row,
            "m01": m01,
        }
        for c in range(N_CORES)
    ]
    res = run_bass_kernel_spmd(nc, in_maps, list(range(N_CORES)))
    out_p = np.concatenate([res.results[c]["out"] for c in range(N_CORES)], axis=0)
    out = np.empty_like(out_p)
    out[:, PERM, :] = out_p
    return out


# revision 60
# speedup vs baseline: 1.4467x; 1.0087x over previous
"""Trainium2 Bass kernel for nn_Attention_13700945674736 (sparse local-window attention).

Strategy (8 NeuronCores, data-parallel over batch, 4 samples/core):
  - Permute the sequence axis s = 64*i + j  ->  s' = 16*j + i (image transpose).
    The 7x11 local window becomes a 1-D band |ds'| <= 83, so each 128-query
    tile only attends to 3 aligned 128-key chunks instead of 1024.
  - Exact per-chunk band is [128c-80, 128c+208): pairs at distance 81..83
    need |dj|=5 AND |di|>=1 simultaneously, which the 2-D mask forbids for
    the first/last 3 keys of a chunk, so halfwidth 80 suffices on the low
    side (and symmetrically 79+128 on the high side; 208 = 128+80).
  - Heads are padded to 64-partition slots (host-padded weights) so every
    engine access pattern starts at a 32-aligned partition.
  - All matmul operands are bf16; PSUM accumulation and softmax reductions
    stay fp32.
  - attnT[k, q] band tiles; exp on ScalarE (3 chunk-groups per head -> few,
    large activations); binary window mask applied multiplicatively on DVE;
    @V uses lhsT=[V|0|ones|0] so softmax denominators land at partitions
    64:112 of the same PSUM tile; the projection bias is folded in via a
    constant-1 row of aoT.
  - po is a single persistent 2-bank PSUM tile; q-tile t of every head uses
    column (t%2)*512+(t//2)*128 so concurrently-open accumulation groups
    alternate banks.  Closed tiles keep their data across same-bank
    start=True (only has_written bits are cleared), so the per-head divides
    run in 2 halves while the next head's @V already accumulates.
  - PSUM budget: pat 2x[128,1024] (4 banks) + proj 2x[128,512] (2 banks)
    + po [128,1024] (2 banks) = 8 banks exactly.
  - Evacuation copies (Q/K/V/out PSUM->SBUF) are split between ScalarE and
    DVE by static schedule tables; output stores go through the SP queue so
    DMA setup does not stall the Activation sequencer.
  - Software pipelining: attention(b) drains "filler" issue-units after each
    head -- sample b's own deferred Q/K pair-2/3 projection, proj(b+1)'s
    units, and out_proj(b-1)'s units (round-robin mixed) -- so the PE always
    has independent matmul work while the per-head softmax-divide chains
    (@V -> reciprocal -> multiply, serialized by the shared po tile) drain
    on DVE.  The last sample's final head divides in q-tile pairs so the
    closing output projection unblocks progressively.
"""

import sys

sys.path.insert(0, "/opt/trn_rl_repo")

import numpy as np

import concourse.bass as bass
from concourse import bacc
import concourse.mybir as mybir
import concourse.tile as tile
from concourse.bass_utils import run_bass_kernel_spmd

# ---------------------------------------------------------------- constants
B, S, C = 32, 1024, 384
H, D = 8, 48
HI, WI = 16, 64
N_CORES = 8
BL = B // N_CORES  # samples per core
SCALE = float(D) ** -0.5
F32 = mybir.dt.float32
BF16 = mybir.dt.bfloat16
PD = BF16  # precision of expT / m01 / vv

# s' = 16*j + i  <->  s = 64*i + j ;  PERM[s'] = s
_sp = np.arange(S)
PERM = (_sp % HI) * WI + (_sp // HI)

NQT = S // 128  # 8 query tiles (and key chunks)
WPADQ = 64 * H  # padded Q (and K) section width: 512
WQW = 2 * WPADQ + C  # 1408

# exact per-chunk bands: key-chunk c attends to queries [QLO[c], QHI[c])
QLO = [max(0, 128 * c - 80) for c in range(NQT)]
QHI = [min(S, 128 * c + 208) for c in range(NQT)]
WC = [QHI[c] - QLO[c] for c in range(NQT)]
OFFC = list(np.cumsum([0] + WC[:-1]))
BAND_W = sum(WC)  # 2144

# chunk groups per PSUM pat tile (each group width <= 1024 f32 = 2 banks)
CH_GROUPS = [(0, 1, 2), (3, 4, 5), (6, 7)]
GRP_BASE = [OFFC[g[0]] for g in CH_GROUPS]
GRP_W = [OFFC[g[-1]] + WC[g[-1]] - OFFC[g[0]] for g in CH_GROUPS]
assert max(GRP_W) <= 1024

# po column slot for q-tile t (identical for every head; open groups t,t+1
# always land in different PSUM banks)
PO_COL = [(t % 2) * 512 + (t // 2) * 128 for t in range(NQT)]

# ------------------------------------------------- engine schedule tables
# 'a' = ScalarE(Act) copy, 'd' = DVE tensor_copy
QK_EVAC = "aadaadaadaadaada"  # 16 per sample (Q then K, pair-major)
V_EVAC = "aaadaaaa"  # 8 per sample
OUT_EVAC = "aaaaaaaa"  # 8 per sample
# 'd' = DVE, 'p' = Pool(GPSIMD) for the 3 mask multiplies of each head.
# g1 (chunks 3-5) sits mid-head where its latency hides; g0/g2 gate the
# @V start/tail chains, so they stay on the fast DVE.
# heads 6-7 push their mid/tail mask groups to Pool: it relieves DVE right
# before the end-of-sample divide burst
MASK_ENG = ["ddd"] * 6 + ["dpp", "dpp"]
# divide granularity: q-tile ranges; later ranges close later, and finer
# tail pieces release the po columns the next head's early @V chunks need.
DIV_RANGES = [(0, 8)]
# defer Q/K pairs 2-3 of proj(b) into attention(b)'s own filler list
DEFER_LATE = True

# ---------------------------------------------------------------- bass program
_CACHE = {}


def _build():
    if "nc" in _CACHE:
        return _CACHE["nc"]

    nc = bacc.Bacc(None, target_bir_lowering=False)
    xT_d = nc.declare_dram_parameter("xT", [BL, C, S], BF16, isOutput=False)
    wq_d = nc.declare_dram_parameter("wq_pad", [C, WQW], BF16, isOutput=False)
    wp_d = nc.declare_dram_parameter("wp_pad", [4, 128, C], BF16, isOutput=False)
    ones_d = nc.declare_dram_parameter("ones_row", [1, S], BF16, isOutput=False)
    m_d = nc.declare_dram_parameter("m01", [128, BAND_W], PD, isOutput=False)
    out_d = nc.declare_dram_parameter("out", [BL, S, C], F32, isOutput=True)

    with tile.TileContext(nc) as tc:
        with (
            tc.tile_pool(name="singles", bufs=1) as singles,
            tc.tile_pool(name="xt_pool", bufs=3) as xt_pool,
            tc.tile_pool(name="out_pool", bufs=4) as out_pool,
            tc.tile_pool(name="ps_proj", bufs=2, space="PSUM") as ps_proj,
            tc.tile_pool(name="ps_pat", bufs=2, space="PSUM") as ps_pat,
            tc.tile_pool(name="ps_po", bufs=1, space="PSUM") as ps_po,
        ):
            # ---- constants.  Q/K/V weight sections load as separate DMAs so
            # the first projection matmuls only wait for their own section.
            w_sb = singles.tile([128, 3, WQW], BF16)
            wq_v = wq_d.rearrange("(c p) w -> p c w", p=128)
            nc.scalar.dma_start(w_sb[:, :, 0:WPADQ], wq_v[:, :, 0:WPADQ])
            nc.scalar.dma_start(
                w_sb[:, :, WPADQ : 2 * WPADQ], wq_v[:, :, WPADQ : 2 * WPADQ]
            )
            nc.scalar.dma_start(w_sb[:, :, 2 * WPADQ :], wq_v[:, :, 2 * WPADQ :])
            wp_sb = singles.tile([128, 4, C], BF16)
            m_sb = singles.tile([128, BAND_W], PD)

            def load_consts():
                # issued after proj(0) so these transfers cannot jump ahead
                # of the startup-critical xt/wQ DMAs on the DMA engines
                nc.sync.dma_start(m_sb, m_d[:, :])
                nc.sync.dma_start(wp_sb[:, :, :], wp_d.rearrange("f p c -> p f c"))

            # persistent attention-output PSUM tile (2 banks)
            po = ps_po.tile([128, S], F32, tag="po")
            _prj_n = [0]

            def prj_slot():
                _prj_n[0] += 1
                ps = ps_proj.tile([128, 512], F32, tag="mm", name=f"prj{_prj_n[0]}")
                return ps

            # ---- per-sample tiles, double-buffered for cross-sample overlap
            qTs, kTs, vvs, aoTs, expTs, dens = [], [], [], [], [], []
            for i in range(2):
                qTs.append(singles.tile([128, 4, S], BF16, name=f"qT{i}"))
                kTs.append(singles.tile([128, 4, S], BF16, name=f"kT{i}"))
                vvs.append(singles.tile([128, NQT, H, 128], PD, name=f"vv{i}"))
                aoTs.append(singles.tile([128, 4, S], BF16, name=f"aoT{i}"))
                expTs.append(singles.tile([128, BAND_W], PD, name=f"expT{i}"))
                dens.append(singles.tile([48, 2, 512], F32, name=f"den{i}"))
            def setup_set(i):
                """One-time constant sections of buffer set i (Pool engine).
                Set 1 is deferred until after proj(0) is issued so sample-0
                mask multiplies are not queued behind 20us of memsets."""
                vv, aoT = vvs[i], aoTs[i]
                nc.gpsimd.memset(vv[:, :, :, D : D + 16], 0.0)
                nc.gpsimd.memset(vv[:, :, :, D + 16 : 112], 1.0)
                nc.gpsimd.memset(vv[:, :, :, 112:128], 0.0)
                # zero dead rows (48:64, 112:128); starts must be 32-aligned so
                # cover 32:64 / 96:128 — live rows are rewritten by the divides.
                nc.gpsimd.memset(aoT[32:64, :, :], 0.0)
                nc.gpsimd.memset(aoT[96:128, :, :], 0.0)
                # constant-1 row: proj picks up b_proj from wp_pad[0][48]
                nc.gpsimd.dma_start(aoT[48:49, 0, :], ones_d[:, :])

            def evac(engine, dst, src):
                if engine == "a":
                    nc.scalar.copy(dst, src)
                else:
                    nc.vector.tensor_copy(dst, src)

            def load_xt(b, split=False):
                xt = xt_pool.tile([128, 3, S], BF16)
                src = xT_d[b].rearrange("(c p) s -> p c s", p=128)
                if split:
                    # startup: two sync-queue halves so the first projection
                    # matmuls only wait for the first half
                    nc.sync.dma_start(xt[:, :, 0:512], src[:, :, 0:512])
                    nc.sync.dma_start(xt[:, :, 512:S], src[:, :, 512:S])
                else:
                    nc.sync.dma_start(xt[:, :, :], src)
                return xt

            def _qk_mm(xt, qk, pair, half, dstps):
                ncol = qk * WPADQ + pair * 128
                for ci in range(3):
                    nc.tensor.matmul(
                        dstps,
                        w_sb[:, ci, ncol : ncol + 128],
                        xt[:, ci, half * 512 : (half + 1) * 512],
                        start=(ci == 0),
                        stop=(ci == 2),
                    )

            def _v_mm(xt, st, dstps):
                for ci in range(3):
                    nc.tensor.matmul(
                        dstps,
                        xt[:, ci, st * 128 : (st + 1) * 128],
                        w_sb[:, ci, 2 * WPADQ : 2 * WPADQ + C],
                        start=(ci == 0),
                        stop=(ci == 2),
                    )

            def proj_units(b, xt):
                """QKV projection for sample b as narrow issue-units over the
                two rotating psP halves."""
                qT, kT, vv = qTs[b % 2], kTs[b % 2], vvs[b % 2]

                def qk_unit(qk, pair, half, eng):
                    def u():
                        dst = qT if qk == 0 else kT
                        ps = prj_slot()
                        _qk_mm(xt, qk, pair, half, ps)
                        evac(
                            eng,
                            dst[:, pair, half * 512 : (half + 1) * 512],
                            ps,
                        )

                    return u

                def v_unit(st, eng):
                    def u():
                        ps = prj_slot()
                        _v_mm(xt, st, ps[:, 0:C])
                        evac(
                            eng,
                            vv[:, st, :, 0:D],
                            ps[:, 0:C].rearrange("p (h d) -> p h d", h=H),
                        )

                    return u

                ei = iter(QK_EVAC)
                qkp = {}
                for qk in range(2):
                    for pair in range(4):
                        for half in range(2):
                            qkp[(qk, pair, half)] = qk_unit(qk, pair, half, next(ei))
                vi = iter(V_EVAC)
                v_units = [v_unit(st, next(vi)) for st in range(NQT)]
                # early: Q/K pairs 0-1 + all V (needed by heads 0-3 and @V);
                # late: Q/K pairs 2-3, deferrable into the next attention's
                # own filler list (pair 2 first used by head 4).
                early, late = [], []
                for (qk, pair, half), u in qkp.items():
                    (early if pair < 2 else late).append(u)
                early.extend(v_units)
                return early, late

            def attention(b, fillers=(), tail_div=False, must=()):
                """Per-head attention; after each head, drain a few filler
                units (next sample's projection / previous sample's output
                projection) so PE has work during the divide chains.
                `must` units carry this sample's own deferred Q/K pair-2/3
                projection: they MUST all be issued by the end of head 2,
                before head 4 issues reads of those qT/kT sections (the tile
                framework resolves dependencies in issue order).
                tail_div: the last head's divide runs in q-tile pairs so the
                final output projection unblocks progressively."""
                fillers = list(must) + list(fillers)
                n_must = len(must)
                fi = 0
                qT, kT, vv, aoT = qTs[b % 2], kTs[b % 2], vvs[b % 2], aoTs[b % 2]
                for h in range(H):
                    div_ranges = (
                        [(0, 2), (2, 4), (4, 6), (6, 8)]
                        if (tail_div and h == H - 1)
                        else DIV_RANGES
                    )
                    pair, sub = divmod(h, 2)
                    p0 = sub * 64
                    expT = expTs[h % 2]
                    den_sb = dens[h % 2]
                    # ---- QK^T band + exp (3 chunk-groups); masks for g0/g1
                    # issue here, g2's mask issues later so DVE's in-order
                    # queue does not park the divides behind it.
                    def mask_mul(gi):
                        gbase, gw = GRP_BASE[gi], GRP_W[gi]
                        tt = (
                            nc.vector.tensor_tensor
                            if MASK_ENG[h][gi] == "d"
                            else nc.gpsimd.tensor_tensor
                        )
                        tt(
                            expT[:, gbase : gbase + gw],
                            expT[:, gbase : gbase + gw],
                            m_sb[:, gbase : gbase + gw],
                            mybir.AluOpType.mult,
                        )

                    for gi, grp in enumerate(CH_GROUPS):
                        gbase, gw = GRP_BASE[gi], GRP_W[gi]
                        pat = ps_pat.tile([128, 1024], F32, tag="attn")
                        for c in grp:
                            lo = OFFC[c] - gbase
                            hi = lo + WC[c]
                            a = lo
                            while a < hi:
                                b2 = min(hi, (a // 512 + 1) * 512)
                                nc.tensor.matmul(
                                    pat[:, a:b2],
                                    kT[p0 : p0 + D, pair, c * 128 : (c + 1) * 128],
                                    qT[
                                        p0 : p0 + D,
                                        pair,
                                        QLO[c] + (a - lo) : QLO[c] + (b2 - lo),
                                    ],
                                    start=True,
                                    stop=True,
                                )
                                a = b2
                        nc.scalar.activation(
                            expT[:, gbase : gbase + gw],
                            pat[:, 0:gw],
                            mybir.ActivationFunctionType.Exp,
                            scale=SCALE,
                        )
                        if gi < 2:
                            mask_mul(gi)

                    # ---- @V with ones rows at 64:112 -> denominators,
                    # interleaved with the per-range normalizes so divides
                    # start the moment their q-tiles close.
                    po_q = po[:, :].rearrange(
                        "p (tlo thi u) -> p thi tlo u", tlo=2, thi=4, u=128
                    )
                    den_flat = den_sb[:, :, :].rearrange("p a b -> p (a b)")

                    def av_chunk(c):
                        lhsT = vv[:, c, h, :]
                        for t in range(max(c - 1, 0), min(c + 2, NQT)):
                            pc = PO_COL[t]
                            qs = max(128 * t, QLO[c])
                            qe = min(128 * t + 128, QHI[c])
                            nc.tensor.matmul(
                                po[:, pc + (qs - 128 * t) : pc + (qe - 128 * t)],
                                lhsT,
                                expT[:, OFFC[c] + (qs - QLO[c]) : OFFC[c] + (qe - QLO[c])],
                                start=(c == max(t - 1, 0)),
                                stop=(c == min(t + 1, NQT - 1)),
                            )

                    def divide(ta, tb):
                        nt = (tb - ta) // 2
                        den_v = den_flat[:, ta * 128 : tb * 128].rearrange(
                            "p (thi tlo u) -> p thi tlo u", thi=nt, tlo=2, u=128
                        )
                        ao_v = aoT[
                            p0 : p0 + D, pair, ta * 128 : tb * 128
                        ].rearrange("p (thi tlo u) -> p thi tlo u", thi=nt, tlo=2, u=128)
                        nc.vector.reciprocal(
                            den_v, po_q[64 : 64 + D, ta // 2 : tb // 2]
                        )
                        nc.vector.tensor_tensor(
                            ao_v,
                            po_q[0:D, ta // 2 : tb // 2],
                            den_v,
                            mybir.AluOpType.mult,
                        )

                    g2_start = CH_GROUPS[2][0]
                    for c in range(NQT):
                        if c == g2_start:
                            mask_mul(2)  # just before @V enters group 2
                        av_chunk(c)
                        for ta, tb in div_ranges:
                            # tile tb-1 stops at chunk min(tb, NQT-1)
                            if min(tb, NQT - 1) == c:
                                divide(ta, tb)
                    # drain filler units evenly across heads.  The deferred
                    # own-projection units sit at the head of the list: the
                    # drain targets guarantee they are issued by the end of
                    # head 2, before head 4 issues reads of qT/kT pairs 2-3
                    # (the tile framework resolves deps in issue order).
                    if len(fillers) >= 20:
                        target = (h + 1) * len(fillers) // H
                    else:
                        # thin filler supply (last sample): save units for the
                        # late heads where the divide chains stall hardest
                        target = ((h + 1) * (h + 2) * len(fillers)) // (H * (H + 1))
                    if h == 2 and n_must > target:
                        target = n_must
                    while fi < target:
                        fillers[fi]()
                        fi += 1

            def _op_mm(aoT, st, dstps):
                for p in range(4):
                    nc.tensor.matmul(
                        dstps,
                        aoT[:, p, st * 128 : (st + 1) * 128],
                        wp_sb[:, p, :],
                        start=(p == 0),
                        stop=(p == 3),
                    )

            def out_proj_units(b):
                aoT = aoTs[b % 2]
                units = []

                def st_unit(st, eng):
                    def u():
                        ot = out_pool.tile([128, C], F32, tag="ot", name=f"ot{b}_{st}")
                        ps = prj_slot()
                        _op_mm(aoT, st, ps[:, 0:C])
                        evac(eng, ot[:, :], ps[:, 0:C])
                        nc.sync.dma_start(
                            out_d[b, st * 128 : (st + 1) * 128, :], ot[:, :]
                        )

                    return u

                oi = iter(OUT_EVAC)
                for st in range(NQT):
                    units.append(st_unit(st, next(oi)))
                return units

            def out_proj_tail(b):
                # final sample: narrow evacs on alternating engines so the
                # drain is not serialized on one engine or on psP
                aoT = aoTs[b % 2]
                for sp in range(NQT // 2):
                    ot = out_pool.tile([128, 2, C], F32)
                    for k, eng in ((0, "a"), (1, "d")):
                        st = 2 * sp + k
                        ps = prj_slot()
                        _op_mm(aoT, st, ps[:, 0:C])
                        evac(eng, ot[:, k, :], ps[:, 0:C])
                    nc.sync.dma_start(
                        out_d[b, 2 * sp * 128 : (2 * sp + 2) * 128, :].rearrange(
                            "(k p) c -> p k c", k=2
                        ),
                        ot[:, :, :],
                    )

            # ---------------- main pipeline: attention(b) runs interleaved
            # with filler units: the tail of sample b's own projection
            # (Q/K pairs 2-3, first used by head 4), proj(b+1)'s early
            # units, and out_proj(b-1).
            setup_set(0)
            xts = {0: load_xt(0, split=True)}
            early0, late0 = proj_units(0, xts[0])
            for u in early0:
                u()
            load_consts()
            setup_set(1)
            carry = late0 if DEFER_LATE else []
            if not DEFER_LATE:
                for u in late0:
                    u()
            for b in range(BL):
                must = list(carry)
                carry = []
                fillers = []
                if b + 1 < BL:
                    xts[b + 1] = load_xt(b + 1)  # prefetch during attention(b)
                    early, late = proj_units(b + 1, xts[b + 1])
                    fillers += early
                    if DEFER_LATE:
                        carry = late
                    else:
                        fillers += late
                if b > 0:
                    # interleave the previous sample's output-projection units
                    # among the projection units (round-robin) so each drain
                    # point mixes both kinds of work
                    ou = out_proj_units(b - 1)
                    mixed = []
                    k = max(1, len(fillers) // max(1, len(ou)))
                    oi2 = iter(ou)
                    for idx, u in enumerate(fillers):
                        mixed.append(u)
                        if idx % k == k - 1:
                            nu = next(oi2, None)
                            if nu is not None:
                                mixed.append(nu)
                    mixed.extend(oi2)
                    fillers = mixed
                attention(b, fillers, tail_div=(b == BL - 1), must=must)
            out_proj_tail(BL - 1)

    nc.finalize()
    _CACHE["nc"] = nc
    return nc


# ---------------------------------------------------------------- host wrapper
def _np_bf16(a):
    import ml_dtypes

    return np.asarray(a, dtype=ml_dtypes.bfloat16)


def _build_m01(mask):
    """[128, BAND_W] banded 0/1 mask in exact-band layout (rows = key within
    chunk c, cols = q in [QLO[c], QHI[c]))."""
    mp = np.asarray(mask)[np.ix_(PERM, PERM)]
    good = np.isfinite(mp) & (mp == 0.0)
    m01 = np.zeros((128, BAND_W), np.float32)
    covered = 0
    for c in range(NQT):
        blk = good[QLO[c] : QHI[c], c * 128 : (c + 1) * 128]  # [q, k]
        m01[:, OFFC[c] : OFFC[c] + WC[c]] = blk.T.astype(np.float32)
        covered += int(blk.sum())
    assert covered == int(good.sum()), "mask not covered by band layout"
    return m01


def _pad_wqkv(w_qkv):
    """[384, 1152] -> [384, 1408]: Q/K head h at cols h*64..h*64+48 (zero pad),
    V kept natural at cols 1024:1408."""
    out = np.zeros((C, WQW), np.float32)
    for sec in range(2):  # Q, K
        for h in range(H):
            out[:, sec * WPADQ + h * 64 : sec * WPADQ + h * 64 + D] = w_qkv[
                :, sec * C + h * D : sec * C + (h + 1) * D
            ]
    out[:, 2 * WPADQ :] = w_qkv[:, 2 * C :]
    return out


def _pad_wproj(w_proj, b_proj):
    """[384, 384] -> [4, 128, 384]: pair p rows 0:48 = head 2p, 64:112 = head 2p+1.
    Row 48 of pair 0 carries b_proj (matched by the constant-1 row in aoT)."""
    out = np.zeros((4, 128, C), np.float32)
    for p in range(4):
        out[p, 0:D] = w_proj[(2 * p) * D : (2 * p + 1) * D]
        out[p, 64 : 64 + D] = w_proj[(2 * p + 1) * D : (2 * p + 2) * D]
    out[0, D] = b_proj
    return out


def kernel(x, w_qkv, w_proj, b_proj, mask):
    x = np.asarray(x, np.float32)
    w_qkv = np.asarray(w_qkv, np.float32)
    w_proj = np.asarray(w_proj, np.float32)
    b_proj = np.asarray(b_proj, np.float32)

    nc = _build()

    xT = _np_bf16(np.ascontiguousarray(x[:, PERM, :].transpose(0, 2, 1)))  # [B, C, S']
    wq_pad = _np_bf16(_pad_wqkv(w_qkv))
    wp_pad = _np_bf16(_pad_wproj(w_proj, b_proj))
    ones_row = _np_bf16(np.ones((1, S), np.float32))
    m01 = _build_m01(mask)
    if PD == BF16:
        m01 = _np_bf16(m01)

    in_maps = [
        {
            "xT": xT[c * BL : (c + 1) * BL],
            "wq_pad": wq_pad,
            "wp_pad": wp_pad,
            "ones_row": ones_row,
            "m01": m01,
        }
        for c in range(N_CORES)
    ]
    res = run_bass_kernel_spmd(nc, in_maps, list(range(N_CORES)))
    out_p = np.concatenate([res.results[c]["out"] for c in range(N_CORES)], axis=0)
    out = np.empty_like(out_p)
    out[:, PERM, :] = out_p
    return out


# revision 61
# speedup vs baseline: 1.4581x; 1.0078x over previous
"""Trainium2 Bass kernel for nn_Attention_13700945674736 (sparse local-window attention).

Strategy (8 NeuronCores, data-parallel over batch, 4 samples/core):
  - Permute the sequence axis s = 64*i + j  ->  s' = 16*j + i (image transpose).
    The 7x11 local window becomes a 1-D band |ds'| <= 83, so each 128-query
    tile only attends to 3 aligned 128-key chunks instead of 1024.
  - Exact per-chunk band is [128c-80, 128c+208): pairs at distance 81..83
    need |dj|=5 AND |di|>=1 simultaneously, which the 2-D mask forbids for
    the first/last 3 keys of a chunk, so halfwidth 80 suffices on the low
    side (and symmetrically 79+128 on the high side; 208 = 128+80).
  - Heads are padded to 64-partition slots (host-padded weights) so every
    engine access pattern starts at a 32-aligned partition.
  - All matmul operands are bf16; PSUM accumulation and softmax reductions
    stay fp32.
  - attnT[k, q] band tiles; exp on ScalarE (3 chunk-groups per head -> few,
    large activations); binary window mask applied multiplicatively on DVE;
    @V uses lhsT=[V|0|ones|0] so softmax denominators land at partitions
    64:112 of the same PSUM tile; the projection bias is folded in via a
    constant-1 row of aoT.
  - po is a single persistent 2-bank PSUM tile; q-tile t of every head uses
    column (t%2)*512+(t//2)*128 so concurrently-open accumulation groups
    alternate banks.  Closed tiles keep their data across same-bank
    start=True (only has_written bits are cleared), so the per-head divides
    run in 2 halves while the next head's @V already accumulates.
  - PSUM budget: pat 2x[128,1024] (4 banks) + proj 2x[128,512] (2 banks)
    + po [128,1024] (2 banks) = 8 banks exactly.
  - Evacuation copies (Q/K/V/out PSUM->SBUF) are split between ScalarE and
    DVE by static schedule tables; output stores go through the SP queue so
    DMA setup does not stall the Activation sequencer.
  - Software pipelining: attention(b) drains "filler" issue-units after each
    head -- sample b's own deferred Q/K pair-2/3 projection, proj(b+1)'s
    units, and out_proj(b-1)'s units (round-robin mixed) -- so the PE always
    has independent matmul work while the per-head softmax-divide chains
    (@V -> reciprocal -> multiply, serialized by the shared po tile) drain
    on DVE.  The last sample's final head divides in q-tile pairs so the
    closing output projection unblocks progressively.
"""

import sys

sys.path.insert(0, "/opt/trn_rl_repo")

import numpy as np

import concourse.bass as bass
from concourse import bacc
import concourse.mybir as mybir
import concourse.tile as tile
from concourse.bass_utils import run_bass_kernel_spmd

# ---------------------------------------------------------------- constants
B, S, C = 32, 1024, 384
H, D = 8, 48
HI, WI = 16, 64
N_CORES = 8
BL = B // N_CORES  # samples per core
SCALE = float(D) ** -0.5
F32 = mybir.dt.float32
BF16 = mybir.dt.bfloat16
PD = BF16  # precision of expT / m01 / vv

# s' = 16*j + i  <->  s = 64*i + j ;  PERM[s'] = s
_sp = np.arange(S)
PERM = (_sp % HI) * WI + (_sp // HI)

NQT = S // 128  # 8 query tiles (and key chunks)
WPADQ = 64 * H  # padded Q (and K) section width: 512
WQW = 2 * WPADQ + C  # 1408

# exact per-chunk bands: key-chunk c attends to queries [QLO[c], QHI[c])
QLO = [max(0, 128 * c - 80) for c in range(NQT)]
QHI = [min(S, 128 * c + 208) for c in range(NQT)]
WC = [QHI[c] - QLO[c] for c in range(NQT)]
OFFC = list(np.cumsum([0] + WC[:-1]))
BAND_W = sum(WC)  # 2144

# chunk groups per PSUM pat tile (each group width <= 1024 f32 = 2 banks)
CH_GROUPS = [(0, 1, 2), (3, 4, 5), (6, 7)]
GRP_BASE = [OFFC[g[0]] for g in CH_GROUPS]
GRP_W = [OFFC[g[-1]] + WC[g[-1]] - OFFC[g[0]] for g in CH_GROUPS]
assert max(GRP_W) <= 1024

# po column slot for q-tile t (identical for every head; open groups t,t+1
# always land in different PSUM banks)
PO_COL = [(t % 2) * 512 + (t // 2) * 128 for t in range(NQT)]

# ------------------------------------------------- engine schedule tables
# 'a' = ScalarE(Act) copy, 'd' = DVE tensor_copy
QK_EVAC = "aadaadaadaadaada"  # 16 per sample (Q then K, pair-major)
V_EVAC = "aadaaada"  # 8 per sample
OUT_EVAC = "aaaaaaaa"  # 8 per sample
# 'd' = DVE, 'p' = Pool(GPSIMD) for the 3 mask multiplies of each head.
# g1 (chunks 3-5) sits mid-head where its latency hides; g0/g2 gate the
# @V start/tail chains, so they stay on the fast DVE.
# heads 6-7 push their mid/tail mask groups to Pool: it relieves DVE right
# before the end-of-sample divide burst
MASK_ENG = ["ddd"] * 6 + ["dpp", "dpp"]
# divide granularity: q-tile ranges; later ranges close later, and finer
# tail pieces release the po columns the next head's early @V chunks need.
DIV_RANGES = [(0, 8)]
# defer Q/K pairs 2-3 of proj(b) into attention(b)'s own filler list
DEFER_LATE = True

# ---------------------------------------------------------------- bass program
_CACHE = {}


def _build():
    if "nc" in _CACHE:
        return _CACHE["nc"]

    nc = bacc.Bacc(None, target_bir_lowering=False)
    xT_d = nc.declare_dram_parameter("xT", [BL, C, S], BF16, isOutput=False)
    wq_d = nc.declare_dram_parameter("wq_pad", [C, WQW], BF16, isOutput=False)
    wp_d = nc.declare_dram_parameter("wp_pad", [4, 128, C], BF16, isOutput=False)
    ones_d = nc.declare_dram_parameter("ones_row", [1, S], BF16, isOutput=False)
    m_d = nc.declare_dram_parameter("m01", [128, BAND_W], PD, isOutput=False)
    out_d = nc.declare_dram_parameter("out", [BL, S, C], F32, isOutput=True)

    with tile.TileContext(nc) as tc:
        with (
            tc.tile_pool(name="singles", bufs=1) as singles,
            tc.tile_pool(name="xt_pool", bufs=3) as xt_pool,
            tc.tile_pool(name="out_pool", bufs=4) as out_pool,
            tc.tile_pool(name="ps_proj", bufs=2, space="PSUM") as ps_proj,
            tc.tile_pool(name="ps_pat", bufs=2, space="PSUM") as ps_pat,
            tc.tile_pool(name="ps_po", bufs=1, space="PSUM") as ps_po,
        ):
            # ---- constants.  Q/K/V weight sections load as separate DMAs so
            # the first projection matmuls only wait for their own section.
            w_sb = singles.tile([128, 3, WQW], BF16)
            wq_v = wq_d.rearrange("(c p) w -> p c w", p=128)
            nc.scalar.dma_start(w_sb[:, :, 0:WPADQ], wq_v[:, :, 0:WPADQ])
            nc.scalar.dma_start(
                w_sb[:, :, WPADQ : 2 * WPADQ], wq_v[:, :, WPADQ : 2 * WPADQ]
            )
            nc.scalar.dma_start(w_sb[:, :, 2 * WPADQ :], wq_v[:, :, 2 * WPADQ :])
            wp_sb = singles.tile([128, 4, C], BF16)
            m_sb = singles.tile([128, BAND_W], PD)

            def load_consts():
                # issued after proj(0) so these transfers cannot jump ahead
                # of the startup-critical xt/wQ DMAs on the DMA engines
                nc.sync.dma_start(m_sb, m_d[:, :])
                nc.sync.dma_start(wp_sb[:, :, :], wp_d.rearrange("f p c -> p f c"))

            # persistent attention-output PSUM tile (2 banks)
            po = ps_po.tile([128, S], F32, tag="po")
            _prj_n = [0]

            def prj_slot():
                _prj_n[0] += 1
                ps = ps_proj.tile([128, 512], F32, tag="mm", name=f"prj{_prj_n[0]}")
                return ps

            # ---- per-sample tiles, double-buffered for cross-sample overlap
            qTs, kTs, vvs, aoTs, expTs, dens = [], [], [], [], [], []
            for i in range(2):
                qTs.append(singles.tile([128, 4, S], BF16, name=f"qT{i}"))
                kTs.append(singles.tile([128, 4, S], BF16, name=f"kT{i}"))
                vvs.append(singles.tile([128, NQT, H, 128], PD, name=f"vv{i}"))
                aoTs.append(singles.tile([128, 4, S], BF16, name=f"aoT{i}"))
                expTs.append(singles.tile([128, BAND_W], PD, name=f"expT{i}"))
                dens.append(singles.tile([48, 2, 512], F32, name=f"den{i}"))
            def setup_set(i):
                """One-time constant sections of buffer set i (Pool engine).
                Set 1 is deferred until after proj(0) is issued so sample-0
                mask multiplies are not queued behind 20us of memsets."""
                vv, aoT = vvs[i], aoTs[i]
                nc.gpsimd.memset(vv[:, :, :, D : D + 16], 0.0)
                nc.gpsimd.memset(vv[:, :, :, D + 16 : 112], 1.0)
                nc.gpsimd.memset(vv[:, :, :, 112:128], 0.0)
                # zero dead rows (48:64, 112:128); starts must be 32-aligned so
                # cover 32:64 / 96:128 — live rows are rewritten by the divides.
                nc.gpsimd.memset(aoT[32:64, :, :], 0.0)
                nc.gpsimd.memset(aoT[96:128, :, :], 0.0)
                # constant-1 row: proj picks up b_proj from wp_pad[0][48]
                nc.gpsimd.dma_start(aoT[48:49, 0, :], ones_d[:, :])

            def evac(engine, dst, src):
                if engine == "a":
                    nc.scalar.copy(dst, src)
                else:
                    nc.vector.tensor_copy(dst, src)

            def load_xt(b, split=False):
                xt = xt_pool.tile([128, 3, S], BF16)
                src = xT_d[b].rearrange("(c p) s -> p c s", p=128)
                if split:
                    # startup: two sync-queue halves so the first projection
                    # matmuls only wait for the first half
                    nc.sync.dma_start(xt[:, :, 0:512], src[:, :, 0:512])
                    nc.sync.dma_start(xt[:, :, 512:S], src[:, :, 512:S])
                else:
                    nc.sync.dma_start(xt[:, :, :], src)
                return xt

            def _qk_mm(xt, qk, pair, half, dstps):
                ncol = qk * WPADQ + pair * 128
                for ci in range(3):
                    nc.tensor.matmul(
                        dstps,
                        w_sb[:, ci, ncol : ncol + 128],
                        xt[:, ci, half * 512 : (half + 1) * 512],
                        start=(ci == 0),
                        stop=(ci == 2),
                    )

            def _v_mm(xt, st, dstps):
                for ci in range(3):
                    nc.tensor.matmul(
                        dstps,
                        xt[:, ci, st * 128 : (st + 1) * 128],
                        w_sb[:, ci, 2 * WPADQ : 2 * WPADQ + C],
                        start=(ci == 0),
                        stop=(ci == 2),
                    )

            def proj_units(b, xt):
                """QKV projection for sample b as narrow issue-units over the
                two rotating psP halves."""
                qT, kT, vv = qTs[b % 2], kTs[b % 2], vvs[b % 2]

                def qk_unit(qk, pair, half, eng):
                    def u():
                        dst = qT if qk == 0 else kT
                        ps = prj_slot()
                        _qk_mm(xt, qk, pair, half, ps)
                        evac(
                            eng,
                            dst[:, pair, half * 512 : (half + 1) * 512],
                            ps,
                        )

                    return u

                def v_unit(st, eng):
                    def u():
                        ps = prj_slot()
                        _v_mm(xt, st, ps[:, 0:C])
                        evac(
                            eng,
                            vv[:, st, :, 0:D],
                            ps[:, 0:C].rearrange("p (h d) -> p h d", h=H),
                        )

                    return u

                ei = iter(QK_EVAC)
                qkp = {}
                for qk in range(2):
                    for pair in range(4):
                        for half in range(2):
                            qkp[(qk, pair, half)] = qk_unit(qk, pair, half, next(ei))
                vi = iter(V_EVAC)
                v_units = [v_unit(st, next(vi)) for st in range(NQT)]
                # early: Q/K pairs 0-1 + all V (needed by heads 0-3 and @V);
                # late: Q/K pairs 2-3, deferrable into the next attention's
                # own filler list (pair 2 first used by head 4).
                early, late = [], []
                for (qk, pair, half), u in qkp.items():
                    (early if pair < 2 else late).append(u)
                early.extend(v_units)
                return early, late

            def attention(b, fillers=(), tail_div=False, must=()):
                """Per-head attention; after each head, drain a few filler
                units (next sample's projection / previous sample's output
                projection) so PE has work during the divide chains.
                `must` units carry this sample's own deferred Q/K pair-2/3
                projection: they MUST all be issued by the end of head 2,
                before head 4 issues reads of those qT/kT sections (the tile
                framework resolves dependencies in issue order).
                tail_div: the last head's divide runs in q-tile pairs so the
                final output projection unblocks progressively."""
                fillers = list(must) + list(fillers)
                n_must = len(must)
                fi = 0
                qT, kT, vv, aoT = qTs[b % 2], kTs[b % 2], vvs[b % 2], aoTs[b % 2]
                for h in range(H):
                    div_ranges = (
                        [(0, 2), (2, 4), (4, 6), (6, 8)]
                        if (tail_div and h == H - 1)
                        else DIV_RANGES
                    )
                    pair, sub = divmod(h, 2)
                    p0 = sub * 64
                    expT = expTs[h % 2]
                    den_sb = dens[h % 2]
                    # ---- QK^T band + exp (3 chunk-groups); masks for g0/g1
                    # issue here, g2's mask issues later so DVE's in-order
                    # queue does not park the divides behind it.
                    def mask_mul(gi):
                        gbase, gw = GRP_BASE[gi], GRP_W[gi]
                        tt = (
                            nc.vector.tensor_tensor
                            if MASK_ENG[h][gi] == "d"
                            else nc.gpsimd.tensor_tensor
                        )
                        tt(
                            expT[:, gbase : gbase + gw],
                            expT[:, gbase : gbase + gw],
                            m_sb[:, gbase : gbase + gw],
                            mybir.AluOpType.mult,
                        )

                    for gi, grp in enumerate(CH_GROUPS):
                        gbase, gw = GRP_BASE[gi], GRP_W[gi]
                        pat = ps_pat.tile([128, 1024], F32, tag="attn")
                        for c in grp:
                            lo = OFFC[c] - gbase
                            hi = lo + WC[c]
                            a = lo
                            while a < hi:
                                b2 = min(hi, (a // 512 + 1) * 512)
                                nc.tensor.matmul(
                                    pat[:, a:b2],
                                    kT[p0 : p0 + D, pair, c * 128 : (c + 1) * 128],
                                    qT[
                                        p0 : p0 + D,
                                        pair,
                                        QLO[c] + (a - lo) : QLO[c] + (b2 - lo),
                                    ],
                                    start=True,
                                    stop=True,
                                )
                                a = b2
                        nc.scalar.activation(
                            expT[:, gbase : gbase + gw],
                            pat[:, 0:gw],
                            mybir.ActivationFunctionType.Exp,
                            scale=SCALE,
                        )
                        if gi < 2:
                            mask_mul(gi)

                    # ---- @V with ones rows at 64:112 -> denominators,
                    # interleaved with the per-range normalizes so divides
                    # start the moment their q-tiles close.
                    po_q = po[:, :].rearrange(
                        "p (tlo thi u) -> p thi tlo u", tlo=2, thi=4, u=128
                    )
                    den_flat = den_sb[:, :, :].rearrange("p a b -> p (a b)")

                    def av_chunk(c):
                        lhsT = vv[:, c, h, :]
                        for t in range(max(c - 1, 0), min(c + 2, NQT)):
                            pc = PO_COL[t]
                            qs = max(128 * t, QLO[c])
                            qe = min(128 * t + 128, QHI[c])
                            nc.tensor.matmul(
                                po[:, pc + (qs - 128 * t) : pc + (qe - 128 * t)],
                                lhsT,
                                expT[:, OFFC[c] + (qs - QLO[c]) : OFFC[c] + (qe - QLO[c])],
                                start=(c == max(t - 1, 0)),
                                stop=(c == min(t + 1, NQT - 1)),
                            )

                    def divide(ta, tb):
                        nt = (tb - ta) // 2
                        den_v = den_flat[:, ta * 128 : tb * 128].rearrange(
                            "p (thi tlo u) -> p thi tlo u", thi=nt, tlo=2, u=128
                        )
                        ao_v = aoT[
                            p0 : p0 + D, pair, ta * 128 : tb * 128
                        ].rearrange("p (thi tlo u) -> p thi tlo u", thi=nt, tlo=2, u=128)
                        nc.vector.reciprocal(
                            den_v, po_q[64 : 64 + D, ta // 2 : tb // 2]
                        )
                        nc.vector.tensor_tensor(
                            ao_v,
                            po_q[0:D, ta // 2 : tb // 2],
                            den_v,
                            mybir.AluOpType.mult,
                        )

                    g2_start = CH_GROUPS[2][0]
                    for c in range(NQT):
                        if c == g2_start:
                            mask_mul(2)  # just before @V enters group 2
                        av_chunk(c)
                        for ta, tb in div_ranges:
                            # tile tb-1 stops at chunk min(tb, NQT-1)
                            if min(tb, NQT - 1) == c:
                                divide(ta, tb)
                    # drain filler units evenly across heads.  The deferred
                    # own-projection units sit at the head of the list: the
                    # drain targets guarantee they are issued by the end of
                    # head 2, before head 4 issues reads of qT/kT pairs 2-3
                    # (the tile framework resolves deps in issue order).
                    if len(fillers) >= 20:
                        target = (h + 1) * len(fillers) // H
                    else:
                        # thin filler supply (last sample): save units for the
                        # late heads where the divide chains stall hardest
                        target = ((h + 1) * (h + 2) * len(fillers)) // (H * (H + 1))
                    if h == 2 and n_must > target:
                        target = n_must
                    while fi < target:
                        fillers[fi]()
                        fi += 1

            def _op_mm(aoT, st, dstps):
                for p in range(4):
                    nc.tensor.matmul(
                        dstps,
                        aoT[:, p, st * 128 : (st + 1) * 128],
                        wp_sb[:, p, :],
                        start=(p == 0),
                        stop=(p == 3),
                    )

            def out_proj_units(b):
                aoT = aoTs[b % 2]
                units = []

                def st_unit(st, eng):
                    def u():
                        ot = out_pool.tile([128, C], F32, tag="ot", name=f"ot{b}_{st}")
                        ps = prj_slot()
                        _op_mm(aoT, st, ps[:, 0:C])
                        evac(eng, ot[:, :], ps[:, 0:C])
                        nc.sync.dma_start(
                            out_d[b, st * 128 : (st + 1) * 128, :], ot[:, :]
                        )

                    return u

                oi = iter(OUT_EVAC)
                for st in range(NQT):
                    units.append(st_unit(st, next(oi)))
                return units

            def out_proj_tail(b):
                # final sample: narrow evacs on alternating engines so the
                # drain is not serialized on one engine or on psP
                aoT = aoTs[b % 2]
                for sp in range(NQT // 2):
                    ot = out_pool.tile([128, 2, C], F32)
                    for k, eng in ((0, "a"), (1, "d")):
                        st = 2 * sp + k
                        ps = prj_slot()
                        _op_mm(aoT, st, ps[:, 0:C])
                        evac(eng, ot[:, k, :], ps[:, 0:C])
                    nc.sync.dma_start(
                        out_d[b, 2 * sp * 128 : (2 * sp + 2) * 128, :].rearrange(
                            "(k p) c -> p k c", k=2
                        ),
                        ot[:, :, :],
                    )

            # ---------------- main pipeline: attention(b) runs interleaved
            # with filler units: the tail of sample b's own projection
            # (Q/K pairs 2-3, first used by head 4), proj(b+1)'s early
            # units, and out_proj(b-1).
            setup_set(0)
            xts = {0: load_xt(0, split=True)}
            early0, late0 = proj_units(0, xts[0])
            for u in early0:
                u()
            load_consts()
            setup_set(1)
            carry = late0 if DEFER_LATE else []
            if not DEFER_LATE:
                for u in late0:
                    u()
            for b in range(BL):
                must = list(carry)
                carry = []
                fillers = []
                if b + 1 < BL:
                    xts[b + 1] = load_xt(b + 1)  # prefetch during attention(b)
                    early, late = proj_units(b + 1, xts[b + 1])
                    fillers += early
                    if DEFER_LATE:
                        carry = late
                    else:
                        fillers += late
                if b > 0:
                    # interleave the previous sample's output-projection units
                    # among the projection units (round-robin) so each drain
                    # point mixes both kinds of work
                    ou = out_proj_units(b - 1)
                    mixed = []
                    k = max(1, len(fillers) // max(1, len(ou)))
                    oi2 = iter(ou)
                    for idx, u in enumerate(fillers):
                        mixed.append(u)
                        if idx % k == k - 1:
                            nu = next(oi2, None)
                            if nu is not None:
                                mixed.append(nu)
                    mixed.extend(oi2)
                    fillers = mixed
                attention(b, fillers, tail_div=(b == BL - 1), must=must)
            out_proj_tail(BL - 1)

    nc.finalize()
    _CACHE["nc"] = nc
    return nc


# ---------------------------------------------------------------- host wrapper
def _np_bf16(a):
    import ml_dtypes

    return np.asarray(a, dtype=ml_dtypes.bfloat16)


def _build_m01(mask):
    """[128, BAND_W] banded 0/1 mask in exact-band layout (rows = key within
    chunk c, cols = q in [QLO[c], QHI[c]))."""
    mp = np.asarray(mask)[np.ix_(PERM, PERM)]
    good = np.isfinite(mp) & (mp == 0.0)
    m01 = np.zeros((128, BAND_W), np.float32)
    covered = 0
    for c in range(NQT):
        blk = good[QLO[c] : QHI[c], c * 128 : (c + 1) * 128]  # [q, k]
        m01[:, OFFC[c] : OFFC[c] + WC[c]] = blk.T.astype(np.float32)
        covered += int(blk.sum())
    assert covered == int(good.sum()), "mask not covered by band layout"
    return m01


def _pad_wqkv(w_qkv):
    """[384, 1152] -> [384, 1408]: Q/K head h at cols h*64..h*64+48 (zero pad),
    V kept natural at cols 1024:1408."""
    out = np.zeros((C, WQW), np.float32)
    for sec in range(2):  # Q, K
        for h in range(H):
            out[:, sec * WPADQ + h * 64 : sec * WPADQ + h * 64 + D] = w_qkv[
                :, sec * C + h * D : sec * C + (h + 1) * D
            ]
    out[:, 2 * WPADQ :] = w_qkv[:, 2 * C :]
    return out


def _pad_wproj(w_proj, b_proj):
    """[384, 384] -> [4, 128, 384]: pair p rows 0:48 = head 2p, 64:112 = head 2p+1.
    Row 48 of pair 0 carries b_proj (matched by the constant-1 row in aoT)."""
    out = np.zeros((4, 128, C), np.float32)
    for p in range(4):
        out[p, 0:D] = w_proj[(2 * p) * D : (2 * p + 1) * D]
        out[p, 64 : 64 + D] = w_proj[(2 * p + 1) * D : (2 * p + 2) * D]
    out[0, D] = b_proj
    return out


def kernel(x, w_qkv, w_proj, b_proj, mask):
    x = np.asarray(x, np.float32)
    w_qkv = np.asarray(w_qkv, np.float32)
    w_proj = np.asarray(w_proj, np.float32)
    b_proj = np.asarray(b_proj, np.float32)

    nc = _build()

    xT = _np_bf16(np.ascontiguousarray(x[:, PERM, :].transpose(0, 2, 1)))  # [B, C, S']
    wq_pad = _np_bf16(_pad_wqkv(w_qkv))
    wp_pad = _np_bf16(_pad_wproj(w_proj, b_proj))
    ones_row = _np_bf16(np.ones((1, S), np.float32))
    m01 = _build_m01(mask)
    if PD == BF16:
        m01 = _np_bf16(m01)

    in_maps = [
        {
            "xT": xT[c * BL : (c + 1) * BL],
            "wq_pad": wq_pad,
            "wp_pad": wp_pad,
            "ones_row": ones_row,
            "m01": m01,
        }
        for c in range(N_CORES)
    ]
    res = run_bass_kernel_spmd(nc, in_maps, list(range(N_CORES)))
    out_p = np.concatenate([res.results[c]["out"] for c in range(N_CORES)], axis=0)
    out = np.empty_like(out_p)
    out[:, PERM, :] = out_p
    return out


# revision 62
# speedup vs baseline: 1.4658x; 1.0053x over previous
"""Trainium2 Bass kernel for nn_Attention_13700945674736 (sparse local-window attention).

Strategy (8 NeuronCores, data-parallel over batch, 4 samples/core):
  - Permute the sequence axis s = 64*i + j  ->  s' = 16*j + i (image transpose).
    The 7x11 local window becomes a 1-D band |ds'| <= 83, so each 128-query
    tile only attends to 3 aligned 128-key chunks instead of 1024.
  - Exact per-chunk band is [128c-80, 128c+208): pairs at distance 81..83
    need |dj|=5 AND |di|>=1 simultaneously, which the 2-D mask forbids for
    the first/last 3 keys of a chunk, so halfwidth 80 suffices on the low
    side (and symmetrically 79+128 on the high side; 208 = 128+80).
  - Heads are padded to 64-partition slots (host-padded weights) so every
    engine access pattern starts at a 32-aligned partition.
  - All matmul operands are bf16; PSUM accumulation and softmax reductions
    stay fp32.
  - attnT[k, q] band tiles; exp on ScalarE (3 chunk-groups per head -> few,
    large activations); binary window mask applied multiplicatively on DVE;
    @V uses lhsT=[V|0|ones|0] so softmax denominators land at partitions
    64:112 of the same PSUM tile; the projection bias is folded in via a
    constant-1 row of aoT.
  - po is a single persistent 2-bank PSUM tile; q-tile t of every head uses
    column (t%2)*512+(t//2)*128 so concurrently-open accumulation groups
    alternate banks.  Closed tiles keep their data across same-bank
    start=True (only has_written bits are cleared), so the per-head divides
    run in 2 halves while the next head's @V already accumulates.
  - PSUM budget: pat 2x[128,1024] (4 banks) + proj 2x[128,512] (2 banks)
    + po [128,1024] (2 banks) = 8 banks exactly.
  - Evacuation copies (Q/K/V/out PSUM->SBUF) are split between ScalarE and
    DVE by static schedule tables; output stores go through the SP queue so
    DMA setup does not stall the Activation sequencer.
  - Software pipelining: attention(b) drains "filler" issue-units after each
    head -- sample b's own deferred Q/K pair-2/3 projection, proj(b+1)'s
    units, and out_proj(b-1)'s units (round-robin mixed) -- so the PE always
    has independent matmul work while the per-head softmax-divide chains
    (@V -> reciprocal -> multiply, serialized by the shared po tile) drain
    on DVE.  The last sample's final head divides in q-tile pairs so the
    closing output projection unblocks progressively.
"""

import sys

sys.path.insert(0, "/opt/trn_rl_repo")

import numpy as np

import concourse.bass as bass
from concourse import bacc
import concourse.mybir as mybir
import concourse.tile as tile
from concourse.bass_utils import run_bass_kernel_spmd

# ---------------------------------------------------------------- constants
B, S, C = 32, 1024, 384
H, D = 8, 48
HI, WI = 16, 64
N_CORES = 8
BL = B // N_CORES  # samples per core
SCALE = float(D) ** -0.5
F32 = mybir.dt.float32
BF16 = mybir.dt.bfloat16
PD = BF16  # precision of expT / m01 / vv

# s' = 16*j + i  <->  s = 64*i + j ;  PERM[s'] = s
_sp = np.arange(S)
PERM = (_sp % HI) * WI + (_sp // HI)

NQT = S // 128  # 8 query tiles (and key chunks)
WPADQ = 64 * H  # padded Q (and K) section width: 512
WQW = 2 * WPADQ + C  # 1408

# exact per-chunk bands: key-chunk c attends to queries [QLO[c], QHI[c])
QLO = [max(0, 128 * c - 80) for c in range(NQT)]
QHI = [min(S, 128 * c + 208) for c in range(NQT)]
WC = [QHI[c] - QLO[c] for c in range(NQT)]
OFFC = list(np.cumsum([0] + WC[:-1]))
BAND_W = sum(WC)  # 2144

# chunk groups per PSUM pat tile (each group width <= 1024 f32 = 2 banks)
CH_GROUPS = [(0, 1, 2), (3, 4, 5), (6, 7)]
GRP_BASE = [OFFC[g[0]] for g in CH_GROUPS]
GRP_W = [OFFC[g[-1]] + WC[g[-1]] - OFFC[g[0]] for g in CH_GROUPS]
assert max(GRP_W) <= 1024

# po column slot for q-tile t (identical for every head; open groups t,t+1
# always land in different PSUM banks)
PO_COL = [(t % 2) * 512 + (t // 2) * 128 for t in range(NQT)]

# ------------------------------------------------- engine schedule tables
# 'a' = ScalarE(Act) copy, 'd' = DVE tensor_copy
QK_EVAC = "aadaadaadaadaada"  # 16 per sample (Q then K, pair-major)
V_EVAC = "aadaadaa"  # 8 per sample
OUT_EVAC = "aaaaaaaa"  # 8 per sample
# 'd' = DVE, 'p' = Pool(GPSIMD) for the 3 mask multiplies of each head.
# g1 (chunks 3-5) sits mid-head where its latency hides; g0/g2 gate the
# @V start/tail chains, so they stay on the fast DVE.
# heads 5-7 push their mid/tail mask groups to Pool: it relieves DVE right
# before the end-of-sample divide burst
MASK_ENG = ["ddd"] * 5 + ["dpp"] * 3
# divide granularity: q-tile ranges; later ranges close later, and finer
# tail pieces release the po columns the next head's early @V chunks need.
DIV_RANGES = [(0, 8)]
# defer Q/K pairs 2-3 of proj(b) into attention(b)'s own filler list
DEFER_LATE = True

# ---------------------------------------------------------------- bass program
_CACHE = {}


def _build():
    if "nc" in _CACHE:
        return _CACHE["nc"]

    nc = bacc.Bacc(None, target_bir_lowering=False)
    xT_d = nc.declare_dram_parameter("xT", [BL, C, S], BF16, isOutput=False)
    wq_d = nc.declare_dram_parameter("wq_pad", [C, WQW], BF16, isOutput=False)
    wp_d = nc.declare_dram_parameter("wp_pad", [4, 128, C], BF16, isOutput=False)
    ones_d = nc.declare_dram_parameter("ones_row", [1, S], BF16, isOutput=False)
    m_d = nc.declare_dram_parameter("m01", [128, BAND_W], PD, isOutput=False)
    out_d = nc.declare_dram_parameter("out", [BL, S, C], F32, isOutput=True)

    with tile.TileContext(nc) as tc:
        with (
            tc.tile_pool(name="singles", bufs=1) as singles,
            tc.tile_pool(name="xt_pool", bufs=3) as xt_pool,
            tc.tile_pool(name="out_pool", bufs=4) as out_pool,
            tc.tile_pool(name="ps_proj", bufs=2, space="PSUM") as ps_proj,
            tc.tile_pool(name="ps_pat", bufs=2, space="PSUM") as ps_pat,
            tc.tile_pool(name="ps_po", bufs=1, space="PSUM") as ps_po,
        ):
            # ---- constants.  Q/K/V weight sections load as separate DMAs so
            # the first projection matmuls only wait for their own section.
            w_sb = singles.tile([128, 3, WQW], BF16)
            wq_v = wq_d.rearrange("(c p) w -> p c w", p=128)
            nc.scalar.dma_start(w_sb[:, :, 0:WPADQ], wq_v[:, :, 0:WPADQ])
            nc.scalar.dma_start(
                w_sb[:, :, WPADQ : 2 * WPADQ], wq_v[:, :, WPADQ : 2 * WPADQ]
            )
            nc.scalar.dma_start(w_sb[:, :, 2 * WPADQ :], wq_v[:, :, 2 * WPADQ :])
            wp_sb = singles.tile([128, 4, C], BF16)
            m_sb = singles.tile([128, BAND_W], PD)

            def load_consts():
                # issued after proj(0) so these transfers cannot jump ahead
                # of the startup-critical xt/wQ DMAs on the DMA engines
                nc.sync.dma_start(m_sb, m_d[:, :])
                nc.sync.dma_start(wp_sb[:, :, :], wp_d.rearrange("f p c -> p f c"))

            # persistent attention-output PSUM tile (2 banks)
            po = ps_po.tile([128, S], F32, tag="po")
            _prj_n = [0]

            def prj_slot():
                _prj_n[0] += 1
                ps = ps_proj.tile([128, 512], F32, tag="mm", name=f"prj{_prj_n[0]}")
                return ps

            # ---- per-sample tiles, double-buffered for cross-sample overlap
            qTs, kTs, vvs, aoTs, expTs, dens = [], [], [], [], [], []
            for i in range(2):
                qTs.append(singles.tile([128, 4, S], BF16, name=f"qT{i}"))
                kTs.append(singles.tile([128, 4, S], BF16, name=f"kT{i}"))
                vvs.append(singles.tile([128, NQT, H, 128], PD, name=f"vv{i}"))
                aoTs.append(singles.tile([128, 4, S], BF16, name=f"aoT{i}"))
                expTs.append(singles.tile([128, BAND_W], PD, name=f"expT{i}"))
                dens.append(singles.tile([48, 2, 512], F32, name=f"den{i}"))
            def setup_set(i):
                """One-time constant sections of buffer set i (Pool engine).
                Set 1 is deferred until after proj(0) is issued so sample-0
                mask multiplies are not queued behind 20us of memsets."""
                vv, aoT = vvs[i], aoTs[i]
                nc.gpsimd.memset(vv[:, :, :, D : D + 16], 0.0)
                nc.gpsimd.memset(vv[:, :, :, D + 16 : 112], 1.0)
                nc.gpsimd.memset(vv[:, :, :, 112:128], 0.0)
                # zero dead rows (48:64, 112:128); starts must be 32-aligned so
                # cover 32:64 / 96:128 — live rows are rewritten by the divides.
                nc.gpsimd.memset(aoT[32:64, :, :], 0.0)
                nc.gpsimd.memset(aoT[96:128, :, :], 0.0)
                # constant-1 row: proj picks up b_proj from wp_pad[0][48]
                nc.gpsimd.dma_start(aoT[48:49, 0, :], ones_d[:, :])

            def evac(engine, dst, src):
                if engine == "a":
                    nc.scalar.copy(dst, src)
                else:
                    nc.vector.tensor_copy(dst, src)

            def load_xt(b, split=False):
                xt = xt_pool.tile([128, 3, S], BF16)
                src = xT_d[b].rearrange("(c p) s -> p c s", p=128)
                if split:
                    # startup: two sync-queue halves so the first projection
                    # matmuls only wait for the first half
                    nc.sync.dma_start(xt[:, :, 0:512], src[:, :, 0:512])
                    nc.sync.dma_start(xt[:, :, 512:S], src[:, :, 512:S])
                else:
                    nc.sync.dma_start(xt[:, :, :], src)
                return xt

            def _qk_mm(xt, qk, pair, half, dstps):
                ncol = qk * WPADQ + pair * 128
                for ci in range(3):
                    nc.tensor.matmul(
                        dstps,
                        w_sb[:, ci, ncol : ncol + 128],
                        xt[:, ci, half * 512 : (half + 1) * 512],
                        start=(ci == 0),
                        stop=(ci == 2),
                    )

            def _v_mm(xt, st, dstps):
                for ci in range(3):
                    nc.tensor.matmul(
                        dstps,
                        xt[:, ci, st * 128 : (st + 1) * 128],
                        w_sb[:, ci, 2 * WPADQ : 2 * WPADQ + C],
                        start=(ci == 0),
                        stop=(ci == 2),
                    )

            def proj_units(b, xt):
                """QKV projection for sample b as narrow issue-units over the
                two rotating psP halves."""
                qT, kT, vv = qTs[b % 2], kTs[b % 2], vvs[b % 2]

                def qk_unit(qk, pair, half, eng):
                    def u():
                        dst = qT if qk == 0 else kT
                        ps = prj_slot()
                        _qk_mm(xt, qk, pair, half, ps)
                        evac(
                            eng,
                            dst[:, pair, half * 512 : (half + 1) * 512],
                            ps,
                        )

                    return u

                def v_unit(st, eng):
                    def u():
                        ps = prj_slot()
                        _v_mm(xt, st, ps[:, 0:C])
                        evac(
                            eng,
                            vv[:, st, :, 0:D],
                            ps[:, 0:C].rearrange("p (h d) -> p h d", h=H),
                        )

                    return u

                ei = iter(QK_EVAC)
                qkp = {}
                for qk in range(2):
                    for pair in range(4):
                        for half in range(2):
                            qkp[(qk, pair, half)] = qk_unit(qk, pair, half, next(ei))
                vi = iter(V_EVAC)
                v_units = [v_unit(st, next(vi)) for st in range(NQT)]
                # early: Q/K pairs 0-1 + all V (needed by heads 0-3 and @V);
                # late: Q/K pairs 2-3, deferrable into the next attention's
                # own filler list (pair 2 first used by head 4).
                early, late = [], []
                for (qk, pair, half), u in qkp.items():
                    (early if pair < 2 else late).append(u)
                early.extend(v_units)
                return early, late

            def attention(b, fillers=(), tail_div=False, must=()):
                """Per-head attention; after each head, drain a few filler
                units (next sample's projection / previous sample's output
                projection) so PE has work during the divide chains.
                `must` units carry this sample's own deferred Q/K pair-2/3
                projection: they MUST all be issued by the end of head 2,
                before head 4 issues reads of those qT/kT sections (the tile
                framework resolves dependencies in issue order).
                tail_div: the last head's divide runs in q-tile pairs so the
                final output projection unblocks progressively."""
                fillers = list(must) + list(fillers)
                n_must = len(must)
                fi = 0
                qT, kT, vv, aoT = qTs[b % 2], kTs[b % 2], vvs[b % 2], aoTs[b % 2]
                for h in range(H):
                    div_ranges = (
                        [(0, 2), (2, 4), (4, 6), (6, 8)]
                        if (tail_div and h == H - 1)
                        else DIV_RANGES
                    )
                    pair, sub = divmod(h, 2)
                    p0 = sub * 64
                    expT = expTs[h % 2]
                    den_sb = dens[h % 2]
                    # ---- QK^T band + exp (3 chunk-groups); masks for g0/g1
                    # issue here, g2's mask issues later so DVE's in-order
                    # queue does not park the divides behind it.
                    def mask_mul(gi):
                        gbase, gw = GRP_BASE[gi], GRP_W[gi]
                        tt = (
                            nc.vector.tensor_tensor
                            if MASK_ENG[h][gi] == "d"
                            else nc.gpsimd.tensor_tensor
                        )
                        tt(
                            expT[:, gbase : gbase + gw],
                            expT[:, gbase : gbase + gw],
                            m_sb[:, gbase : gbase + gw],
                            mybir.AluOpType.mult,
                        )

                    for gi, grp in enumerate(CH_GROUPS):
                        gbase, gw = GRP_BASE[gi], GRP_W[gi]
                        pat = ps_pat.tile([128, 1024], F32, tag="attn")
                        for c in grp:
                            lo = OFFC[c] - gbase
                            hi = lo + WC[c]
                            a = lo
                            while a < hi:
                                b2 = min(hi, (a // 512 + 1) * 512)
                                nc.tensor.matmul(
                                    pat[:, a:b2],
                                    kT[p0 : p0 + D, pair, c * 128 : (c + 1) * 128],
                                    qT[
                                        p0 : p0 + D,
                                        pair,
                                        QLO[c] + (a - lo) : QLO[c] + (b2 - lo),
                                    ],
                                    start=True,
                                    stop=True,
                                )
                                a = b2
                        nc.scalar.activation(
                            expT[:, gbase : gbase + gw],
                            pat[:, 0:gw],
                            mybir.ActivationFunctionType.Exp,
                            scale=SCALE,
                        )
                        if gi < 2:
                            mask_mul(gi)

                    # ---- @V with ones rows at 64:112 -> denominators,
                    # interleaved with the per-range normalizes so divides
                    # start the moment their q-tiles close.
                    po_q = po[:, :].rearrange(
                        "p (tlo thi u) -> p thi tlo u", tlo=2, thi=4, u=128
                    )
                    den_flat = den_sb[:, :, :].rearrange("p a b -> p (a b)")

                    def av_chunk(c):
                        lhsT = vv[:, c, h, :]
                        for t in range(max(c - 1, 0), min(c + 2, NQT)):
                            pc = PO_COL[t]
                            qs = max(128 * t, QLO[c])
                            qe = min(128 * t + 128, QHI[c])
                            nc.tensor.matmul(
                                po[:, pc + (qs - 128 * t) : pc + (qe - 128 * t)],
                                lhsT,
                                expT[:, OFFC[c] + (qs - QLO[c]) : OFFC[c] + (qe - QLO[c])],
                                start=(c == max(t - 1, 0)),
                                stop=(c == min(t + 1, NQT - 1)),
                            )

                    def divide(ta, tb):
                        nt = (tb - ta) // 2
                        den_v = den_flat[:, ta * 128 : tb * 128].rearrange(
                            "p (thi tlo u) -> p thi tlo u", thi=nt, tlo=2, u=128
                        )
                        ao_v = aoT[
                            p0 : p0 + D, pair, ta * 128 : tb * 128
                        ].rearrange("p (thi tlo u) -> p thi tlo u", thi=nt, tlo=2, u=128)
                        nc.vector.reciprocal(
                            den_v, po_q[64 : 64 + D, ta // 2 : tb // 2]
                        )
                        nc.vector.tensor_tensor(
                            ao_v,
                            po_q[0:D, ta // 2 : tb // 2],
                            den_v,
                            mybir.AluOpType.mult,
                        )

                    g2_start = CH_GROUPS[2][0]
                    for c in range(NQT):
                        if c == g2_start:
                            mask_mul(2)  # just before @V enters group 2
                        av_chunk(c)
                        for ta, tb in div_ranges:
                            # tile tb-1 stops at chunk min(tb, NQT-1)
                            if min(tb, NQT - 1) == c:
                                divide(ta, tb)
                    # drain filler units evenly across heads.  The deferred
                    # own-projection units sit at the head of the list: the
                    # drain targets guarantee they are issued by the end of
                    # head 2, before head 4 issues reads of qT/kT pairs 2-3
                    # (the tile framework resolves deps in issue order).
                    if len(fillers) >= 20:
                        target = (h + 1) * len(fillers) // H
                    else:
                        # thin filler supply (last sample): save units for the
                        # late heads where the divide chains stall hardest
                        target = ((h + 1) * (h + 2) * len(fillers)) // (H * (H + 1))
                    if h == 2 and n_must > target:
                        target = n_must
                    while fi < target:
                        fillers[fi]()
                        fi += 1

            def _op_mm(aoT, st, dstps):
                for p in range(4):
                    nc.tensor.matmul(
                        dstps,
                        aoT[:, p, st * 128 : (st + 1) * 128],
                        wp_sb[:, p, :],
                        start=(p == 0),
                        stop=(p == 3),
                    )

            def out_proj_units(b):
                aoT = aoTs[b % 2]
                units = []

                def st_unit(st, eng):
                    def u():
                        ot = out_pool.tile([128, C], F32, tag="ot", name=f"ot{b}_{st}")
                        ps = prj_slot()
                        _op_mm(aoT, st, ps[:, 0:C])
                        evac(eng, ot[:, :], ps[:, 0:C])
                        nc.sync.dma_start(
                            out_d[b, st * 128 : (st + 1) * 128, :], ot[:, :]
                        )

                    return u

                oi = iter(OUT_EVAC)
                for st in range(NQT):
                    units.append(st_unit(st, next(oi)))
                return units

            def out_proj_tail(b):
                # final sample: narrow evacs on alternating engines so the
                # drain is not serialized on one engine or on psP
                aoT = aoTs[b % 2]
                for sp in range(NQT // 2):
                    ot = out_pool.tile([128, 2, C], F32)
                    for k, eng in ((0, "a"), (1, "d")):
                        st = 2 * sp + k
                        ps = prj_slot()
                        _op_mm(aoT, st, ps[:, 0:C])
                        evac(eng, ot[:, k, :], ps[:, 0:C])
                    nc.sync.dma_start(
                        out_d[b, 2 * sp * 128 : (2 * sp + 2) * 128, :].rearrange(
                            "(k p) c -> p k c", k=2
                        ),
                        ot[:, :, :],
                    )

            # ---------------- main pipeline: attention(b) runs interleaved
            # with filler units: the tail of sample b's own projection
            # (Q/K pairs 2-3, first used by head 4), proj(b+1)'s early
            # units, and out_proj(b-1).
            setup_set(0)
            xts = {0: load_xt(0, split=True)}
            early0, late0 = proj_units(0, xts[0])
            for u in early0:
                u()
            load_consts()
            setup_set(1)
            carry = late0 if DEFER_LATE else []
            if not DEFER_LATE:
                for u in late0:
                    u()
            for b in range(BL):
                must = list(carry)
                carry = []
                fillers = []
                if b + 1 < BL:
                    xts[b + 1] = load_xt(b + 1)  # prefetch during attention(b)
                    early, late = proj_units(b + 1, xts[b + 1])
                    fillers += early
                    if DEFER_LATE:
                        carry = late
                    else:
                        fillers += late
                if b > 0:
                    # interleave the previous sample's output-projection units
                    # among the projection units (round-robin) so each drain
                    # point mixes both kinds of work
                    ou = out_proj_units(b - 1)
                    mixed = []
                    k = max(1, len(fillers) // max(1, len(ou)))
                    oi2 = iter(ou)
                    for idx, u in enumerate(fillers):
                        mixed.append(u)
                        if idx % k == k - 1:
                            nu = next(oi2, None)
                            if nu is not None:
                                mixed.append(nu)
                    mixed.extend(oi2)
                    fillers = mixed
                attention(b, fillers, tail_div=(b == BL - 1), must=must)
            out_proj_tail(BL - 1)

    nc.finalize()
    _CACHE["nc"] = nc
    return nc


# ---------------------------------------------------------------- host wrapper
def _np_bf16(a):
    import ml_dtypes

    return np.asarray(a, dtype=ml_dtypes.bfloat16)


def _build_m01(mask):
    """[128, BAND_W] banded 0/1 mask in exact-band layout (rows = key within
    chunk c, cols = q in [QLO[c], QHI[c]))."""
    mp = np.asarray(mask)[np.ix_(PERM, PERM)]
    good = np.isfinite(mp) & (mp == 0.0)
    m01 = np.zeros((128, BAND_W), np.float32)
    covered = 0
    for c in range(NQT):
        blk = good[QLO[c] : QHI[c], c * 128 : (c + 1) * 128]  # [q, k]
        m01[:, OFFC[c] : OFFC[c] + WC[c]] = blk.T.astype(np.float32)
        covered += int(blk.sum())
    assert covered == int(good.sum()), "mask not covered by band layout"
    return m01


def _pad_wqkv(w_qkv):
    """[384, 1152] -> [384, 1408]: Q/K head h at cols h*64..h*64+48 (zero pad),
    V kept natural at cols 1024:1408."""
    out = np.zeros((C, WQW), np.float32)
    for sec in range(2):  # Q, K
        for h in range(H):
            out[:, sec * WPADQ + h * 64 : sec * WPADQ + h * 64 + D] = w_qkv[
                :, sec * C + h * D : sec * C + (h + 1) * D
            ]
    out[:, 2 * WPADQ :] = w_qkv[:, 2 * C :]
    return out


def _pad_wproj(w_proj, b_proj):
    """[384, 384] -> [4, 128, 384]: pair p rows 0:48 = head 2p, 64:112 = head 2p+1.
    Row 48 of pair 0 carries b_proj (matched by the constant-1 row in aoT)."""
    out = np.zeros((4, 128, C), np.float32)
    for p in range(4):
        out[p, 0:D] = w_proj[(2 * p) * D : (2 * p + 1) * D]
        out[p, 64 : 64 + D] = w_proj[(2 * p + 1) * D : (2 * p + 2) * D]
    out[0, D] = b_proj
    return out


def kernel(x, w_qkv, w_proj, b_proj, mask):
    x = np.asarray(x, np.float32)
    w_qkv = np.asarray(w_qkv, np.float32)
    w_proj = np.asarray(w_proj, np.float32)
    b_proj = np.asarray(b_proj, np.float32)

    nc = _build()

    xT = _np_bf16(np.ascontiguousarray(x[:, PERM, :].transpose(0, 2, 1)))  # [B, C, S']
    wq_pad = _np_bf16(_pad_wqkv(w_qkv))
    wp_pad = _np_bf16(_pad_wproj(w_proj, b_proj))
    ones_row = _np_bf16(np.ones((1, S), np.float32))
    m01 = _build_m01(mask)
    if PD == BF16:
        m01 = _np_bf16(m01)

    in_maps = [
        {
            "xT": xT[c * BL : (c + 1) * BL],
            "wq_pad": wq_pad,
            "wp_pad": wp_pad,
            "ones_row": ones_row,
            "m01": m01,
        }
        for c in range(N_CORES)
    ]
    res = run_bass_kernel_spmd(nc, in_maps, list(range(N_CORES)))
    out_p = np.concatenate([res.results[c]["out"] for c in range(N_CORES)], axis=0)
    out = np.empty_like(out_p)
    out[:, PERM, :] = out_p
    return out


# revision 63
# speedup vs baseline: 1.4712x; 1.0036x over previous
"""Trainium2 Bass kernel for nn_Attention_13700945674736 (sparse local-window attention).

Strategy (8 NeuronCores, data-parallel over batch, 4 samples/core):
  - Permute the sequence axis s = 64*i + j  ->  s' = 16*j + i (image transpose).
    The 7x11 local window becomes a 1-D band |ds'| <= 83, so each 128-query
    tile only attends to 3 aligned 128-key chunks instead of 1024.
  - Exact per-chunk band is [128c-80, 128c+208): pairs at distance 81..83
    need |dj|=5 AND |di|>=1 simultaneously, which the 2-D mask forbids for
    the first/last 3 keys of a chunk, so halfwidth 80 suffices on the low
    side (and symmetrically 79+128 on the high side; 208 = 128+80).
  - Heads are padded to 64-partition slots (host-padded weights) so every
    engine access pattern starts at a 32-aligned partition.
  - All matmul operands are bf16; PSUM accumulation and softmax reductions
    stay fp32.
  - attnT[k, q] band tiles; exp on ScalarE (3 chunk-groups per head -> few,
    large activations); binary window mask applied multiplicatively on DVE;
    @V uses lhsT=[V|0|ones|0] so softmax denominators land at partitions
    64:112 of the same PSUM tile; the projection bias is folded in via a
    constant-1 row of aoT.
  - po is a single persistent 2-bank PSUM tile; q-tile t of every head uses
    column (t%2)*512+(t//2)*128 so concurrently-open accumulation groups
    alternate banks.  Closed tiles keep their data across same-bank
    start=True (only has_written bits are cleared), so the per-head divides
    run in 2 halves while the next head's @V already accumulates.
  - PSUM budget: pat 2x[128,1024] (4 banks) + proj 2x[128,512] (2 banks)
    + po [128,1024] (2 banks) = 8 banks exactly.
  - Evacuation copies (Q/K/V/out PSUM->SBUF) are split between ScalarE and
    DVE by static schedule tables; output stores go through the SP queue so
    DMA setup does not stall the Activation sequencer.
  - Software pipelining: attention(b) drains "filler" issue-units after each
    head -- sample b's own deferred Q/K pair-2/3 projection, proj(b+1)'s
    units, and out_proj(b-1)'s units (round-robin mixed) -- so the PE always
    has independent matmul work while the per-head softmax-divide chains
    (@V -> reciprocal -> multiply, serialized by the shared po tile) drain
    on DVE.  The last sample's final head divides in q-tile pairs so the
    closing output projection unblocks progressively.
"""

import sys

sys.path.insert(0, "/opt/trn_rl_repo")

import numpy as np

import concourse.bass as bass
from concourse import bacc
import concourse.mybir as mybir
import concourse.tile as tile
from concourse.bass_utils import run_bass_kernel_spmd

# ---------------------------------------------------------------- constants
B, S, C = 32, 1024, 384
H, D = 8, 48
HI, WI = 16, 64
N_CORES = 8
BL = B // N_CORES  # samples per core
SCALE = float(D) ** -0.5
F32 = mybir.dt.float32
BF16 = mybir.dt.bfloat16
PD = BF16  # precision of expT / m01 / vv

# s' = 16*j + i  <->  s = 64*i + j ;  PERM[s'] = s
_sp = np.arange(S)
PERM = (_sp % HI) * WI + (_sp // HI)

NQT = S // 128  # 8 query tiles (and key chunks)
WPADQ = 64 * H  # padded Q (and K) section width: 512
WQW = 2 * WPADQ + C  # 1408

# exact per-chunk bands: key-chunk c attends to queries [QLO[c], QHI[c])
QLO = [max(0, 128 * c - 80) for c in range(NQT)]
QHI = [min(S, 128 * c + 208) for c in range(NQT)]
WC = [QHI[c] - QLO[c] for c in range(NQT)]
OFFC = list(np.cumsum([0] + WC[:-1]))
BAND_W = sum(WC)  # 2144

# chunk groups per PSUM pat tile (each group width <= 1024 f32 = 2 banks)
CH_GROUPS = [(0, 1, 2), (3, 4, 5), (6, 7)]
GRP_BASE = [OFFC[g[0]] for g in CH_GROUPS]
GRP_W = [OFFC[g[-1]] + WC[g[-1]] - OFFC[g[0]] for g in CH_GROUPS]
assert max(GRP_W) <= 1024

# po column slot for q-tile t (identical for every head; open groups t,t+1
# always land in different PSUM banks)
PO_COL = [(t % 2) * 512 + (t // 2) * 128 for t in range(NQT)]

# ------------------------------------------------- engine schedule tables
# 'a' = ScalarE(Act) copy, 'd' = DVE tensor_copy
QK_EVAC = "aadaadaadaadaada"  # 16 per sample (Q then K, pair-major)
V_EVAC = "aadaadaa"  # 8 per sample
OUT_EVAC = "aaaaaaaa"  # 8 per sample
# 'd' = DVE, 'p' = Pool(GPSIMD) for the 3 mask multiplies of each head.
# g1 (chunks 3-5) sits mid-head where its latency hides; g0/g2 gate the
# @V start/tail chains, so they stay on the fast DVE.
# heads 5-7 push their mid/tail mask groups to Pool: it relieves DVE right
# before the end-of-sample divide burst
MASK_ENG = ["ddd"] * 5 + ["dpp"] * 3
# divide granularity: q-tile ranges; later ranges close later, and finer
# tail pieces release the po columns the next head's early @V chunks need.
DIV_RANGES = [(0, 8)]
# defer Q/K pairs 2-3 of proj(b) into attention(b)'s own filler list
DEFER_LATE = True

# ---------------------------------------------------------------- bass program
_CACHE = {}


def _build():
    if "nc" in _CACHE:
        return _CACHE["nc"]

    nc = bacc.Bacc(None, target_bir_lowering=False)
    xT_d = nc.declare_dram_parameter("xT", [BL, C, S], BF16, isOutput=False)
    wq_d = nc.declare_dram_parameter("wq_pad", [C, WQW], BF16, isOutput=False)
    wp_d = nc.declare_dram_parameter("wp_pad", [4, 128, C], BF16, isOutput=False)
    ones_d = nc.declare_dram_parameter("ones_row", [1, S], BF16, isOutput=False)
    m_d = nc.declare_dram_parameter("m01", [128, BAND_W], PD, isOutput=False)
    out_d = nc.declare_dram_parameter("out", [BL, S, C], F32, isOutput=True)

    with tile.TileContext(nc) as tc:
        with (
            tc.tile_pool(name="singles", bufs=1) as singles,
            tc.tile_pool(name="xt_pool", bufs=3) as xt_pool,
            tc.tile_pool(name="out_pool", bufs=4) as out_pool,
            tc.tile_pool(name="ps_proj", bufs=2, space="PSUM") as ps_proj,
            tc.tile_pool(name="ps_pat", bufs=2, space="PSUM") as ps_pat,
            tc.tile_pool(name="ps_po", bufs=1, space="PSUM") as ps_po,
        ):
            # ---- constants.  Q/K/V weight sections load as separate DMAs so
            # the first projection matmuls only wait for their own section.
            w_sb = singles.tile([128, 3, WQW], BF16)
            wq_v = wq_d.rearrange("(c p) w -> p c w", p=128)
            # Q pair-0 weights first (tiny transfer): the first projection
            # matmul waits only on this slice plus the first xt half
            nc.scalar.dma_start(w_sb[:, :, 0:128], wq_v[:, :, 0:128])
            nc.scalar.dma_start(w_sb[:, :, 128:WPADQ], wq_v[:, :, 128:WPADQ])
            nc.scalar.dma_start(
                w_sb[:, :, WPADQ : 2 * WPADQ], wq_v[:, :, WPADQ : 2 * WPADQ]
            )
            nc.scalar.dma_start(w_sb[:, :, 2 * WPADQ :], wq_v[:, :, 2 * WPADQ :])
            wp_sb = singles.tile([128, 4, C], BF16)
            m_sb = singles.tile([128, BAND_W], PD)

            def load_consts():
                # issued after proj(0) so these transfers cannot jump ahead
                # of the startup-critical xt/wQ DMAs on the DMA engines
                nc.sync.dma_start(m_sb, m_d[:, :])
                nc.sync.dma_start(wp_sb[:, :, :], wp_d.rearrange("f p c -> p f c"))

            # persistent attention-output PSUM tile (2 banks)
            po = ps_po.tile([128, S], F32, tag="po")
            _prj_n = [0]

            def prj_slot():
                _prj_n[0] += 1
                ps = ps_proj.tile([128, 512], F32, tag="mm", name=f"prj{_prj_n[0]}")
                return ps

            # ---- per-sample tiles, double-buffered for cross-sample overlap
            qTs, kTs, vvs, aoTs, expTs, dens = [], [], [], [], [], []
            for i in range(2):
                qTs.append(singles.tile([128, 4, S], BF16, name=f"qT{i}"))
                kTs.append(singles.tile([128, 4, S], BF16, name=f"kT{i}"))
                vvs.append(singles.tile([128, NQT, H, 128], PD, name=f"vv{i}"))
                aoTs.append(singles.tile([128, 4, S], BF16, name=f"aoT{i}"))
                expTs.append(singles.tile([128, BAND_W], PD, name=f"expT{i}"))
                dens.append(singles.tile([48, 2, 512], F32, name=f"den{i}"))
            def setup_set(i):
                """One-time constant sections of buffer set i (Pool engine).
                Set 1 is deferred until after proj(0) is issued so sample-0
                mask multiplies are not queued behind 20us of memsets."""
                vv, aoT = vvs[i], aoTs[i]
                nc.gpsimd.memset(vv[:, :, :, D : D + 16], 0.0)
                nc.gpsimd.memset(vv[:, :, :, D + 16 : 112], 1.0)
                nc.gpsimd.memset(vv[:, :, :, 112:128], 0.0)
                # zero dead rows (48:64, 112:128); starts must be 32-aligned so
                # cover 32:64 / 96:128 — live rows are rewritten by the divides.
                nc.gpsimd.memset(aoT[32:64, :, :], 0.0)
                nc.gpsimd.memset(aoT[96:128, :, :], 0.0)
                # constant-1 row: proj picks up b_proj from wp_pad[0][48]
                nc.gpsimd.dma_start(aoT[48:49, 0, :], ones_d[:, :])

            def evac(engine, dst, src):
                if engine == "a":
                    nc.scalar.copy(dst, src)
                else:
                    nc.vector.tensor_copy(dst, src)

            def load_xt(b, split=False):
                xt = xt_pool.tile([128, 3, S], BF16)
                src = xT_d[b].rearrange("(c p) s -> p c s", p=128)
                if split:
                    # startup: two sync-queue halves so the first projection
                    # matmuls only wait for the first half
                    nc.sync.dma_start(xt[:, :, 0:512], src[:, :, 0:512])
                    nc.sync.dma_start(xt[:, :, 512:S], src[:, :, 512:S])
                else:
                    nc.sync.dma_start(xt[:, :, :], src)
                return xt

            def _qk_mm(xt, qk, pair, half, dstps):
                ncol = qk * WPADQ + pair * 128
                for ci in range(3):
                    nc.tensor.matmul(
                        dstps,
                        w_sb[:, ci, ncol : ncol + 128],
                        xt[:, ci, half * 512 : (half + 1) * 512],
                        start=(ci == 0),
                        stop=(ci == 2),
                    )

            def _v_mm(xt, st, dstps):
                for ci in range(3):
                    nc.tensor.matmul(
                        dstps,
                        xt[:, ci, st * 128 : (st + 1) * 128],
                        w_sb[:, ci, 2 * WPADQ : 2 * WPADQ + C],
                        start=(ci == 0),
                        stop=(ci == 2),
                    )

            def proj_units(b, xt):
                """QKV projection for sample b as narrow issue-units over the
                two rotating psP halves."""
                qT, kT, vv = qTs[b % 2], kTs[b % 2], vvs[b % 2]

                def qk_unit(qk, pair, half, eng):
                    def u():
                        dst = qT if qk == 0 else kT
                        ps = prj_slot()
                        _qk_mm(xt, qk, pair, half, ps)
                        evac(
                            eng,
                            dst[:, pair, half * 512 : (half + 1) * 512],
                            ps,
                        )

                    return u

                def v_unit(st, eng):
                    def u():
                        ps = prj_slot()
                        _v_mm(xt, st, ps[:, 0:C])
                        evac(
                            eng,
                            vv[:, st, :, 0:D],
                            ps[:, 0:C].rearrange("p (h d) -> p h d", h=H),
                        )

                    return u

                ei = iter(QK_EVAC)
                qkp = {}
                for qk in range(2):
                    for pair in range(4):
                        for half in range(2):
                            qkp[(qk, pair, half)] = qk_unit(qk, pair, half, next(ei))
                vi = iter(V_EVAC)
                v_units = [v_unit(st, next(vi)) for st in range(NQT)]
                # early: Q/K pairs 0-1 + all V (needed by heads 0-3 and @V);
                # late: Q/K pairs 2-3, deferrable into the next attention's
                # own filler list (pair 2 first used by head 4).
                early, late = [], []
                for (qk, pair, half), u in qkp.items():
                    (early if pair < 2 else late).append(u)
                early.extend(v_units)
                return early, late

            def attention(b, fillers=(), tail_div=False, must=()):
                """Per-head attention; after each head, drain a few filler
                units (next sample's projection / previous sample's output
                projection) so PE has work during the divide chains.
                `must` units carry this sample's own deferred Q/K pair-2/3
                projection: they MUST all be issued by the end of head 2,
                before head 4 issues reads of those qT/kT sections (the tile
                framework resolves dependencies in issue order).
                tail_div: the last head's divide runs in q-tile pairs so the
                final output projection unblocks progressively."""
                fillers = list(must) + list(fillers)
                n_must = len(must)
                fi = 0
                qT, kT, vv, aoT = qTs[b % 2], kTs[b % 2], vvs[b % 2], aoTs[b % 2]
                for h in range(H):
                    div_ranges = (
                        [(0, 2), (2, 4), (4, 6), (6, 8)]
                        if (tail_div and h == H - 1)
                        else DIV_RANGES
                    )
                    pair, sub = divmod(h, 2)
                    p0 = sub * 64
                    expT = expTs[h % 2]
                    den_sb = dens[h % 2]
                    # ---- QK^T band + exp (3 chunk-groups); masks for g0/g1
                    # issue here, g2's mask issues later so DVE's in-order
                    # queue does not park the divides behind it.
                    def mask_mul(gi):
                        gbase, gw = GRP_BASE[gi], GRP_W[gi]
                        tt = (
                            nc.vector.tensor_tensor
                            if MASK_ENG[h][gi] == "d"
                            else nc.gpsimd.tensor_tensor
                        )
                        tt(
                            expT[:, gbase : gbase + gw],
                            expT[:, gbase : gbase + gw],
                            m_sb[:, gbase : gbase + gw],
                            mybir.AluOpType.mult,
                        )

                    for gi, grp in enumerate(CH_GROUPS):
                        gbase, gw = GRP_BASE[gi], GRP_W[gi]
                        pat = ps_pat.tile([128, 1024], F32, tag="attn")
                        for c in grp:
                            lo = OFFC[c] - gbase
                            hi = lo + WC[c]
                            a = lo
                            while a < hi:
                                b2 = min(hi, (a // 512 + 1) * 512)
                                nc.tensor.matmul(
                                    pat[:, a:b2],
                                    kT[p0 : p0 + D, pair, c * 128 : (c + 1) * 128],
                                    qT[
                                        p0 : p0 + D,
                                        pair,
                                        QLO[c] + (a - lo) : QLO[c] + (b2 - lo),
                                    ],
                                    start=True,
                                    stop=True,
                                )
                                a = b2
                        nc.scalar.activation(
                            expT[:, gbase : gbase + gw],
                            pat[:, 0:gw],
                            mybir.ActivationFunctionType.Exp,
                            scale=SCALE,
                        )
                        if gi < 2:
                            mask_mul(gi)

                    # ---- @V with ones rows at 64:112 -> denominators,
                    # interleaved with the per-range normalizes so divides
                    # start the moment their q-tiles close.
                    po_q = po[:, :].rearrange(
                        "p (tlo thi u) -> p thi tlo u", tlo=2, thi=4, u=128
                    )
                    den_flat = den_sb[:, :, :].rearrange("p a b -> p (a b)")

                    def av_chunk(c):
                        lhsT = vv[:, c, h, :]
                        for t in range(max(c - 1, 0), min(c + 2, NQT)):
                            pc = PO_COL[t]
                            qs = max(128 * t, QLO[c])
                            qe = min(128 * t + 128, QHI[c])
                            nc.tensor.matmul(
                                po[:, pc + (qs - 128 * t) : pc + (qe - 128 * t)],
                                lhsT,
                                expT[:, OFFC[c] + (qs - QLO[c]) : OFFC[c] + (qe - QLO[c])],
                                start=(c == max(t - 1, 0)),
                                stop=(c == min(t + 1, NQT - 1)),
                            )

                    def divide(ta, tb):
                        nt = (tb - ta) // 2
                        den_v = den_flat[:, ta * 128 : tb * 128].rearrange(
                            "p (thi tlo u) -> p thi tlo u", thi=nt, tlo=2, u=128
                        )
                        ao_v = aoT[
                            p0 : p0 + D, pair, ta * 128 : tb * 128
                        ].rearrange("p (thi tlo u) -> p thi tlo u", thi=nt, tlo=2, u=128)
                        nc.vector.reciprocal(
                            den_v, po_q[64 : 64 + D, ta // 2 : tb // 2]
                        )
                        nc.vector.tensor_tensor(
                            ao_v,
                            po_q[0:D, ta // 2 : tb // 2],
                            den_v,
                            mybir.AluOpType.mult,
                        )

                    g2_start = CH_GROUPS[2][0]
                    for c in range(NQT):
                        if c == g2_start:
                            mask_mul(2)  # just before @V enters group 2
                        av_chunk(c)
                        for ta, tb in div_ranges:
                            # tile tb-1 stops at chunk min(tb, NQT-1)
                            if min(tb, NQT - 1) == c:
                                divide(ta, tb)
                    # drain filler units evenly across heads.  The deferred
                    # own-projection units sit at the head of the list: the
                    # drain targets guarantee they are issued by the end of
                    # head 2, before head 4 issues reads of qT/kT pairs 2-3
                    # (the tile framework resolves deps in issue order).
                    if len(fillers) >= 20:
                        target = (h + 1) * len(fillers) // H
                    else:
                        # thin filler supply (last sample): save units for the
                        # late heads where the divide chains stall hardest
                        target = ((h + 1) * (h + 2) * len(fillers)) // (H * (H + 1))
                    if h == 2 and n_must > target:
                        target = n_must
                    while fi < target:
                        fillers[fi]()
                        fi += 1

            def _op_mm(aoT, st, dstps):
                for p in range(4):
                    nc.tensor.matmul(
                        dstps,
                        aoT[:, p, st * 128 : (st + 1) * 128],
                        wp_sb[:, p, :],
                        start=(p == 0),
                        stop=(p == 3),
                    )

            def out_proj_units(b):
                aoT = aoTs[b % 2]
                units = []

                def st_unit(st, eng):
                    def u():
                        ot = out_pool.tile([128, C], F32, tag="ot", name=f"ot{b}_{st}")
                        ps = prj_slot()
                        _op_mm(aoT, st, ps[:, 0:C])
                        evac(eng, ot[:, :], ps[:, 0:C])
                        nc.sync.dma_start(
                            out_d[b, st * 128 : (st + 1) * 128, :], ot[:, :]
                        )

                    return u

                oi = iter(OUT_EVAC)
                for st in range(NQT):
                    units.append(st_unit(st, next(oi)))
                return units

            def out_proj_tail(b):
                # final sample: narrow evacs on alternating engines so the
                # drain is not serialized on one engine or on psP
                aoT = aoTs[b % 2]
                for sp in range(NQT // 2):
                    ot = out_pool.tile([128, 2, C], F32)
                    for k, eng in ((0, "a"), (1, "d")):
                        st = 2 * sp + k
                        ps = prj_slot()
                        _op_mm(aoT, st, ps[:, 0:C])
                        evac(eng, ot[:, k, :], ps[:, 0:C])
                    nc.sync.dma_start(
                        out_d[b, 2 * sp * 128 : (2 * sp + 2) * 128, :].rearrange(
                            "(k p) c -> p k c", k=2
                        ),
                        ot[:, :, :],
                    )

            # ---------------- main pipeline: attention(b) runs interleaved
            # with filler units: the tail of sample b's own projection
            # (Q/K pairs 2-3, first used by head 4), proj(b+1)'s early
            # units, and out_proj(b-1).
            setup_set(0)
            xts = {0: load_xt(0, split=True)}
            early0, late0 = proj_units(0, xts[0])
            for u in early0:
                u()
            load_consts()
            setup_set(1)
            carry = late0 if DEFER_LATE else []
            if not DEFER_LATE:
                for u in late0:
                    u()
            for b in range(BL):
                must = list(carry)
                carry = []
                fillers = []
                if b + 1 < BL:
                    xts[b + 1] = load_xt(b + 1)  # prefetch during attention(b)
                    early, late = proj_units(b + 1, xts[b + 1])
                    fillers += early
                    if DEFER_LATE:
                        carry = late
                    else:
                        fillers += late
                if b > 0:
                    # interleave the previous sample's output-projection units
                    # among the projection units (round-robin) so each drain
                    # point mixes both kinds of work
                    ou = out_proj_units(b - 1)
                    mixed = []
                    k = max(1, len(fillers) // max(1, len(ou)))
                    oi2 = iter(ou)
                    for idx, u in enumerate(fillers):
                        mixed.append(u)
                        if idx % k == k - 1:
                            nu = next(oi2, None)
                            if nu is not None:
                                mixed.append(nu)
                    mixed.extend(oi2)
                    fillers = mixed
                attention(b, fillers, tail_div=(b == BL - 1), must=must)
            out_proj_tail(BL - 1)

    nc.finalize()
    _CACHE["nc"] = nc
    return nc


# ---------------------------------------------------------------- host wrapper
def _np_bf16(a):
    import ml_dtypes

    return np.asarray(a, dtype=ml_dtypes.bfloat16)


def _build_m01(mask):
    """[128, BAND_W] banded 0/1 mask in exact-band layout (rows = key within
    chunk c, cols = q in [QLO[c], QHI[c]))."""
    mp = np.asarray(mask)[np.ix_(PERM, PERM)]
    good = np.isfinite(mp) & (mp == 0.0)
    m01 = np.zeros((128, BAND_W), np.float32)
    covered = 0
    for c in range(NQT):
        blk = good[QLO[c] : QHI[c], c * 128 : (c + 1) * 128]  # [q, k]
        m01[:, OFFC[c] : OFFC[c] + WC[c]] = blk.T.astype(np.float32)
        covered += int(blk.sum())
    assert covered == int(good.sum()), "mask not covered by band layout"
    return m01


def _pad_wqkv(w_qkv):
    """[384, 1152] -> [384, 1408]: Q/K head h at cols h*64..h*64+48 (zero pad),
    V kept natural at cols 1024:1408."""
    out = np.zeros((C, WQW), np.float32)
    for sec in range(2):  # Q, K
        for h in range(H):
            out[:, sec * WPADQ + h * 64 : sec * WPADQ + h * 64 + D] = w_qkv[
                :, sec * C + h * D : sec * C + (h + 1) * D
            ]
    out[:, 2 * WPADQ :] = w_qkv[:, 2 * C :]
    return out


def _pad_wproj(w_proj, b_proj):
    """[384, 384] -> [4, 128, 384]: pair p rows 0:48 = head 2p, 64:112 = head 2p+1.
    Row 48 of pair 0 carries b_proj (matched by the constant-1 row in aoT)."""
    out = np.zeros((4, 128, C), np.float32)
    for p in range(4):
        out[p, 0:D] = w_proj[(2 * p) * D : (2 * p + 1) * D]
        out[p, 64 : 64 + D] = w_proj[(2 * p + 1) * D : (2 * p + 2) * D]
    out[0, D] = b_proj
    return out


def kernel(x, w_qkv, w_proj, b_proj, mask):
    x = np.asarray(x, np.float32)
    w_qkv = np.asarray(w_qkv, np.float32)
    w_proj = np.asarray(w_proj, np.float32)
    b_proj = np.asarray(b_proj, np.float32)

    nc = _build()

    xT = _np_bf16(np.ascontiguousarray(x[:, PERM, :].transpose(0, 2, 1)))  # [B, C, S']
    wq_pad = _np_bf16(_pad_wqkv(w_qkv))
    wp_pad = _np_bf16(_pad_wproj(w_proj, b_proj))
    ones_row = _np_bf16(np.ones((1, S), np.float32))
    m01 = _build_m01(mask)
    if PD == BF16:
        m01 = _np_bf16(m01)

    in_maps = [
        {
            "xT": xT[c * BL : (c + 1) * BL],
            "wq_pad": wq_pad,
            "wp_pad": wp_pad,
            "ones_row": ones_row,
            "m01": m01,
        }
        for c in range(N_CORES)
    ]
    res = run_bass_kernel_spmd(nc, in_maps, list(range(N_CORES)))
    out_p = np.concatenate([res.results[c]["out"] for c in range(N_CORES)], axis=0)
    out = np.empty_like(out_p)
    out[:, PERM, :] = out_p
    return out


# revision 71
# speedup vs baseline: 1.4738x; 1.0018x over previous
"""Trainium2 Bass kernel for nn_Attention_13700945674736 (sparse local-window attention).

Strategy (8 NeuronCores, data-parallel over batch, 4 samples/core):
  - Permute the sequence axis s = 64*i + j  ->  s' = 16*j + i (image transpose).
    The 7x11 local window becomes a 1-D band |ds'| <= 83, so each 128-query
    tile only attends to 3 aligned 128-key chunks instead of 1024.
  - Exact per-chunk band is [128c-80, 128c+208): pairs at distance 81..83
    need |dj|=5 AND |di|>=1 simultaneously, which the 2-D mask forbids for
    the first/last 3 keys of a chunk, so halfwidth 80 suffices on the low
    side (and symmetrically 79+128 on the high side; 208 = 128+80).
  - Heads are padded to 64-partition slots (host-padded weights) so every
    engine access pattern starts at a 32-aligned partition.
  - All matmul operands are bf16; PSUM accumulation and softmax reductions
    stay fp32.
  - attnT[k, q] band tiles; exp on ScalarE (3 chunk-groups per head -> few,
    large activations); binary window mask applied multiplicatively on DVE;
    @V uses lhsT=[V|0|ones|0] so softmax denominators land at partitions
    64:112 of the same PSUM tile; the projection bias is folded in via a
    constant-1 row of aoT.
  - po is a single persistent 2-bank PSUM tile; q-tile t of every head uses
    column (t%2)*512+(t//2)*128 so concurrently-open accumulation groups
    alternate banks.  Closed tiles keep their data across same-bank
    start=True (only has_written bits are cleared), so the per-head divides
    run in 2 halves while the next head's @V already accumulates.
  - PSUM budget: pat 2x[128,1024] (4 banks) + proj 2x[128,512] (2 banks)
    + po [128,1024] (2 banks) = 8 banks exactly.
  - Evacuation copies (Q/K/V/out PSUM->SBUF) are split between ScalarE and
    DVE by static schedule tables; output stores go through the SP queue so
    DMA setup does not stall the Activation sequencer.
  - Software pipelining: attention(b) drains "filler" issue-units after each
    head -- sample b's own deferred Q/K pair-2/3 projection, proj(b+1)'s
    units, and out_proj(b-1)'s units (round-robin mixed) -- so the PE always
    has independent matmul work while the per-head softmax-divide chains
    (@V -> reciprocal -> multiply, serialized by the shared po tile) drain
    on DVE.  The last sample's final head divides in q-tile pairs so the
    closing output projection unblocks progressively.
"""

import sys

sys.path.insert(0, "/opt/trn_rl_repo")

import numpy as np

import concourse.bass as bass
from concourse import bacc
import concourse.mybir as mybir
import concourse.tile as tile
from concourse.bass_utils import run_bass_kernel_spmd

# ---------------------------------------------------------------- constants
B, S, C = 32, 1024, 384
H, D = 8, 48
HI, WI = 16, 64
N_CORES = 8
BL = B // N_CORES  # samples per core
SCALE = float(D) ** -0.5
F32 = mybir.dt.float32
BF16 = mybir.dt.bfloat16
PD = BF16  # precision of expT / m01 / vv

# s' = 16*j + i  <->  s = 64*i + j ;  PERM[s'] = s
_sp = np.arange(S)
PERM = (_sp % HI) * WI + (_sp // HI)

NQT = S // 128  # 8 query tiles (and key chunks)
WPADQ = 64 * H  # padded Q (and K) section width: 512
WQW = 2 * WPADQ + C  # 1408

# exact per-chunk bands: key-chunk c attends to queries [QLO[c], QHI[c])
QLO = [max(0, 128 * c - 80) for c in range(NQT)]
QHI = [min(S, 128 * c + 208) for c in range(NQT)]
WC = [QHI[c] - QLO[c] for c in range(NQT)]
OFFC = list(np.cumsum([0] + WC[:-1]))
BAND_W = sum(WC)  # 2144

# chunk groups per PSUM pat tile (each group width <= 1024 f32 = 2 banks)
CH_GROUPS = [(0, 1, 2), (3, 4, 5), (6, 7)]
GRP_BASE = [OFFC[g[0]] for g in CH_GROUPS]
GRP_W = [OFFC[g[-1]] + WC[g[-1]] - OFFC[g[0]] for g in CH_GROUPS]
assert max(GRP_W) <= 1024

# po column slot for q-tile t (identical for every head; open groups t,t+1
# always land in different PSUM banks)
PO_COL = [(t % 2) * 512 + (t // 2) * 128 for t in range(NQT)]

# ------------------------------------------------- engine schedule tables
# 'a' = ScalarE(Act) copy, 'd' = DVE tensor_copy
QK_EVAC = "aadaadaadaadaada"  # 16 per sample (Q then K, pair-major)
V_EVAC = "aadaadaa"  # 8 per sample
OUT_EVAC = "aaaaaaaa"  # 8 per sample
# 'd' = DVE, 'p' = Pool(GPSIMD) for the 3 mask multiplies of each head.
# g1 (chunks 3-5) sits mid-head where its latency hides; g0/g2 gate the
# @V start/tail chains, so they stay on the fast DVE.
# heads 5-7 push their mid/tail mask groups to Pool: it relieves DVE right
# before the end-of-sample divide burst
MASK_ENG = ["ddd"] * 5 + ["dpp"] * 3
# divide granularity: q-tile ranges; later ranges close later, and finer
# tail pieces release the po columns the next head's early @V chunks need.
DIV_RANGES = [(0, 8)]
# defer Q/K pairs 2-3 of proj(b) into attention(b)'s own filler list
DEFER_LATE = True

# ---------------------------------------------------------------- bass program
_CACHE = {}


def _build():
    if "nc" in _CACHE:
        return _CACHE["nc"]

    nc = bacc.Bacc(None, target_bir_lowering=False)
    xT_d = nc.declare_dram_parameter("xT", [BL, C, S], BF16, isOutput=False)
    wq_d = nc.declare_dram_parameter("wq_pad", [C, WQW], BF16, isOutput=False)
    wp_d = nc.declare_dram_parameter("wp_pad", [4, 128, C], BF16, isOutput=False)
    ones_d = nc.declare_dram_parameter("ones_row", [1, S], BF16, isOutput=False)
    m_d = nc.declare_dram_parameter("m01", [128, BAND_W], PD, isOutput=False)
    out_d = nc.declare_dram_parameter("out", [BL, S, C], F32, isOutput=True)

    with tile.TileContext(nc) as tc:
        with (
            tc.tile_pool(name="singles", bufs=1) as singles,
            tc.tile_pool(name="xt_pool", bufs=3) as xt_pool,
            tc.tile_pool(name="out_pool", bufs=6) as out_pool,
            tc.tile_pool(name="ps_proj", bufs=2, space="PSUM") as ps_proj,
            tc.tile_pool(name="ps_pat", bufs=2, space="PSUM") as ps_pat,
            tc.tile_pool(name="ps_po", bufs=1, space="PSUM") as ps_po,
        ):
            # ---- constants.  Q/K/V weight sections load as separate DMAs so
            # the first projection matmuls only wait for their own section.
            w_sb = singles.tile([128, 3, WQW], BF16)
            wq_v = wq_d.rearrange("(c p) w -> p c w", p=128)
            # Q pair-0 weights first (tiny transfer): the first projection
            # matmul waits only on this slice plus the first xt half
            nc.scalar.dma_start(w_sb[:, :, 0:128], wq_v[:, :, 0:128])
            nc.scalar.dma_start(w_sb[:, :, 128:WPADQ], wq_v[:, :, 128:WPADQ])
            nc.scalar.dma_start(
                w_sb[:, :, WPADQ : 2 * WPADQ], wq_v[:, :, WPADQ : 2 * WPADQ]
            )
            nc.scalar.dma_start(w_sb[:, :, 2 * WPADQ :], wq_v[:, :, 2 * WPADQ :])
            wp_sb = singles.tile([128, 4, C], BF16)
            m_sb = singles.tile([128, BAND_W], PD)

            def load_consts():
                # issued after proj(0) so these transfers cannot jump ahead
                # of the startup-critical xt/wQ DMAs on the DMA engines
                nc.sync.dma_start(m_sb, m_d[:, :])
                nc.sync.dma_start(wp_sb[:, :, :], wp_d.rearrange("f p c -> p f c"))

            # persistent attention-output PSUM tile (2 banks)
            po = ps_po.tile([128, S], F32, tag="po")
            _prj_n = [0]

            def prj_slot():
                _prj_n[0] += 1
                ps = ps_proj.tile([128, 512], F32, tag="mm", name=f"prj{_prj_n[0]}")
                return ps

            # ---- per-sample tiles, double-buffered for cross-sample overlap
            qTs, kTs, vvs, aoTs, expTs, dens = [], [], [], [], [], []
            for i in range(2):
                qTs.append(singles.tile([128, 4, S], BF16, name=f"qT{i}"))
                kTs.append(singles.tile([128, 4, S], BF16, name=f"kT{i}"))
                vvs.append(singles.tile([128, NQT, H, 128], PD, name=f"vv{i}"))
                aoTs.append(singles.tile([128, 4, S], BF16, name=f"aoT{i}"))
                expTs.append(singles.tile([128, BAND_W], PD, name=f"expT{i}"))
                dens.append(singles.tile([48, 2, 512], F32, name=f"den{i}"))
            def setup_set(i):
                """One-time constant sections of buffer set i (Pool engine).
                Set 1 is deferred until after proj(0) is issued so sample-0
                mask multiplies are not queued behind 20us of memsets."""
                vv, aoT = vvs[i], aoTs[i]
                nc.gpsimd.memset(vv[:, :, :, D : D + 16], 0.0)
                nc.gpsimd.memset(vv[:, :, :, D + 16 : 112], 1.0)
                nc.gpsimd.memset(vv[:, :, :, 112:128], 0.0)
                # zero dead rows (48:64, 112:128); starts must be 32-aligned so
                # cover 32:64 / 96:128 — live rows are rewritten by the divides.
                nc.gpsimd.memset(aoT[32:64, :, :], 0.0)
                nc.gpsimd.memset(aoT[96:128, :, :], 0.0)
                # constant-1 row: proj picks up b_proj from wp_pad[0][48]
                nc.gpsimd.dma_start(aoT[48:49, 0, :], ones_d[:, :])

            def evac(engine, dst, src):
                if engine == "a":
                    nc.scalar.copy(dst, src)
                else:
                    nc.vector.tensor_copy(dst, src)

            def load_xt(b, split=False):
                xt = xt_pool.tile([128, 3, S], BF16)
                src = xT_d[b].rearrange("(c p) s -> p c s", p=128)
                if split:
                    # startup: two sync-queue halves so the first projection
                    # matmuls only wait for the first half
                    nc.sync.dma_start(xt[:, :, 0:512], src[:, :, 0:512])
                    nc.sync.dma_start(xt[:, :, 512:S], src[:, :, 512:S])
                else:
                    nc.sync.dma_start(xt[:, :, :], src)
                return xt

            def _qk_mm(xt, qk, pair, half, dstps):
                ncol = qk * WPADQ + pair * 128
                for ci in range(3):
                    nc.tensor.matmul(
                        dstps,
                        w_sb[:, ci, ncol : ncol + 128],
                        xt[:, ci, half * 512 : (half + 1) * 512],
                        start=(ci == 0),
                        stop=(ci == 2),
                    )

            def _v_mm(xt, st, dstps):
                for ci in range(3):
                    nc.tensor.matmul(
                        dstps,
                        xt[:, ci, st * 128 : (st + 1) * 128],
                        w_sb[:, ci, 2 * WPADQ : 2 * WPADQ + C],
                        start=(ci == 0),
                        stop=(ci == 2),
                    )

            def proj_units(b, xt):
                """QKV projection for sample b as narrow issue-units over the
                two rotating psP halves."""
                qT, kT, vv = qTs[b % 2], kTs[b % 2], vvs[b % 2]

                def qk_unit(qk, pair, half, eng):
                    def u():
                        dst = qT if qk == 0 else kT
                        ps = prj_slot()
                        _qk_mm(xt, qk, pair, half, ps)
                        evac(
                            eng,
                            dst[:, pair, half * 512 : (half + 1) * 512],
                            ps,
                        )

                    return u

                def v_unit(st, eng):
                    def u():
                        ps = prj_slot()
                        _v_mm(xt, st, ps[:, 0:C])
                        evac(
                            eng,
                            vv[:, st, :, 0:D],
                            ps[:, 0:C].rearrange("p (h d) -> p h d", h=H),
                        )

                    return u

                ei = iter(QK_EVAC)
                qkp = {}
                for qk in range(2):
                    for pair in range(4):
                        for half in range(2):
                            qkp[(qk, pair, half)] = qk_unit(qk, pair, half, next(ei))
                vi = iter(V_EVAC)
                v_units = [v_unit(st, next(vi)) for st in range(NQT)]
                # early: Q/K pairs 0-1 + all V (needed by heads 0-3 and @V);
                # late: Q/K pairs 2-3, deferrable into the next attention's
                # own filler list (pair 2 first used by head 4).
                early, late = [], []
                for (qk, pair, half), u in qkp.items():
                    (early if pair < 2 else late).append(u)
                early.extend(v_units)
                return early, late

            def attention(b, fillers=(), tail_div=False, must=()):
                """Per-head attention; after each head, drain a few filler
                units (next sample's projection / previous sample's output
                projection) so PE has work during the divide chains.
                `must` units carry this sample's own deferred Q/K pair-2/3
                projection: they MUST all be issued by the end of head 2,
                before head 4 issues reads of those qT/kT sections (the tile
                framework resolves dependencies in issue order).
                tail_div: the last head's divide runs in q-tile pairs so the
                final output projection unblocks progressively."""
                fillers = list(must) + list(fillers)
                n_must = len(must)
                fi = 0
                qT, kT, vv, aoT = qTs[b % 2], kTs[b % 2], vvs[b % 2], aoTs[b % 2]
                for h in range(H):
                    div_ranges = (
                        [(0, 2), (2, 4), (4, 6), (6, 8)]
                        if (tail_div and h == H - 1)
                        else DIV_RANGES
                    )
                    pair, sub = divmod(h, 2)
                    p0 = sub * 64
                    expT = expTs[h % 2]
                    den_sb = dens[h % 2]
                    # ---- QK^T band + exp (3 chunk-groups); masks for g0/g1
                    # issue here, g2's mask issues later so DVE's in-order
                    # queue does not park the divides behind it.
                    def mask_mul(gi):
                        gbase, gw = GRP_BASE[gi], GRP_W[gi]
                        tt = (
                            nc.vector.tensor_tensor
                            if MASK_ENG[h][gi] == "d"
                            else nc.gpsimd.tensor_tensor
                        )
                        tt(
                            expT[:, gbase : gbase + gw],
                            expT[:, gbase : gbase + gw],
                            m_sb[:, gbase : gbase + gw],
                            mybir.AluOpType.mult,
                        )

                    for gi, grp in enumerate(CH_GROUPS):
                        gbase, gw = GRP_BASE[gi], GRP_W[gi]
                        pat = ps_pat.tile([128, 1024], F32, tag="attn")
                        for c in grp:
                            lo = OFFC[c] - gbase
                            hi = lo + WC[c]
                            a = lo
                            while a < hi:
                                b2 = min(hi, (a // 512 + 1) * 512)
                                nc.tensor.matmul(
                                    pat[:, a:b2],
                                    kT[p0 : p0 + D, pair, c * 128 : (c + 1) * 128],
                                    qT[
                                        p0 : p0 + D,
                                        pair,
                                        QLO[c] + (a - lo) : QLO[c] + (b2 - lo),
                                    ],
                                    start=True,
                                    stop=True,
                                )
                                a = b2
                        nc.scalar.activation(
                            expT[:, gbase : gbase + gw],
                            pat[:, 0:gw],
                            mybir.ActivationFunctionType.Exp,
                            scale=SCALE,
                        )
                        if gi < 2:
                            mask_mul(gi)

                    # ---- @V with ones rows at 64:112 -> denominators,
                    # interleaved with the per-range normalizes so divides
                    # start the moment their q-tiles close.
                    po_q = po[:, :].rearrange(
                        "p (tlo thi u) -> p thi tlo u", tlo=2, thi=4, u=128
                    )
                    den_flat = den_sb[:, :, :].rearrange("p a b -> p (a b)")

                    def av_chunk(c):
                        lhsT = vv[:, c, h, :]
                        for t in range(max(c - 1, 0), min(c + 2, NQT)):
                            pc = PO_COL[t]
                            qs = max(128 * t, QLO[c])
                            qe = min(128 * t + 128, QHI[c])
                            nc.tensor.matmul(
                                po[:, pc + (qs - 128 * t) : pc + (qe - 128 * t)],
                                lhsT,
                                expT[:, OFFC[c] + (qs - QLO[c]) : OFFC[c] + (qe - QLO[c])],
                                start=(c == max(t - 1, 0)),
                                stop=(c == min(t + 1, NQT - 1)),
                            )

                    def divide(ta, tb):
                        nt = (tb - ta) // 2
                        den_v = den_flat[:, ta * 128 : tb * 128].rearrange(
                            "p (thi tlo u) -> p thi tlo u", thi=nt, tlo=2, u=128
                        )
                        ao_v = aoT[
                            p0 : p0 + D, pair, ta * 128 : tb * 128
                        ].rearrange("p (thi tlo u) -> p thi tlo u", thi=nt, tlo=2, u=128)
                        nc.vector.reciprocal(
                            den_v, po_q[64 : 64 + D, ta // 2 : tb // 2]
                        )
                        nc.vector.tensor_tensor(
                            ao_v,
                            po_q[0:D, ta // 2 : tb // 2],
                            den_v,
                            mybir.AluOpType.mult,
                        )

                    g2_start = CH_GROUPS[2][0]
                    for c in range(NQT):
                        if c == g2_start:
                            mask_mul(2)  # just before @V enters group 2
                        av_chunk(c)
                        for ta, tb in div_ranges:
                            # tile tb-1 stops at chunk min(tb, NQT-1)
                            if min(tb, NQT - 1) == c:
                                divide(ta, tb)
                    # drain filler units evenly across heads.  The deferred
                    # own-projection units sit at the head of the list: the
                    # drain targets guarantee they are issued by the end of
                    # head 2, before head 4 issues reads of qT/kT pairs 2-3
                    # (the tile framework resolves deps in issue order).
                    if len(fillers) >= 20:
                        target = (h + 1) * len(fillers) // H
                    else:
                        # thin filler supply (last sample): save units for the
                        # late heads where the divide chains stall hardest
                        target = ((h + 1) * (h + 2) * len(fillers)) // (H * (H + 1))
                    if h == 3 and n_must > target:
                        target = n_must
                    while fi < target:
                        fillers[fi]()
                        fi += 1

            def _op_mm(aoT, st, dstps):
                for p in range(4):
                    nc.tensor.matmul(
                        dstps,
                        aoT[:, p, st * 128 : (st + 1) * 128],
                        wp_sb[:, p, :],
                        start=(p == 0),
                        stop=(p == 3),
                    )

            def out_proj_units(b):
                aoT = aoTs[b % 2]
                units = []

                def st_unit(st, eng):
                    def u():
                        ot = out_pool.tile([128, C], F32, tag="ot", name=f"ot{b}_{st}")
                        ps = prj_slot()
                        _op_mm(aoT, st, ps[:, 0:C])
                        evac(eng, ot[:, :], ps[:, 0:C])
                        nc.sync.dma_start(
                            out_d[b, st * 128 : (st + 1) * 128, :], ot[:, :]
                        )

                    return u

                oi = iter(OUT_EVAC)
                for st in range(NQT):
                    units.append(st_unit(st, next(oi)))
                return units

            def out_proj_tail(b):
                # final sample: narrow evacs on alternating engines so the
                # drain is not serialized on one engine or on psP; the last
                # store pair splits across two DMA queues so the closing
                # transfers overlap
                aoT = aoTs[b % 2]
                for sp in range(NQT // 2):
                    ot = out_pool.tile([128, 2, C], F32)
                    for k, eng in ((0, "a"), (1, "d")):
                        st = 2 * sp + k
                        ps = prj_slot()
                        _op_mm(aoT, st, ps[:, 0:C])
                        evac(eng, ot[:, k, :], ps[:, 0:C])
                        if sp == NQT // 2 - 1:
                            dma = nc.sync.dma_start if k == 0 else nc.scalar.dma_start
                            dma(out_d[b, st * 128 : (st + 1) * 128, :], ot[:, k, :])
                    if sp < NQT // 2 - 1:
                        nc.sync.dma_start(
                            out_d[b, 2 * sp * 128 : (2 * sp + 2) * 128, :].rearrange(
                                "(k p) c -> p k c", k=2
                            ),
                            ot[:, :, :],
                        )

            # ---------------- main pipeline: attention(b) runs interleaved
            # with filler units: the tail of sample b's own projection
            # (Q/K pairs 2-3, first used by head 4), proj(b+1)'s early
            # units, and out_proj(b-1).
            setup_set(0)
            xts = {0: load_xt(0, split=True)}
            early0, late0 = proj_units(0, xts[0])
            for u in early0:
                u()
            load_consts()
            setup_set(1)
            carry = late0 if DEFER_LATE else []
            if not DEFER_LATE:
                for u in late0:
                    u()
            for b in range(BL):
                must = list(carry)
                carry = []
                fillers = []
                if b + 1 < BL:
                    xts[b + 1] = load_xt(b + 1)  # prefetch during attention(b)
                    early, late = proj_units(b + 1, xts[b + 1])
                    fillers += early
                    if DEFER_LATE:
                        carry = late
                    else:
                        fillers += late
                if b > 0:
                    # interleave the previous sample's output-projection units
                    # among the projection units (round-robin) so each drain
                    # point mixes both kinds of work
                    ou = out_proj_units(b - 1)
                    mixed = []
                    k = max(1, len(fillers) // max(1, len(ou)))
                    oi2 = iter(ou)
                    for idx, u in enumerate(fillers):
                        mixed.append(u)
                        if idx % k == k - 1:
                            nu = next(oi2, None)
                            if nu is not None:
                                mixed.append(nu)
                    mixed.extend(oi2)
                    fillers = mixed
                attention(b, fillers, tail_div=(b == BL - 1), must=must)
            out_proj_tail(BL - 1)

    nc.finalize()
    _CACHE["nc"] = nc
    return nc


# ---------------------------------------------------------------- host wrapper
def _np_bf16(a):
    import ml_dtypes

    return np.asarray(a, dtype=ml_dtypes.bfloat16)


def _build_m01(mask):
    """[128, BAND_W] banded 0/1 mask in exact-band layout (rows = key within
    chunk c, cols = q in [QLO[c], QHI[c]))."""
    mp = np.asarray(mask)[np.ix_(PERM, PERM)]
    good = np.isfinite(mp) & (mp == 0.0)
    m01 = np.zeros((128, BAND_W), np.float32)
    covered = 0
    for c in range(NQT):
        blk = good[QLO[c] : QHI[c], c * 128 : (c + 1) * 128]  # [q, k]
        m01[:, OFFC[c] : OFFC[c] + WC[c]] = blk.T.astype(np.float32)
        covered += int(blk.sum())
    assert covered == int(good.sum()), "mask not covered by band layout"
    return m01


def _pad_wqkv(w_qkv):
    """[384, 1152] -> [384, 1408]: Q/K head h at cols h*64..h*64+48 (zero pad),
    V kept natural at cols 1024:1408."""
    out = np.zeros((C, WQW), np.float32)
    for sec in range(2):  # Q, K
        for h in range(H):
            out[:, sec * WPADQ + h * 64 : sec * WPADQ + h * 64 + D] = w_qkv[
                :, sec * C + h * D : sec * C + (h + 1) * D
            ]
    out[:, 2 * WPADQ :] = w_qkv[:, 2 * C :]
    return out


def _pad_wproj(w_proj, b_proj):
    """[384, 384] -> [4, 128, 384]: pair p rows 0:48 = head 2p, 64:112 = head 2p+1.
    Row 48 of pair 0 carries b_proj (matched by the constant-1 row in aoT)."""
    out = np.zeros((4, 128, C), np.float32)
    for p in range(4):
        out[p, 0:D] = w_proj[(2 * p) * D : (2 * p + 1) * D]
        out[p, 64 : 64 + D] = w_proj[(2 * p + 1) * D : (2 * p + 2) * D]
    out[0, D] = b_proj
    return out


def kernel(x, w_qkv, w_proj, b_proj, mask):
    x = np.asarray(x, np.float32)
    w_qkv = np.asarray(w_qkv, np.float32)
    w_proj = np.asarray(w_proj, np.float32)
    b_proj = np.asarray(b_proj, np.float32)

    nc = _build()

    xT = _np_bf16(np.ascontiguousarray(x[:, PERM, :].transpose(0, 2, 1)))  # [B, C, S']
    wq_pad = _np_bf16(_pad_wqkv(w_qkv))
    wp_pad = _np_bf16(_pad_wproj(w_proj, b_proj))
    ones_row = _np_bf16(np.ones((1, S), np.float32))
    m01 = _build_m01(mask)
    if PD == BF16:
        m01 = _np_bf16(m01)

    in_maps = [
        {
            "xT": xT[c * BL : (c + 1) * BL],
            "wq_pad": wq_pad,
            "wp_pad": wp_pad,
            "ones_row": ones_row,
            "m01": m01,
        }
        for c in range(N_CORES)
    ]
    res = run_bass_kernel_spmd(nc, in_maps, list(range(N_CORES)))
    out_p = np.concatenate([res.results[c]["out"] for c in range(N_CORES)], axis=0)
    out = np.empty_like(out_p)
    out[:, PERM, :] = out_p
    return out
